# revision 19
# baseline (speedup 1.0000x reference)
"""CrystalGraphConvNet forward pass as a distributed Bass/Tile kernel on 8 TRN2
NeuronCores.

Strategy (graph/data parallel, per sharding hint):
  - Atoms sharded contiguously across 8 cores (7500 each, padded to 7552).
  - Per conv layer, each core computes Y_self = x @ Wf[:F], Y_nbr = x @ Wf[F:2F]
    for its atom shard; Y_nbr shards are AllGathered into a replicated f16
    table viewed as PAIR rows (two atoms = 512 B per row, plus a leading zero
    row), so a single int16-indexed dma_gather per tile fetches both parity
    candidates of every edge in one 512 B packet (row index = 1 + j//2).
  - The gather runs un-transposed (contiguous SBUF writes, edge-major):
    ge[p, i, 0:128] / [128:256] hold the even/odd atom of edge i*128+p.  A
    single copy_predicated with a tiny resident per-edge parity mask
    (broadcast along channels) selects the right atom; 12 accumulating
    transpose-matmuls against an f16 identity then fold the selected rows
    into the channel-major PSUM accumulator on the TensorEngine, on top of
    the nbr_fea projection and the one-hot self term.
  - Training-mode batchnorm needs global stats, so gated values are staged to
    DRAM scratch in f16 while per-channel sum/sumsq accumulate (scalar-engine
    copy/square with accum); a tiny AllReduce yields the affine.  The second
    pass is a SINGLE pass per tile pair: sigmoid is computed as
    1/(1+exp(-a)) with a DVE reciprocal, so every activation in the program
    lives in the one exp/ln table set and no ACT table reloads occur.
  - Crystal mean-pooling is one accumulating one-hot matmul per tile into a
    512-crystal window, scattered by int32 indirect DMA into a global crystal
    array and AllReduced; the tiny MLP head runs redundantly on every core.
"""

import math
import os
import numpy as np

import concourse.bass as bass
import concourse.bacc as bacc
import concourse.tile as tile
from concourse import mybir
from contextlib import ExitStack


def _single_act_table(orig):
    """Route every exp/ln/copy/square/identity activation to the one ACT
    table set that contains BOTH exp and ln.  The default chooser assigns
    exp->exp_and_others and ln->natural_log, which thrashes table reloads
    (~1.3us each) on every exp<->ln transition in the batchnorm second pass.
    Removing those funcs from every other set leaves the chooser exactly one
    legal assignment; set ids keep their act_info.json indices so walrus
    loads the right tables."""
    AF = mybir.ActivationFunctionType
    both = {AF.Exp, AF.Ln}
    shared = {AF.Exp, AF.Ln, AF.Copy, AF.Square, AF.Identity}
    home = None
    for name, funcs in orig.items():
        if both <= funcs:
            home = name
            break
    if home is None:
        return orig
    out = {}
    for name, funcs in orig.items():
        out[name] = funcs if name == home else (funcs - shared)
    return out


_orig_get_activation_tables = bacc.get_activation_tables


def _patched_get_activation_tables(arch):
    return _single_act_table(_orig_get_activation_tables(arch))


bacc.get_activation_tables = _patched_get_activation_tables

F16 = mybir.dt.float16
F32 = mybir.dt.float32
I8 = mybir.dt.int8
I16 = mybir.dt.int16
I32 = mybir.dt.int32


def make_cfg(N=60000, M=12, F0=92, FB=41, F=64, H=128, NC=2000, NCONV=3,
             EPS=1e-5, NCORES=8, TA=128):
    c = dict(N=N, M=M, F0=F0, FB=FB, F=F, H=H, NC=NC, NCONV=NCONV, EPS=EPS,
             NCORES=NCORES, TA=TA)
    assert N % NCORES == 0
    c["A_shard"] = N // NCORES
    c["ntile"] = (c["A_shard"] + TA - 1) // TA
    c["A_pad"] = c["ntile"] * TA
    c["E_tile"] = TA * M
    c["E_loc"] = c["ntile"] * c["E_tile"]
    assert (NCORES * c["A_pad"]) % 2 == 0
    c["R2"] = 1 + NCORES * c["A_pad"] // 2      # pair-table rows (zero row at 0)
    assert c["R2"] <= 32768, "pair table must stay int16-addressable"
    c["W16"] = c["E_tile"] // 16
    c["NCB"] = 512 * ((NC + 511) // 512) + 512  # crystal bounce rows
    c["G"] = 2 * F                              # gated channels
    return c


CFG = make_cfg()


# --------------------------------------------------------------------------
# program builder
# --------------------------------------------------------------------------

def build_program(c, debug=False, dbg_dump=False, stop=None):
    # stop: optional (layer, stage) tuple for bisection; stages within a layer:
    # 0=Y, 1=AG, 2=A, 3=AR1, 4=B, 5=AR2, 6=C; (L,0)=pool, (L,1)=head
    nc = bacc.Bacc("TRN2", target_bir_lowering=False, debug=debug,
                   num_devices=c["NCORES"], num_swdge_queues=4)

    N, M, F0, FB, F, H, NC, L = (c["N"], c["M"], c["F0"], c["FB"], c["F"],
                                 c["H"], c["NC"], c["NCONV"])
    G, TA, NT, AP_, ET, EL = (c["G"], c["TA"], c["ntile"], c["A_pad"],
                              c["E_tile"], c["E_loc"])
    R2, W16, NCB, EPS = c["R2"], c["W16"], c["NCB"], c["EPS"]
    NPAIR = (NT + 1) // 2
    NCHUNK = ET // TA                           # 128-edge chunks per tile

    # ---------------- inputs ----------------
    afT = nc.dram_tensor("afT", [F0, AP_], F16, kind="ExternalInput")
    idxw = nc.dram_tensor("idxw", [128, NT, W16], I16, kind="ExternalInput")
    mparr = nc.dram_tensor("mparr", [128, NT, M], I8, kind="ExternalInput")
    nbrT = nc.dram_tensor("nbrT", [FB, EL], F16, kind="ExternalInput")
    oh_self = nc.dram_tensor("oh_self", [128, ET], F16, kind="ExternalInput")
    pone = nc.dram_tensor("pone", [NT, 128, 512], F16, kind="ExternalInput")
    scidx = nc.dram_tensor("scidx", [128, 32], I16, kind="ExternalInput")
    invcnt = nc.dram_tensor("invcnt", [NCB, 1], F32, kind="ExternalInput")
    w_emb = nc.dram_tensor("w_emb", [F0, F], F16, kind="ExternalInput")
    b_emb = nc.dram_tensor("b_emb", [F, 1], F32, kind="ExternalInput")
    w_self = nc.dram_tensor("w_self", [L, F, G], F16, kind="ExternalInput")
    w_nbr = nc.dram_tensor("w_nbr", [L, F, G], F16, kind="ExternalInput")
    w_b = nc.dram_tensor("w_b", [L, FB, G], F16, kind="ExternalInput")
    g1 = nc.dram_tensor("g1", [L, G, 1], F32, kind="ExternalInput")
    be1 = nc.dram_tensor("be1", [L, G, 1], F32, kind="ExternalInput")
    g2 = nc.dram_tensor("g2", [L, F, 1], F32, kind="ExternalInput")
    be2 = nc.dram_tensor("be2", [L, F, 1], F32, kind="ExternalInput")
    w_fc = nc.dram_tensor("w_fc", [F, H], F16, kind="ExternalInput")
    b_fc = nc.dram_tensor("b_fc", [H, 1], F32, kind="ExternalInput")
    w_out = nc.dram_tensor("w_out", [H, 1], F16, kind="ExternalInput")
    b_out = nc.dram_tensor("b_out", [1, 1], F32, kind="ExternalInput")
    ident = nc.dram_tensor("ident", [128, 128], F32, kind="ExternalInput")
    identh = nc.dram_tensor("identh", [128, 128], F16, kind="ExternalInput")

    out_t = nc.dram_tensor("out", [1, NCB], F32, kind="ExternalOutput")
    if dbg_dump:
        L_, F_, G_, AP2, EL_ = c["NCONV"], c["F"], c["G"], c["A_pad"], c["E_loc"]
        dbgx0 = nc.dram_tensor("dbgx0", [F_, AP2], F32, kind="ExternalOutput")
        dbgx = nc.dram_tensor("dbgx", [L_, F_, AP2], F32, kind="ExternalOutput")
        dbgsum = nc.dram_tensor("dbgsum", [L_, F_, AP2], F32, kind="ExternalOutput")
        dbgst1 = nc.dram_tensor("dbgst1", [L_, G_, 2], F32, kind="ExternalOutput")
        dbggat = nc.dram_tensor("dbggat", [L_, NT, 128, ET], F16,
                                kind="ExternalOutput")
        dbgpool = nc.dram_tensor("dbgpool", [NCB, F_], F32, kind="ExternalOutput")

    # ---------------- internal DRAM ----------------
    yb = nc.dram_tensor("yb", [AP_, G], F16)                        # AG input bounce
    tbl = nc.dram_tensor("tbl", [R2, 2 * G], F16, addr_space="Shared")
    scr = nc.dram_tensor("scr", [NT, 128, ET], F16)
    ar1_in = nc.dram_tensor("ar1_in", [G, 2], F32)
    ar1_out = nc.dram_tensor("ar1_out", [G, 2], F32, addr_space="Shared")
    ar2_in = nc.dram_tensor("ar2_in", [F, 2], F32)
    ar2_out = nc.dram_tensor("ar2_out", [F, 2], F32, addr_space="Shared")
    pool_in = nc.dram_tensor("pool_in", [NCB, F], F32)
    pool_out = nc.dram_tensor("pool_out", [NCB, F], F32, addr_space="Shared")

    rg = [list(range(c["NCORES"]))]
    AF = mybir.ActivationFunctionType
    OP = mybir.AluOpType
    if stop is None:
        stop = (L, 9)

    # Tile cycles DMASW lane sems per Pool DMA in GLOBAL program order (%8);
    # each lane sem is queue-locked, so the SWDGE queue must follow the same
    # global counter (%4), not the per-layer tile index.
    gq = [0]

    def gather_q():
        q = gq[0] % 4
        gq[0] += 1
        return q

    def live(key):
        return key <= stop

    with tile.TileContext(nc) as tc, ExitStack() as top:
        # Tile assigns each Pool-engine DMA inst a DMASW lane (round-robin in
        # program order) and points consumer waits at that lane's semaphore.
        # A prepare_only gather bakes its completion sem into the descriptors
        # (pass 2 attaches no then_inc), so sem= MUST be the very lane sem the
        # tick pass will pick, tracked here with a mirror counter.
        swsems = tc.sems.swdge_block()
        pool_dma_lane = [0]

        def prep_slot():
            """(lane sem, queue) for the next prepared Pool DMA.  queue =
            lane % nqueues keeps every lane sem on one fixed queue (SWDGE
            sems are queue-locked)."""
            j = pool_dma_lane[0]
            pool_dma_lane[0] += 1
            return swsems[j % len(swsems)], (j % len(swsems)) % 4

        # persistent SBUF state
        x_cm = nc.alloc_sbuf_tensor("x_cm", [F, AP_], F32)
        summed = nc.alloc_sbuf_tensor("summed", [F, AP_], F16)
        ysr = nc.alloc_sbuf_tensor("ysr", [128, NT, G], F16)      # Y_self row-major
        idx_all = nc.alloc_sbuf_tensor("idx_all", [128, NT, W16], I16)
        mpar = nc.alloc_sbuf_tensor("mpar", [128, NT, M, 1], I8)

        const = top.enter_context(tc.tile_pool(name="const", bufs=1))
        stats = top.enter_context(tc.tile_pool(name="stats", bufs=1))

        # constants resident all kernel
        ohs_t = const.tile([128, ET], F16)
        nc.sync.dma_start(out=ohs_t[:], in_=oh_self[:, :])
        wemb_t = const.tile([F0, F], F16)
        nc.sync.dma_start(out=wemb_t[:], in_=w_emb[:, :])
        bemb_t = const.tile([F, 1], F32)
        nc.sync.dma_start(out=bemb_t[:], in_=b_emb[:, :])
        id_t = const.tile([128, 128], F32)
        nc.sync.dma_start(out=id_t[:], in_=ident[:, :])
        idh_t = const.tile([128, 128], F16)
        nc.sync.dma_start(out=idh_t[:], in_=identh[:, :])
        eps_t = const.tile([128, 1], F32)
        nc.vector.memset(eps_t[:], EPS)
        zrow = const.tile([1, 2 * G], F16)
        nc.vector.memset(zrow[:], 0.0)
        zt128 = const.tile([128, F], F32)
        nc.vector.memset(zt128[:], 0.0)

        # layer-invariant gather indices and parity masks
        nc.sync.dma_start(out=idx_all[:, :, :], in_=idxw[:, :, :])
        nc.sync.dma_start(out=mpar[:, :, :, 0], in_=mparr[:, :, :])

        wS = []
        wN = []
        wB = []
        for l in range(L):
            t1 = const.tile([F, G], F16, tag=f"wS{l}")
            nc.sync.dma_start(out=t1[:], in_=w_self[l, :, :])
            t2 = const.tile([F, G], F16, tag=f"wN{l}")
            nc.sync.dma_start(out=t2[:], in_=w_nbr[l, :, :])
            t3 = const.tile([FB, G], F16, tag=f"wB{l}")
            nc.sync.dma_start(out=t3[:], in_=w_b[l, :, :])
            wS.append(t1)
            wN.append(t2)
            wB.append(t3)

        # stats buffers
        st1_s = stats.tile([G, NT], F32, tag="st1s")
        st1_q = stats.tile([G, NT], F32, tag="st1q")
        st2_s = stats.tile([F, 2 * NPAIR], F32, tag="st2s")
        st2_q = stats.tile([F, 2 * NPAIR], F32, tag="st2q")

        # zero table guard row + summed pads
        nc.sync.dma_start(out=tbl[0:1, :], in_=zrow[:])
        nc.vector.memset(summed[:, :], 0.0)

        # ---------------- embedding: x = atom_fea @ W_emb + b_emb ----------
        with tc.tile_pool(name="emb", bufs=3) as embp, \
             tc.tile_pool(name="embps", bufs=2, space="PSUM") as embps:
            CH = 512
            for j in range(0, AP_, CH):
                w = min(CH, AP_ - j)
                rhs = embp.tile([F0, CH], F16, tag="embr")
                nc.sync.dma_start(out=rhs[:, :w], in_=afT[:, j:j + w])
                ps = embps.tile([F, CH], F32, tag="embp")
                nc.tensor.matmul(ps[:, :w], lhsT=wemb_t[:], rhs=rhs[:, :w],
                                 start=True, stop=True)
                nc.scalar.activation(out=x_cm[:, j:j + w], in_=ps[:, :w],
                                     func=AF.Identity, bias=bemb_t[:], scale=1.0)
        if dbg_dump:
            nc.sync.dma_start(out=dbgx0[:, :], in_=x_cm[:, :])

        # ---------------- conv layers ----------------
        for l in range(L):
            if not live((l, 0)):
                break
            # ---- phase Y: Y_self (SBUF) / Y_nbr (-> bounce -> AllGather) ----
            with tc.tile_pool(name="yph", bufs=3) as yp, \
                 tc.tile_pool(name="yps", bufs=2, space="PSUM") as yps:
                lastreal = c["A_shard"] - (NT - 1) * TA
                for t in range(NT):
                    xa = yp.tile([F, TA], F16, tag="xa")
                    nc.scalar.activation(out=xa[:], in_=x_cm[:, t * TA:(t + 1) * TA],
                                         func=AF.Copy)
                    psS = yps.tile([TA, G], F32, tag="psS")
                    nc.tensor.matmul(psS[:], lhsT=xa[:], rhs=wS[l][:],
                                     start=True, stop=True)
                    # pad atoms of the last tile must contribute exactly zero
                    # through the self one-hot matmul
                    nreal = TA if t < NT - 1 else lastreal
                    if nreal < TA:
                        nc.vector.memset(ysr[:, t, :], 0.0)
                    nc.scalar.activation(out=ysr[0:nreal, t, :],
                                         in_=psS[0:nreal, :], func=AF.Copy)
                    psN = yps.tile([TA, G], F32, tag="psN")
                    nc.tensor.matmul(psN[:], lhsT=xa[:], rhs=wN[l][:],
                                     start=True, stop=True)
                    yn = yp.tile([TA, G], F16, tag="yn")
                    nc.scalar.activation(out=yn[:], in_=psN[:], func=AF.Copy)
                    nc.sync.dma_start(out=yb[t * TA:(t + 1) * TA, :], in_=yn[:])

            if not live((l, 1)):
                break
            tc.strict_bb_all_engine_barrier()
            nc.gpsimd.collective_compute(
                "AllGather", OP.bypass, replica_groups=rg,
                ins=[yb[:, :]], outs=[tbl[1:R2, :]])
            tc.strict_bb_all_engine_barrier()

            if not live((l, 2)):
                break
            # ---- pass A: edges -> gated scratch + stats1 ----
            with tc.tile_pool(name="pa", bufs=6) as pa, \
                 tc.tile_pool(name="paps", bufs=2, space="PSUM") as paps:
                for t in range(NT):
                    nbt = pa.tile([FB, ET], F16, tag="nbt")
                    nc.sync.dma_start(out=nbt[:], in_=nbrT[:, t * ET:(t + 1) * ET])

                    ge = pa.tile([128, NCHUNK, 2 * G], F16, tag="ge")
                    # prepare_only decouples Q7 desc-gen from the transfer:
                    # the gather DMA fires at trigger time, so transfers on
                    # the 4 SWDGE queues overlap each other and compute
                    # instead of serializing on the Pool engine.
                    nc.gpsimd.dma_gather(ge[:], tbl[:, :], idx_all[:, t, :],
                                         ET, ET, 2 * G, single_packet=False,
                                         queue_num=gather_q())
                    # parity select: overwrite even-atom slab with odd-atom
                    # slab wherever the edge's target index is odd
                    nc.vector.copy_predicated(
                        out=ge[:, :, 0:G],
                        mask=mpar[:, t, :, :].broadcast_to([128, NCHUNK, G]),
                        data=ge[:, :, G:2 * G])

                    # every 128-col region: chunk transpose (start) -> wB ->
                    # one-hot self term (stop)
                    ps = paps.tile([G, ET], F32, tag="aps")
                    for i in range(NCHUNK):
                        cs = slice(i * 128, (i + 1) * 128)
                        nc.tensor.matmul(ps[:, cs], lhsT=ge[:, i, 0:G],
                                         rhs=idh_t[:], start=(i % 4 == 0),
                                         stop=False)
                    for s in range(ET // 512):
                        sl = slice(s * 512, (s + 1) * 512)
                        nc.tensor.matmul(ps[:, sl], lhsT=wB[l][:], rhs=nbt[:, sl],
                                         start=False, stop=False)
                        nc.tensor.matmul(ps[:, sl], lhsT=ysr[:, t, :],
                                         rhs=ohs_t[:, sl], start=False, stop=True)

                    gat = pa.tile([128, ET], F16, tag="gat")
                    nc.scalar.activation(out=gat[:], in_=ps[:], func=AF.Copy,
                                         accum_out=st1_s[:, t:t + 1])
                    sqd = pa.tile([128, ET], F16, tag="sqd")
                    nc.scalar.activation(out=sqd[:], in_=ps[:], func=AF.Square,
                                         accum_out=st1_q[:, t:t + 1])
                    nc.sync.dma_start(out=scr[t, :, :], in_=gat[:])

            if not live((l, 3)):
                break
            # ---- stats1 reduce + AllReduce + affine ----
            with tc.tile_pool(name="s1", bufs=1) as s1p:
                pack1 = s1p.tile([G, 2], F32, tag="pack1")
                nc.vector.tensor_reduce(out=pack1[:, 0:1], in_=st1_s[:],
                                        axis=mybir.AxisListType.X, op=OP.add)
                nc.vector.tensor_reduce(out=pack1[:, 1:2], in_=st1_q[:],
                                        axis=mybir.AxisListType.X, op=OP.add)
                nc.sync.dma_start(out=ar1_in[:, :], in_=pack1[:])
                tc.strict_bb_all_engine_barrier()
                nc.gpsimd.collective_compute(
                    "AllReduce", OP.add, replica_groups=rg,
                    ins=[ar1_in[:, :]], outs=[ar1_out[:, :]])
                tc.strict_bb_all_engine_barrier()

                red1 = s1p.tile([G, 2], F32, tag="red1")
                nc.sync.dma_start(out=red1[:], in_=ar1_out[:, :])
                if dbg_dump:
                    nc.sync.dma_start(out=dbgst1[l, :, :], in_=red1[:])
                    for t in range(NT):
                        nc.gpsimd.dma_start(out=dbggat[l, t, :, :],
                                            in_=scr[t, :, :])
                g1_t = s1p.tile([G, 1], F32, tag="g1t")
                nc.sync.dma_start(out=g1_t[:], in_=g1[l, :, :])
                be1_t = s1p.tile([G, 1], F32, tag="be1t")
                nc.sync.dma_start(out=be1_t[:], in_=be1[l, :, :])

                # rsqrt(var+eps) = exp(-0.5*ln(var+eps)) + one Newton step
                # (no sqrt table needed; Ln/Exp share one ACT table set)
                invE = 1.0 / float(N * M)
                mmt = s1p.tile([G, 8], F32, tag="mmt")
                mcol = mmt[:, 0:1]
                nc.vector.tensor_scalar(out=mcol, in0=red1[:, 0:1], scalar1=invE,
                                        scalar2=None, op0=OP.mult)
                ex2 = mmt[:, 1:2]
                nc.vector.tensor_scalar(out=ex2, in0=red1[:, 1:2], scalar1=invE,
                                        scalar2=None, op0=OP.mult)
                msq = mmt[:, 2:3]
                nc.vector.tensor_tensor(out=msq, in0=mcol, in1=mcol, op=OP.mult)
                var = mmt[:, 3:4]
                nc.vector.tensor_tensor(out=var, in0=ex2, in1=msq, op=OP.subtract)
                lv = mmt[:, 4:5]
                nc.scalar.activation(out=lv, in_=var, func=AF.Ln,
                                     bias=eps_t[0:G, :], scale=1.0)
                r0 = mmt[:, 5:6]
                nc.scalar.activation(out=r0, in_=lv, func=AF.Exp, scale=-0.5)
                # one Newton step: r1 = r0*(1.5 - 0.5*(var+eps)*r0^2)
                vpe = mmt[:, 6:7]
                nc.vector.tensor_scalar(out=vpe, in0=var, scalar1=eps_t[0:G, :],
                                        scalar2=0.5, op0=OP.add, op1=OP.mult)
                r0q = mmt[:, 7:8]
                nc.vector.tensor_tensor(out=r0q, in0=r0, in1=r0, op=OP.mult)
                nc.vector.tensor_tensor(out=r0q, in0=r0q, in1=vpe, op=OP.mult)
                nc.vector.tensor_scalar(out=r0q, in0=r0q, scalar1=-1.0,
                                        scalar2=1.5, op0=OP.mult, op1=OP.add)
                r1 = mmt[:, 6:7]
                nc.vector.tensor_tensor(out=r1, in0=r0, in1=r0q, op=OP.mult)

                s1c_ = s1p.tile([G, 1], F32, tag="s1c")
                nc.vector.tensor_tensor(out=s1c_[:], in0=g1_t[:], in1=r1,
                                        op=OP.mult)
                t1c_ = s1p.tile([G, 1], F32, tag="t1c")
                nc.vector.tensor_tensor(out=t1c_[:], in0=mcol, in1=s1c_[:],
                                        op=OP.mult)
                nc.vector.scalar_tensor_tensor(out=t1c_[:], in0=t1c_[:],
                                               scalar=-1.0, in1=be1_t[:],
                                               op0=OP.mult, op1=OP.add)
                # negated F-half affine (sigmoid via 1/(1+exp(-a)))
                s1n = s1p.tile([G, 1], F32, tag="s1n")
                nc.vector.tensor_scalar(out=s1n[:], in0=s1c_[:], scalar1=-1.0,
                                        scalar2=None, op0=OP.mult)
                t1n = s1p.tile([G, 1], F32, tag="t1n")
                nc.vector.tensor_scalar(out=t1n[:], in0=t1c_[:], scalar1=-1.0,
                                        scalar2=None, op0=OP.mult)
                # replicated (packed-pair) scale/bias
                sF = s1p.tile([128, 1], F32, tag="sF")
                tF = s1p.tile([128, 1], F32, tag="tF")
                sC = s1p.tile([128, 1], F32, tag="sC")
                tC = s1p.tile([128, 1], F32, tag="tC")
                for half in range(2):
                    hp = slice(half * F, half * F + F)
                    nc.sync.dma_start(out=sF[hp, :], in_=s1n[0:F, :])
                    nc.sync.dma_start(out=tF[hp, :], in_=t1n[0:F, :])
                    nc.sync.dma_start(out=sC[hp, :], in_=s1c_[F:G, :])
                    nc.sync.dma_start(out=tC[hp, :], in_=t1c_[F:G, :])

                if not live((l, 4)):
                    break
                # ---- pass B: sigmoid*softplus, neighbor-sum, stats2 ----
                tc.strict_bb_all_engine_barrier()
                with tc.tile_pool(name="pb", bufs=3) as bp:
                    for p in range(NPAIR):
                        t0, t1_ = 2 * p, min(2 * p + 1, NT - 1)
                        single = (2 * p + 1 > NT - 1)
                        pp = 128 if not single else F
                        zf = bp.tile([128, ET], F16, tag="zf")
                        zc = bp.tile([128, ET], F16, tag="zc")
                        nc.sync.dma_start(out=zf[0:F, :], in_=scr[t0, 0:F, :])
                        nc.sync.dma_start(out=zc[0:F, :], in_=scr[t0, F:G, :])
                        if not single:
                            nc.sync.dma_start(out=zf[F:G, :], in_=scr[t1_, 0:F, :])
                            nc.sync.dma_start(out=zc[F:G, :], in_=scr[t1_, F:G, :])
                        # ef = 1 + exp(-(sF*zf+tF)) in f32 (no inf/denorm, so
                        # the bit-trick reciprocal seed is safe)
                        ef = bp.tile([128, ET], F32, tag="ef")
                        nc.scalar.activation(out=ef[0:pp, :], in_=zf[0:pp, :],
                                             func=AF.Exp,
                                             bias=tF[0:pp, :], scale=sF[0:pp, :])
                        ec = bp.tile([128, ET], F32, tag="ec")
                        nc.scalar.activation(out=ec[0:pp, :], in_=zc[0:pp, :],
                                             func=AF.Exp,
                                             bias=tC[0:pp, :], scale=sC[0:pp, :])
                        sp = bp.tile([128, ET], F16, tag="sp")
                        nc.scalar.activation(out=sp[0:pp, :], in_=ec[0:pp, :],
                                             func=AF.Ln, bias=1.0, scale=1.0)
                        # engine rebalance: the two elementwise passes ride the
                        # otherwise-idle Pool engine; DVE keeps the recip +
                        # reduce + stats, scalar keeps the transcendentals
                        nc.gpsimd.tensor_scalar(out=ef[0:pp, :], in0=ef[0:pp, :],
                                                scalar1=1.0, scalar2=None,
                                                op0=OP.add)
                        sg = bp.tile([128, ET], F32, tag="sg")
                        nc.vector.reciprocal_approx_fast(out=sg[0:pp, :],
                                                         in_=ef[0:pp, :])
                        nc.gpsimd.tensor_tensor(out=sp[0:pp, :], in0=sp[0:pp, :],
                                                in1=sg[0:pp, :], op=OP.mult)
                        # one full-width M-reduce for both pair halves
                        red2 = bp.tile([128, TA], F32, tag="red2")
                        nc.vector.tensor_reduce(
                            out=red2[0:pp, :],
                            in_=sp[0:pp, :].rearrange("p (a m) -> p a m", m=M),
                            axis=mybir.AxisListType.X, op=OP.add)
                        halves = 1 if single else 2
                        for hh in range(halves):
                            tt = t0 if hh == 0 else t1_
                            hs = slice(hh * F, hh * F + F)
                            nc.vector.tensor_copy(
                                out=summed[:, tt * TA:(tt + 1) * TA],
                                in_=red2[hs, :])
                            # stats over real atoms only
                            nreal = min(c["A_shard"] - tt * TA, TA)
                            col = 2 * p + hh
                            nc.vector.tensor_reduce(
                                out=st2_s[:, col:col + 1], in_=red2[hs, 0:nreal],
                                axis=mybir.AxisListType.X, op=OP.add)
                            sqt = bp.tile([F, TA], F32, tag="sqt")
                            nc.scalar.activation(
                                out=sqt[:, 0:nreal], in_=red2[hs, 0:nreal],
                                func=AF.Square,
                                accum_out=st2_q[:, col:col + 1])

                if not live((l, 5)):
                    break
                # ---- stats2 AllReduce + affine2 + pass C ----
                ncols = NT
                pack2 = s1p.tile([F, 2], F32, tag="pack2")
                nc.vector.tensor_reduce(out=pack2[:, 0:1],
                                        in_=st2_s[:, 0:ncols],
                                        axis=mybir.AxisListType.X, op=OP.add)
                nc.vector.tensor_reduce(out=pack2[:, 1:2],
                                        in_=st2_q[:, 0:ncols],
                                        axis=mybir.AxisListType.X, op=OP.add)
                nc.sync.dma_start(out=ar2_in[:, :], in_=pack2[:])
                tc.strict_bb_all_engine_barrier()
                nc.gpsimd.collective_compute(
                    "AllReduce", OP.add, replica_groups=rg,
                    ins=[ar2_in[:, :]], outs=[ar2_out[:, :]])
                tc.strict_bb_all_engine_barrier()

                red2 = s1p.tile([F, 2], F32, tag="red2")
                nc.sync.dma_start(out=red2[:], in_=ar2_out[:, :])
                g2_t = s1p.tile([F, 1], F32, tag="g2t")
                nc.sync.dma_start(out=g2_t[:], in_=g2[l, :, :])
                be2_t = s1p.tile([F, 1], F32, tag="be2t")
                nc.sync.dma_start(out=be2_t[:], in_=be2[l, :, :])

                invN = 1.0 / float(N)
                mt2 = s1p.tile([F, 8], F32, tag="mt2")
                m2c = mt2[:, 0:1]
                nc.vector.tensor_scalar(out=m2c, in0=red2[:, 0:1], scalar1=invN,
                                        scalar2=None, op0=OP.mult)
                e2c = mt2[:, 1:2]
                nc.vector.tensor_scalar(out=e2c, in0=red2[:, 1:2], scalar1=invN,
                                        scalar2=None, op0=OP.mult)
                ms2 = mt2[:, 2:3]
                nc.vector.tensor_tensor(out=ms2, in0=m2c, in1=m2c, op=OP.mult)
                v2 = mt2[:, 3:4]
                nc.vector.tensor_tensor(out=v2, in0=e2c, in1=ms2, op=OP.subtract)
                lv2 = mt2[:, 4:5]
                nc.scalar.activation(out=lv2, in_=v2, func=AF.Ln,
                                     bias=eps_t[0:F, :], scale=1.0)
                r02 = mt2[:, 5:6]
                nc.scalar.activation(out=r02, in_=lv2, func=AF.Exp, scale=-0.5)
                vpe2 = mt2[:, 6:7]
                nc.vector.tensor_scalar(out=vpe2, in0=v2, scalar1=eps_t[0:F, :],
                                        scalar2=0.5, op0=OP.add, op1=OP.mult)
                r0q2 = mt2[:, 7:8]
                nc.vector.tensor_tensor(out=r0q2, in0=r02, in1=r02, op=OP.mult)
                nc.vector.tensor_tensor(out=r0q2, in0=r0q2, in1=vpe2, op=OP.mult)
                nc.vector.tensor_scalar(out=r0q2, in0=r0q2, scalar1=-1.0,
                                        scalar2=1.5, op0=OP.mult, op1=OP.add)
                r12 = mt2[:, 6:7]
                nc.vector.tensor_tensor(out=r12, in0=r02, in1=r0q2, op=OP.mult)
                s2c = s1p.tile([F, 1], F32, tag="s2c")
                nc.vector.tensor_tensor(out=s2c[:], in0=g2_t[:], in1=r12,
                                        op=OP.mult)
                t2c = s1p.tile([F, 1], F32, tag="t2c")
                nc.vector.tensor_tensor(out=t2c[:], in0=m2c, in1=s2c[:],
                                        op=OP.mult)
                nc.vector.scalar_tensor_tensor(out=t2c[:], in0=t2c[:],
                                               scalar=-1.0, in1=be2_t[:],
                                               op0=OP.mult, op1=OP.add)

                if not live((l, 6)):
                    break
                # pass C: x = softplus(x + s2*summed + t2) via ln(1+exp)
                with tc.tile_pool(name="pc", bufs=2) as pcp:
                    CW = 2048
                    for j in range(0, AP_, CW):
                        w = min(CW, AP_ - j)
                        pre = pcp.tile([F, CW], F32, tag="pre")
                        nc.vector.scalar_tensor_tensor(
                            out=pre[:, :w], in0=summed[:, j:j + w],
                            scalar=s2c[:, 0:1], in1=x_cm[:, j:j + w],
                            op0=OP.mult, op1=OP.add)
                        pex = pcp.tile([F, CW], F32, tag="pex")
                        nc.scalar.activation(out=pex[:, :w], in_=pre[:, :w],
                                             func=AF.Exp, bias=t2c[:], scale=1.0)
                        nc.scalar.activation(out=x_cm[:, j:j + w], in_=pex[:, :w],
                                             func=AF.Ln, bias=1.0, scale=1.0)
                if dbg_dump:
                    nc.gpsimd.dma_start(out=dbgsum[l, :, :], in_=summed[:, :])
                    nc.sync.dma_start(out=dbgx[l, :, :], in_=x_cm[:, :])

        for _ in range(1 if live((L, 0)) else 0):
            # ---------------- crystal pooling ----------------
            with tc.tile_pool(name="pool", bufs=4) as pp_, \
                 tc.tile_pool(name="poolacc", bufs=1, space="PSUM") as pacc, \
                 tc.tile_pool(name="poolps", bufs=2, space="PSUM") as pps:
                acc = pacc.tile([F, 512], F32, tag="pacc")
                for t in range(NT):
                    xps = pps.tile([TA, F], F32, tag="xps")
                    nc.tensor.transpose(out=xps[:], in_=x_cm[:, t * TA:(t + 1) * TA],
                                        identity=id_t[0:F, 0:F])
                    xrm = pp_.tile([TA, F], F16, tag="xrm")
                    nc.scalar.activation(out=xrm[:], in_=xps[:], func=AF.Copy)
                    oht = pp_.tile([128, 512], F16, tag="oht")
                    nc.sync.dma_start(out=oht[:], in_=pone[t, :, :])
                    nc.tensor.matmul(acc[:], lhsT=xrm[:], rhs=oht[:],
                                     start=(t == 0), stop=(t == NT - 1))
                # zero pool_in, then scatter our window rows
                for r_ in range(NCB // 128):
                    nc.sync.dma_start(out=pool_in[128 * r_:128 * (r_ + 1), :],
                                      in_=zt128[:, 0:F])
                sci = pp_.tile([128, 32], I16, tag="sci")
                nc.sync.dma_start(out=sci[:], in_=scidx[:, :])
                tc.strict_bb_all_engine_barrier()
                accS = pp_.tile([F, 512], F32, tag="accS")
                nc.vector.tensor_copy(out=accS[:], in_=acc[:])
                asb4 = pp_.tile([128, 4, F], F32, tag="asb4")
                for ci in range(4):
                    # [F, 128] crystal block -> row-major [128, F]
                    tps = pps.tile([128, F], F32, tag="tps")
                    nc.tensor.transpose(out=tps[:],
                                        in_=accS[:, ci * 128:(ci + 1) * 128],
                                        identity=id_t[0:F, 0:F])
                    nc.vector.tensor_copy(out=asb4[:, ci, :], in_=tps[:])
                # one SWDGE scatter-add of the 512-crystal window (row i of
                # the window lives at asb4[i%128, i//128, :]); queue follows
                # the global Pool-DMA lane counter like the gathers
                nc.gpsimd.dma_scatter_add(
                    pool_in[:, :], asb4[:], sci[:, :], 512, 512, F,
                    single_packet=False, queue_num=gather_q())
                tc.strict_bb_all_engine_barrier()
                nc.gpsimd.collective_compute(
                    "AllReduce", OP.add, replica_groups=rg,
                    ins=[pool_in[:, :]], outs=[pool_out[:, :]])
                tc.strict_bb_all_engine_barrier()

            if dbg_dump:
                nc.sync.dma_start(out=dbgpool[:, :], in_=pool_out[:, :])


        for _ in range(1 if live((L, 1)) else 0):
            # ---------------- head ----------------
            with tc.tile_pool(name="head", bufs=3) as hp, \
                 tc.tile_pool(name="headps", bufs=2, space="PSUM") as hps:
                crys = nc.alloc_sbuf_tensor("crys", [F, NCB], F16)
                wfc_t = hp.tile([F, H], F16, tag="wfc")
                nc.sync.dma_start(out=wfc_t[:], in_=w_fc[:, :])
                bfc_t = hp.tile([H, 1], F32, tag="bfc")
                nc.sync.dma_start(out=bfc_t[:], in_=b_fc[:, :])
                wout_t = hp.tile([H, 1], F16, tag="wout")
                nc.sync.dma_start(out=wout_t[:], in_=w_out[:, :])
                bout_t = hp.tile([1, 1], F32, tag="bout")
                nc.sync.dma_start(out=bout_t[:], in_=b_out[:, :])

                for r_ in range(NCB // 128):
                    pt = hp.tile([128, F], F32, tag="pt")
                    nc.sync.dma_start(out=pt[:], in_=pool_out[128 * r_:128 * (r_ + 1), :])
                    ic = hp.tile([128, 1], F32, tag="ic")
                    nc.sync.dma_start(out=ic[:], in_=invcnt[128 * r_:128 * (r_ + 1), :])
                    nc.vector.tensor_scalar(out=pt[:], in0=pt[:], scalar1=ic[:, 0:1],
                                            scalar2=None, op0=OP.mult)
                    pex2 = hp.tile([128, F], F32, tag="pex2")
                    nc.scalar.activation(out=pex2[:], in_=pt[:], func=AF.Exp)
                    spt = hp.tile([128, F], F32, tag="spt")
                    nc.scalar.activation(out=spt[:], in_=pex2[:], func=AF.Ln,
                                         bias=1.0, scale=1.0)
                    tps = hps.tile([F, 128], F32, tag="tps")
                    nc.tensor.transpose(out=tps[:], in_=spt[:], identity=id_t[:, :])
                    nc.scalar.activation(out=crys[:, 128 * r_:128 * (r_ + 1)],
                                         in_=tps[:], func=AF.Copy)

                hc = nc.alloc_sbuf_tensor("hc", [H, NCB], F16)
                for j in range(0, NCB, 512):
                    psh = hps.tile([H, 512], F32, tag="psh")
                    nc.tensor.matmul(psh[:], lhsT=wfc_t[:], rhs=crys[:, j:j + 512],
                                     start=True, stop=True)
                    hex_ = hp.tile([H, 512], F32, tag="hex")
                    nc.scalar.activation(out=hex_[:], in_=psh[:],
                                         func=AF.Exp, bias=bfc_t[:], scale=1.0)
                    nc.scalar.activation(out=hc[:, j:j + 512], in_=hex_[:],
                                         func=AF.Ln, bias=1.0, scale=1.0)
                ofin = hp.tile([1, NCB], F32, tag="ofin")
                for j in range(0, NCB, 512):
                    pso = hps.tile([1, 512], F32, tag="pso")
                    nc.tensor.matmul(pso[:], lhsT=wout_t[:], rhs=hc[:, j:j + 512],
                                     start=True, stop=True)
                    nc.scalar.activation(out=ofin[:, j:j + 512], in_=pso[:],
                                         func=AF.Identity, bias=bout_t[:], scale=1.0)
                nc.sync.dma_start(out=out_t[:, :], in_=ofin[:])

    nc.compile()
    return nc


# --------------------------------------------------------------------------
# host-side input preparation
# --------------------------------------------------------------------------

def prepare_inputs(c, atom_fea, nbr_fea, nbr_fea_idx, crystal_atom_idx,
                   W_emb, b_emb, W_full, b_full, g1, be1, g2, be2,
                   W_fc, b_fc, W_out, b_out):
    N, M, F0, FB, F, H, NC, L = (c["N"], c["M"], c["F0"], c["FB"], c["F"],
                                 c["H"], c["NC"], c["NCONV"])
    G, TA, NT, AP_, ET, EL = (c["G"], c["TA"], c["ntile"], c["A_pad"],
                              c["E_tile"], c["E_loc"])
    W16, NCB, K = c["W16"], c["NCB"], c["NCORES"]
    AS = c["A_shard"]

    atom_fea = np.asarray(atom_fea, np.float32)
    nbr_fea = np.asarray(nbr_fea, np.float32)
    nbr_fea_idx = np.asarray(nbr_fea_idx, np.int64)
    crystal_atom_idx = np.asarray(crystal_atom_idx, np.int64)

    # shared (replicated) tensors
    oh = np.zeros((128, ET), np.float16)
    for j in range(ET):
        oh[j // M, j] = 1.0
    shared = {
        "oh_self": oh,
        "w_emb": np.asarray(W_emb, np.float16),
        "b_emb": np.asarray(b_emb, np.float32).reshape(F, 1),
        "w_self": np.asarray(W_full[:, :F, :], np.float16),
        "w_nbr": np.asarray(W_full[:, F:2 * F, :], np.float16),
        "w_b": np.asarray(W_full[:, 2 * F:, :], np.float16),
        "g1": np.asarray(g1, np.float32).reshape(L, G, 1),
        "be1": np.asarray(be1, np.float32).reshape(L, G, 1),
        "g2": np.asarray(g2, np.float32).reshape(L, F, 1),
        "be2": np.asarray(be2, np.float32).reshape(L, F, 1),
        "w_fc": np.asarray(W_fc, np.float16),
        "b_fc": np.asarray(b_fc, np.float32).reshape(H, 1),
        "w_out": np.asarray(W_out, np.float16),
        "b_out": np.asarray(b_out, np.float32).reshape(1, 1),
        "ident": np.eye(128, dtype=np.float32),
        "identh": np.eye(128, dtype=np.float16),
    }
    # crystal counts (global, from index data only)
    cnt = np.bincount(crystal_atom_idx, minlength=NC).astype(np.float32)
    icnt = np.zeros((NCB, 1), np.float32)
    icnt[:NC, 0] = 1.0 / np.maximum(cnt, 1.0)
    shared["invcnt"] = icnt

    # b_full is mathematically irrelevant (cancelled by training-mode BN)

    in_maps = []
    for k in range(K):
        a0 = k * AS
        af = np.zeros((F0, AP_), np.float16)
        af[:, :AS] = atom_fea[a0:a0 + AS].T
        # edge ordering: e = a*M + m within each tile of TA atoms
        gi = np.zeros((NT * TA, M), np.int64)
        gi_raw = nbr_fea_idx[a0:a0 + AS]
        gi[:AS] = gi_raw
        valid = np.zeros((NT * TA, M), bool)
        valid[:AS] = True
        jj = (gi // AS) * AP_ + (gi % AS)          # padded-global atom index
        rows = np.where(valid, 1 + jj // 2, 0).astype(np.int64)
        par = np.where(valid, jj & 1, 0).astype(np.int64)
        idxw = np.zeros((128, NT, W16), np.int16)
        mparr = np.zeros((128, NT, M), np.int8)
        j = np.arange(ET)
        for t in range(NT):
            fl = rows[t * TA:(t + 1) * TA].reshape(ET)
            wrap = np.zeros((16, W16), np.int16)
            wrap[j % 16, j // 16] = fl
            idxw[:, t, :] = np.tile(wrap, (8, 1))
            # ge[p, i, :] holds edge i*128+p -> mask[p, i]
            mparr[:, t, :] = par[t * TA:(t + 1) * TA].reshape(ET)[
                (np.arange(M)[None, :] * 128 + np.arange(128)[:, None])]
        nb = np.zeros((FB, EL), np.float16)
        nb_l = nbr_fea[a0:a0 + AS].reshape(AS * M, FB)
        src = np.zeros((NT * TA * M, FB), np.float32)
        src[:AS * M] = nb_l
        # src is already in (a, m) order; tiles are contiguous runs of ET
        nb[:, :] = src.T.astype(np.float16)

        cry = np.zeros(NT * TA, np.int64)
        cry[:AS] = crystal_atom_idx[a0:a0 + AS]
        cb = int(crystal_atom_idx[a0:a0 + AS].min())
        cmax = int(crystal_atom_idx[a0:a0 + AS].max())
        assert cmax - cb < 512, f"crystal window too wide: {cmax - cb}"
        # pool one-hot: [atom (partition), crystal-window col]
        pone = np.zeros((NT, TA, 512), np.float16)
        for t in range(NT):
            for a in range(TA):
                ga = t * TA + a
                if ga >= AS:
                    continue
                pone[t, a, int(cry[ga]) - cb] = 1.0
        # scatter-add indices: window row j -> crystal cb+j, wrapped in 16
        # partitions and replicated (SWDGE idx convention)
        sw = np.zeros((16, 32), np.int16)
        jj512 = np.arange(512)
        sw[jj512 % 16, jj512 // 16] = (cb + jj512).astype(np.int16)
        scidx = np.tile(sw, (8, 1))
        assert cb + 512 <= NCB

        in_maps.append(dict(shared,
                            afT=af, idxw=idxw, mparr=mparr, nbrT=nb, pone=pone,
                            scidx=scidx))
    return in_maps


# --------------------------------------------------------------------------
# public entry point
# --------------------------------------------------------------------------

_PROG_CACHE = {}


def _get_program(c):
    key = tuple(sorted((k, v) for k, v in c.items()))
    if key not in _PROG_CACHE:
        _PROG_CACHE[key] = build_program(c)
    return _PROG_CACHE[key]


def kernel(atom_fea, nbr_fea, nbr_fea_idx, crystal_atom_idx, W_emb, b_emb,
           W_full, b_full, g1, be1, g2, be2, W_fc, b_fc, W_out, b_out,
           _trace=False):
    from concourse import bass_utils
    c = CFG
    nc = _get_program(c)
    in_maps = prepare_inputs(c, atom_fea, nbr_fea, nbr_fea_idx,
                             crystal_atom_idx, W_emb, b_emb, W_full, b_full,
                             g1, be1, g2, be2, W_fc, b_fc, W_out, b_out)
    res = bass_utils.run_bass_kernel_spmd(
        nc, in_maps, core_ids=list(range(c["NCORES"])), trace=_trace)
    out = np.asarray(res.results[0]["out"], np.float32)
    ret = out[0, :c["NC"]].reshape(c["NC"], 1)
    if _trace:
        return ret, res
    return ret



# revision 22
# speedup vs baseline: 1.7151x; 1.7151x over previous
"""CrystalGraphConvNet forward pass as a distributed Bass/Tile kernel on 8 TRN2
NeuronCores.

Strategy (graph/data parallel, per sharding hint):
  - Atoms sharded contiguously across 8 cores (7500 each, padded to 7552).
  - Per conv layer, each core computes Y_self = x @ Wf[:F], Y_nbr = x @ Wf[F:2F]
    for its atom shard; Y_nbr shards are AllGathered into a replicated f16
    table viewed as PAIR rows (two atoms = 512 B per row, plus a leading zero
    row), so a single int16-indexed dma_gather per tile fetches both parity
    candidates of every edge in one 512 B packet (row index = 1 + j//2).
  - The gather runs un-transposed (contiguous SBUF writes, edge-major):
    ge[p, i, 0:128] / [128:256] hold the even/odd atom of edge i*128+p.  A
    single copy_predicated with a tiny resident per-edge parity mask
    (broadcast along channels) selects the right atom; 12 accumulating
    transpose-matmuls against an f16 identity then fold the selected rows
    into the channel-major PSUM accumulator on the TensorEngine, on top of
    the nbr_fea projection and the one-hot self term.
  - Training-mode batchnorm needs global stats, so gated values are staged to
    DRAM scratch in f16 while per-channel sum/sumsq accumulate (scalar-engine
    copy/square with accum); a tiny AllReduce yields the affine.  The second
    pass is a SINGLE pass per tile pair: sigmoid is computed as
    1/(1+exp(-a)) with a DVE reciprocal, so every activation in the program
    lives in the one exp/ln table set and no ACT table reloads occur.
  - Crystal mean-pooling is one accumulating one-hot matmul per tile into a
    512-crystal window, scattered by int32 indirect DMA into a global crystal
    array and AllReduced; the tiny MLP head runs redundantly on every core.
"""

import math
import os
import ml_dtypes
import numpy as np

import concourse.bass as bass
import concourse.bacc as bacc
import concourse.tile as tile
from concourse import mybir
from contextlib import ExitStack


def _single_act_table(orig):
    """Route every exp/ln/copy/square/identity activation to the one ACT
    table set that contains BOTH exp and ln.  The default chooser assigns
    exp->exp_and_others and ln->natural_log, which thrashes table reloads
    (~1.3us each) on every exp<->ln transition in the batchnorm second pass.
    Removing those funcs from every other set leaves the chooser exactly one
    legal assignment; set ids keep their act_info.json indices so walrus
    loads the right tables."""
    AF = mybir.ActivationFunctionType
    both = {AF.Exp, AF.Ln}
    shared = {AF.Exp, AF.Ln, AF.Copy, AF.Square, AF.Identity}
    home = None
    for name, funcs in orig.items():
        if both <= funcs:
            home = name
            break
    if home is None:
        return orig
    out = {}
    for name, funcs in orig.items():
        out[name] = funcs if name == home else (funcs - shared)
    return out


_orig_get_activation_tables = bacc.get_activation_tables


def _patched_get_activation_tables(arch):
    return _single_act_table(_orig_get_activation_tables(arch))


bacc.get_activation_tables = _patched_get_activation_tables

F16 = mybir.dt.float16
F32 = mybir.dt.float32
F8 = mybir.dt.float8e4
I8 = mybir.dt.int8
I16 = mybir.dt.int16
I32 = mybir.dt.int32


def make_cfg(N=60000, M=12, F0=92, FB=41, F=64, H=128, NC=2000, NCONV=3,
             EPS=1e-5, NCORES=8, TA=128):
    c = dict(N=N, M=M, F0=F0, FB=FB, F=F, H=H, NC=NC, NCONV=NCONV, EPS=EPS,
             NCORES=NCORES, TA=TA)
    assert N % NCORES == 0
    c["A_shard"] = N // NCORES
    c["ntile"] = (c["A_shard"] + TA - 1) // TA
    c["A_pad"] = c["ntile"] * TA
    c["E_tile"] = TA * M
    c["E_loc"] = c["ntile"] * c["E_tile"]
    assert (NCORES * c["A_pad"]) % 2 == 0
    c["R2"] = 1 + NCORES * c["A_pad"] // 2      # pair-table rows (zero row at 0)
    assert c["R2"] <= 32768, "pair table must stay int16-addressable"
    c["W16"] = c["E_tile"] // 16
    c["NCB"] = 512 * ((NC + 511) // 512) + 512  # crystal bounce rows
    c["G"] = 2 * F                              # gated channels
    return c


CFG = make_cfg()


# --------------------------------------------------------------------------
# program builder
# --------------------------------------------------------------------------

def build_program(c, debug=False, dbg_dump=False, stop=None):
    # stop: optional (layer, stage) tuple for bisection; stages within a layer:
    # 0=Y, 1=AG, 2=A, 3=AR1, 4=B, 5=AR2, 6=C; (L,0)=pool, (L,1)=head
    nc = bacc.Bacc("TRN2", target_bir_lowering=False, debug=debug,
                   num_devices=c["NCORES"], num_swdge_queues=4)

    N, M, F0, FB, F, H, NC, L = (c["N"], c["M"], c["F0"], c["FB"], c["F"],
                                 c["H"], c["NC"], c["NCONV"])
    G, TA, NT, AP_, ET, EL = (c["G"], c["TA"], c["ntile"], c["A_pad"],
                              c["E_tile"], c["E_loc"])
    R2, W16, NCB, EPS = c["R2"], c["W16"], c["NCB"], c["EPS"]
    NPAIR = (NT + 1) // 2
    NCHUNK = ET // TA                           # 128-edge chunks per tile

    # ---------------- inputs ----------------
    afT = nc.dram_tensor("afT", [F0, AP_], F16, kind="ExternalInput")
    idxw = nc.dram_tensor("idxw", [128, NT, W16], I16, kind="ExternalInput")
    mparr = nc.dram_tensor("mparr", [128, NT, M], I8, kind="ExternalInput")
    nbrT = nc.dram_tensor("nbrT", [FB, EL], F8, kind="ExternalInput")
    oh_self = nc.dram_tensor("oh_self", [128, ET], F16, kind="ExternalInput")
    pone = nc.dram_tensor("pone", [NT, 128, 512], F16, kind="ExternalInput")
    scidx = nc.dram_tensor("scidx", [128, 32], I16, kind="ExternalInput")
    invcnt = nc.dram_tensor("invcnt", [NCB, 1], F32, kind="ExternalInput")
    w_emb = nc.dram_tensor("w_emb", [F0, F], F16, kind="ExternalInput")
    b_emb = nc.dram_tensor("b_emb", [F, 1], F32, kind="ExternalInput")
    w_self = nc.dram_tensor("w_self", [L, F, G], F16, kind="ExternalInput")
    w_nbr = nc.dram_tensor("w_nbr", [L, F, G], F16, kind="ExternalInput")
    w_b = nc.dram_tensor("w_b", [L, FB, G], F8, kind="ExternalInput")
    g1 = nc.dram_tensor("g1", [L, G, 1], F32, kind="ExternalInput")
    be1 = nc.dram_tensor("be1", [L, G, 1], F32, kind="ExternalInput")
    g2 = nc.dram_tensor("g2", [L, F, 1], F32, kind="ExternalInput")
    be2 = nc.dram_tensor("be2", [L, F, 1], F32, kind="ExternalInput")
    w_fc = nc.dram_tensor("w_fc", [F, H], F16, kind="ExternalInput")
    b_fc = nc.dram_tensor("b_fc", [H, 1], F32, kind="ExternalInput")
    w_out = nc.dram_tensor("w_out", [H, 1], F16, kind="ExternalInput")
    b_out = nc.dram_tensor("b_out", [1, 1], F32, kind="ExternalInput")
    ident = nc.dram_tensor("ident", [128, 128], F32, kind="ExternalInput")
    identh = nc.dram_tensor("identh", [128, 128], F16, kind="ExternalInput")

    out_t = nc.dram_tensor("out", [1, NCB], F32, kind="ExternalOutput")
    if dbg_dump:
        L_, F_, G_, AP2, EL_ = c["NCONV"], c["F"], c["G"], c["A_pad"], c["E_loc"]
        dbgx0 = nc.dram_tensor("dbgx0", [F_, AP2], F32, kind="ExternalOutput")
        dbgx = nc.dram_tensor("dbgx", [L_, F_, AP2], F32, kind="ExternalOutput")
        dbgsum = nc.dram_tensor("dbgsum", [L_, F_, AP2], F32, kind="ExternalOutput")
        dbgst1 = nc.dram_tensor("dbgst1", [L_, G_, 2], F32, kind="ExternalOutput")
        dbggat = nc.dram_tensor("dbggat", [L_, NT, 128, ET], F8,
                                kind="ExternalOutput")
        dbgpool = nc.dram_tensor("dbgpool", [NCB, F_], F32, kind="ExternalOutput")

    # ---------------- internal DRAM ----------------
    yb = nc.dram_tensor("yb", [AP_, G], F16)                        # AG input bounce
    tbl = nc.dram_tensor("tbl", [R2, 2 * G], F16, addr_space="Shared")
    scr = nc.dram_tensor("scr", [NT, 128, ET], F8)
    ar1_in = nc.dram_tensor("ar1_in", [G, 2], F32)
    ar1_out = nc.dram_tensor("ar1_out", [G, 2], F32, addr_space="Shared")
    ar2_in = nc.dram_tensor("ar2_in", [F, 2], F32)
    ar2_out = nc.dram_tensor("ar2_out", [F, 2], F32, addr_space="Shared")
    pool_in = nc.dram_tensor("pool_in", [NCB, F], F32)
    pool_out = nc.dram_tensor("pool_out", [NCB, F], F32, addr_space="Shared")

    rg = [list(range(c["NCORES"]))]
    AF = mybir.ActivationFunctionType
    OP = mybir.AluOpType
    if stop is None:
        stop = (L, 9)

    # Tile cycles DMASW lane sems per Pool DMA in GLOBAL program order (%8);
    # each lane sem is queue-locked, so the SWDGE queue must follow the same
    # global counter (%4), not the per-layer tile index.
    gq = [0]

    def gather_q():
        q = gq[0] % 4
        gq[0] += 1
        return q

    def live(key):
        return key <= stop

    with tile.TileContext(nc) as tc, ExitStack() as top:
        # Tile assigns each Pool-engine DMA inst a DMASW lane (round-robin in
        # program order) and points consumer waits at that lane's semaphore.
        # A prepare_only gather bakes its completion sem into the descriptors
        # (pass 2 attaches no then_inc), so sem= MUST be the very lane sem the
        # tick pass will pick, tracked here with a mirror counter.
        swsems = tc.sems.swdge_block()
        pool_dma_lane = [0]

        def prep_slot():
            """(lane sem, queue) for the next prepared Pool DMA.  queue =
            lane % nqueues keeps every lane sem on one fixed queue (SWDGE
            sems are queue-locked)."""
            j = pool_dma_lane[0]
            pool_dma_lane[0] += 1
            return swsems[j % len(swsems)], (j % len(swsems)) % 4

        # persistent SBUF state
        x_cm = nc.alloc_sbuf_tensor("x_cm", [F, AP_], F32)
        summed = nc.alloc_sbuf_tensor("summed", [F, AP_], F16)
        ysr = nc.alloc_sbuf_tensor("ysr", [128, NT, G], F16)      # Y_self row-major
        idx_all = nc.alloc_sbuf_tensor("idx_all", [128, NT, W16], I16)
        mpar = nc.alloc_sbuf_tensor("mpar", [128, NT, M, 1], I8)

        const = top.enter_context(tc.tile_pool(name="const", bufs=1))
        stats = top.enter_context(tc.tile_pool(name="stats", bufs=1))

        # constants resident all kernel
        ohs_t = const.tile([128, ET], F16)
        nc.sync.dma_start(out=ohs_t[:], in_=oh_self[:, :])
        wemb_t = const.tile([F0, F], F16)
        nc.sync.dma_start(out=wemb_t[:], in_=w_emb[:, :])
        bemb_t = const.tile([F, 1], F32)
        nc.sync.dma_start(out=bemb_t[:], in_=b_emb[:, :])
        id_t = const.tile([128, 128], F32)
        nc.sync.dma_start(out=id_t[:], in_=ident[:, :])
        idh_t = const.tile([128, 128], F16)
        nc.sync.dma_start(out=idh_t[:], in_=identh[:, :])
        eps_t = const.tile([128, 1], F32)
        nc.vector.memset(eps_t[:], EPS)
        zrow = const.tile([1, 2 * G], F16)
        nc.vector.memset(zrow[:], 0.0)
        zt128 = const.tile([128, F], F32)
        nc.vector.memset(zt128[:], 0.0)

        # layer-invariant gather indices and parity masks
        nc.sync.dma_start(out=idx_all[:, :, :], in_=idxw[:, :, :])
        nc.sync.dma_start(out=mpar[:, :, :, 0], in_=mparr[:, :, :])

        wS = []
        wN = []
        wB = []
        for l in range(L):
            t1 = const.tile([F, G], F16, tag=f"wS{l}")
            nc.sync.dma_start(out=t1[:], in_=w_self[l, :, :])
            t2 = const.tile([F, G], F16, tag=f"wN{l}")
            nc.sync.dma_start(out=t2[:], in_=w_nbr[l, :, :])
            t3 = const.tile([FB, G], F8, tag=f"wB{l}")
            nc.sync.dma_start(out=t3[:], in_=w_b[l, :, :])
            wS.append(t1)
            wN.append(t2)
            wB.append(t3)

        # stats buffers
        st1_s = stats.tile([G, NT], F32, tag="st1s")
        st1_q = stats.tile([G, NT], F32, tag="st1q")
        st2_s = stats.tile([F, 2 * NPAIR], F32, tag="st2s")
        st2_q = stats.tile([F, 2 * NPAIR], F32, tag="st2q")

        # zero table guard row + summed pads
        nc.sync.dma_start(out=tbl[0:1, :], in_=zrow[:])
        nc.vector.memset(summed[:, :], 0.0)

        # ---------------- embedding: x = atom_fea @ W_emb + b_emb ----------
        with tc.tile_pool(name="emb", bufs=3) as embp, \
             tc.tile_pool(name="embps", bufs=2, space="PSUM") as embps:
            CH = 512
            for j in range(0, AP_, CH):
                w = min(CH, AP_ - j)
                rhs = embp.tile([F0, CH], F16, tag="embr")
                nc.sync.dma_start(out=rhs[:, :w], in_=afT[:, j:j + w])
                ps = embps.tile([F, CH], F32, tag="embp")
                nc.tensor.matmul(ps[:, :w], lhsT=wemb_t[:], rhs=rhs[:, :w],
                                 start=True, stop=True)
                nc.scalar.activation(out=x_cm[:, j:j + w], in_=ps[:, :w],
                                     func=AF.Identity, bias=bemb_t[:], scale=1.0)
        if dbg_dump:
            nc.sync.dma_start(out=dbgx0[:, :], in_=x_cm[:, :])

        # ---------------- conv layers ----------------
        for l in range(L):
            if not live((l, 0)):
                break
            # ---- phase Y: Y_self (SBUF) / Y_nbr (-> bounce -> AllGather) ----
            with tc.tile_pool(name="yph", bufs=3) as yp, \
                 tc.tile_pool(name="yps", bufs=2, space="PSUM") as yps:
                lastreal = c["A_shard"] - (NT - 1) * TA
                for t in range(NT):
                    xa = yp.tile([F, TA], F16, tag="xa")
                    nc.scalar.activation(out=xa[:], in_=x_cm[:, t * TA:(t + 1) * TA],
                                         func=AF.Copy)
                    psS = yps.tile([TA, G], F32, tag="psS")
                    nc.tensor.matmul(psS[:], lhsT=xa[:], rhs=wS[l][:],
                                     start=True, stop=True)
                    # pad atoms of the last tile must contribute exactly zero
                    # through the self one-hot matmul
                    nreal = TA if t < NT - 1 else lastreal
                    if nreal < TA:
                        nc.vector.memset(ysr[:, t, :], 0.0)
                    nc.scalar.activation(out=ysr[0:nreal, t, :],
                                         in_=psS[0:nreal, :], func=AF.Copy)
                    psN = yps.tile([TA, G], F32, tag="psN")
                    nc.tensor.matmul(psN[:], lhsT=xa[:], rhs=wN[l][:],
                                     start=True, stop=True)
                    yn = yp.tile([TA, G], F16, tag="yn")
                    nc.scalar.activation(out=yn[:], in_=psN[:], func=AF.Copy)
                    nc.sync.dma_start(out=yb[t * TA:(t + 1) * TA, :], in_=yn[:])

            if not live((l, 1)):
                break
            tc.strict_bb_all_engine_barrier()
            nc.gpsimd.collective_compute(
                "AllGather", OP.bypass, replica_groups=rg,
                ins=[yb[:, :]], outs=[tbl[1:R2, :]])
            tc.strict_bb_all_engine_barrier()

            if not live((l, 2)):
                break
            # ---- pass A: edges -> gated scratch + stats1 ----
            with tc.tile_pool(name="pa", bufs=6) as pa, \
                 tc.tile_pool(name="paps", bufs=2, space="PSUM") as paps:
                for t in range(NT):
                    nbt = pa.tile([FB, ET], F8, tag="nbt")
                    nc.sync.dma_start(out=nbt[:], in_=nbrT[:, t * ET:(t + 1) * ET])

                    ge = pa.tile([128, NCHUNK, 2 * G], F16, tag="ge")
                    # prepare_only decouples Q7 desc-gen from the transfer:
                    # the gather DMA fires at trigger time, so transfers on
                    # the 4 SWDGE queues overlap each other and compute
                    # instead of serializing on the Pool engine.
                    nc.gpsimd.dma_gather(ge[:], tbl[:, :], idx_all[:, t, :],
                                         ET, ET, 2 * G, single_packet=False,
                                         queue_num=gather_q())
                    # parity select: overwrite even-atom slab with odd-atom
                    # slab wherever the edge's target index is odd
                    nc.vector.copy_predicated(
                        out=ge[:, :, 0:G],
                        mask=mpar[:, t, :, :].broadcast_to([128, NCHUNK, G]),
                        data=ge[:, :, G:2 * G])

                    # every 128-col region: chunk transpose (start) -> wB ->
                    # one-hot self term (stop)
                    ps = paps.tile([G, ET], F32, tag="aps")
                    for i in range(NCHUNK):
                        cs = slice(i * 128, (i + 1) * 128)
                        nc.tensor.matmul(ps[:, cs], lhsT=ge[:, i, 0:G],
                                         rhs=idh_t[:], start=(i % 4 == 0),
                                         stop=False)
                    for s in range(ET // 512):
                        sl = slice(s * 512, (s + 1) * 512)
                        nc.tensor.matmul(ps[:, sl], lhsT=wB[l][:], rhs=nbt[:, sl],
                                         start=False, stop=False)
                        nc.tensor.matmul(ps[:, sl], lhsT=ysr[:, t, :],
                                         rhs=ohs_t[:, sl], start=False, stop=True)

                    gat = pa.tile([128, ET], F8, tag="gat")
                    nc.scalar.activation(out=gat[:], in_=ps[:], func=AF.Copy,
                                         accum_out=st1_s[:, t:t + 1])
                    sqd = pa.tile([128, ET], F16, tag="sqd")
                    nc.scalar.activation(out=sqd[:], in_=ps[:], func=AF.Square,
                                         accum_out=st1_q[:, t:t + 1])
                    nc.sync.dma_start(out=scr[t, :, :], in_=gat[:])

            if not live((l, 3)):
                break
            # ---- stats1 reduce + AllReduce + affine ----
            with tc.tile_pool(name="s1", bufs=1) as s1p:
                pack1 = s1p.tile([G, 2], F32, tag="pack1")
                nc.vector.tensor_reduce(out=pack1[:, 0:1], in_=st1_s[:],
                                        axis=mybir.AxisListType.X, op=OP.add)
                nc.vector.tensor_reduce(out=pack1[:, 1:2], in_=st1_q[:],
                                        axis=mybir.AxisListType.X, op=OP.add)
                nc.sync.dma_start(out=ar1_in[:, :], in_=pack1[:])
                tc.strict_bb_all_engine_barrier()
                nc.gpsimd.collective_compute(
                    "AllReduce", OP.add, replica_groups=rg,
                    ins=[ar1_in[:, :]], outs=[ar1_out[:, :]])
                tc.strict_bb_all_engine_barrier()

                red1 = s1p.tile([G, 2], F32, tag="red1")
                nc.sync.dma_start(out=red1[:], in_=ar1_out[:, :])
                if dbg_dump:
                    nc.sync.dma_start(out=dbgst1[l, :, :], in_=red1[:])
                    for t in range(NT):
                        nc.gpsimd.dma_start(out=dbggat[l, t, :, :],
                                            in_=scr[t, :, :])
                g1_t = s1p.tile([G, 1], F32, tag="g1t")
                nc.sync.dma_start(out=g1_t[:], in_=g1[l, :, :])
                be1_t = s1p.tile([G, 1], F32, tag="be1t")
                nc.sync.dma_start(out=be1_t[:], in_=be1[l, :, :])

                # rsqrt(var+eps) = exp(-0.5*ln(var+eps)) + one Newton step
                # (no sqrt table needed; Ln/Exp share one ACT table set)
                invE = 1.0 / float(N * M)
                mmt = s1p.tile([G, 8], F32, tag="mmt")
                mcol = mmt[:, 0:1]
                nc.vector.tensor_scalar(out=mcol, in0=red1[:, 0:1], scalar1=invE,
                                        scalar2=None, op0=OP.mult)
                ex2 = mmt[:, 1:2]
                nc.vector.tensor_scalar(out=ex2, in0=red1[:, 1:2], scalar1=invE,
                                        scalar2=None, op0=OP.mult)
                msq = mmt[:, 2:3]
                nc.vector.tensor_tensor(out=msq, in0=mcol, in1=mcol, op=OP.mult)
                var = mmt[:, 3:4]
                nc.vector.tensor_tensor(out=var, in0=ex2, in1=msq, op=OP.subtract)
                lv = mmt[:, 4:5]
                nc.scalar.activation(out=lv, in_=var, func=AF.Ln,
                                     bias=eps_t[0:G, :], scale=1.0)
                r0 = mmt[:, 5:6]
                nc.scalar.activation(out=r0, in_=lv, func=AF.Exp, scale=-0.5)
                # one Newton step: r1 = r0*(1.5 - 0.5*(var+eps)*r0^2)
                vpe = mmt[:, 6:7]
                nc.vector.tensor_scalar(out=vpe, in0=var, scalar1=eps_t[0:G, :],
                                        scalar2=0.5, op0=OP.add, op1=OP.mult)
                r0q = mmt[:, 7:8]
                nc.vector.tensor_tensor(out=r0q, in0=r0, in1=r0, op=OP.mult)
                nc.vector.tensor_tensor(out=r0q, in0=r0q, in1=vpe, op=OP.mult)
                nc.vector.tensor_scalar(out=r0q, in0=r0q, scalar1=-1.0,
                                        scalar2=1.5, op0=OP.mult, op1=OP.add)
                r1 = mmt[:, 6:7]
                nc.vector.tensor_tensor(out=r1, in0=r0, in1=r0q, op=OP.mult)

                s1c_ = s1p.tile([G, 1], F32, tag="s1c")
                nc.vector.tensor_tensor(out=s1c_[:], in0=g1_t[:], in1=r1,
                                        op=OP.mult)
                t1c_ = s1p.tile([G, 1], F32, tag="t1c")
                nc.vector.tensor_tensor(out=t1c_[:], in0=mcol, in1=s1c_[:],
                                        op=OP.mult)
                nc.vector.scalar_tensor_tensor(out=t1c_[:], in0=t1c_[:],
                                               scalar=-1.0, in1=be1_t[:],
                                               op0=OP.mult, op1=OP.add)
                # negated F-half affine (sigmoid via 1/(1+exp(-a)))
                s1n = s1p.tile([G, 1], F32, tag="s1n")
                nc.vector.tensor_scalar(out=s1n[:], in0=s1c_[:], scalar1=-1.0,
                                        scalar2=None, op0=OP.mult)
                t1n = s1p.tile([G, 1], F32, tag="t1n")
                nc.vector.tensor_scalar(out=t1n[:], in0=t1c_[:], scalar1=-1.0,
                                        scalar2=None, op0=OP.mult)
                # replicated (packed-pair) scale/bias
                sF = s1p.tile([128, 1], F32, tag="sF")
                tF = s1p.tile([128, 1], F32, tag="tF")
                sC = s1p.tile([128, 1], F32, tag="sC")
                tC = s1p.tile([128, 1], F32, tag="tC")
                for half in range(2):
                    hp = slice(half * F, half * F + F)
                    nc.sync.dma_start(out=sF[hp, :], in_=s1n[0:F, :])
                    nc.sync.dma_start(out=tF[hp, :], in_=t1n[0:F, :])
                    nc.sync.dma_start(out=sC[hp, :], in_=s1c_[F:G, :])
                    nc.sync.dma_start(out=tC[hp, :], in_=t1c_[F:G, :])

                if not live((l, 4)):
                    break
                # ---- pass B: sigmoid*softplus, neighbor-sum, stats2 ----
                tc.strict_bb_all_engine_barrier()
                with tc.tile_pool(name="pb", bufs=3) as bp:
                    for p in range(NPAIR):
                        t0, t1_ = 2 * p, min(2 * p + 1, NT - 1)
                        single = (2 * p + 1 > NT - 1)
                        pp = 128 if not single else F
                        zf = bp.tile([128, ET], F8, tag="zf")
                        zc = bp.tile([128, ET], F8, tag="zc")
                        nc.sync.dma_start(out=zf[0:F, :], in_=scr[t0, 0:F, :])
                        nc.sync.dma_start(out=zc[0:F, :], in_=scr[t0, F:G, :])
                        if not single:
                            nc.sync.dma_start(out=zf[F:G, :], in_=scr[t1_, 0:F, :])
                            nc.sync.dma_start(out=zc[F:G, :], in_=scr[t1_, F:G, :])
                        # ef = 1 + exp(-(sF*zf+tF)) in f32 (no inf/denorm, so
                        # the bit-trick reciprocal seed is safe)
                        ef = bp.tile([128, ET], F32, tag="ef")
                        nc.scalar.activation(out=ef[0:pp, :], in_=zf[0:pp, :],
                                             func=AF.Exp,
                                             bias=tF[0:pp, :], scale=sF[0:pp, :])
                        ec = bp.tile([128, ET], F32, tag="ec")
                        nc.scalar.activation(out=ec[0:pp, :], in_=zc[0:pp, :],
                                             func=AF.Exp,
                                             bias=tC[0:pp, :], scale=sC[0:pp, :])
                        sp = bp.tile([128, ET], F16, tag="sp")
                        nc.scalar.activation(out=sp[0:pp, :], in_=ec[0:pp, :],
                                             func=AF.Ln, bias=1.0, scale=1.0)
                        nc.vector.tensor_scalar(out=ef[0:pp, :], in0=ef[0:pp, :],
                                                scalar1=1.0, scalar2=None,
                                                op0=OP.add)
                        sg = bp.tile([128, ET], F32, tag="sg")
                        nc.vector.reciprocal_approx_fast(out=sg[0:pp, :],
                                                         in_=ef[0:pp, :])
                        nc.vector.tensor_tensor(out=sp[0:pp, :], in0=sp[0:pp, :],
                                                in1=sg[0:pp, :], op=OP.mult)
                        # one full-width M-reduce for both pair halves
                        red2 = bp.tile([128, TA], F32, tag="red2")
                        nc.vector.tensor_reduce(
                            out=red2[0:pp, :],
                            in_=sp[0:pp, :].rearrange("p (a m) -> p a m", m=M),
                            axis=mybir.AxisListType.X, op=OP.add)
                        halves = 1 if single else 2
                        for hh in range(halves):
                            tt = t0 if hh == 0 else t1_
                            hs = slice(hh * F, hh * F + F)
                            # partition-aligned copy can ride the DVE; the
                            # shifted one hits a DVE slow path, keep on scalar
                            if hh == 0:
                                nc.vector.tensor_copy(
                                    out=summed[:, tt * TA:(tt + 1) * TA],
                                    in_=red2[hs, :])
                            else:
                                nc.scalar.activation(
                                    out=summed[:, tt * TA:(tt + 1) * TA],
                                    in_=red2[hs, :], func=AF.Copy)
                            # stats over real atoms only
                            nreal = min(c["A_shard"] - tt * TA, TA)
                            col = 2 * p + hh
                            nc.vector.tensor_reduce(
                                out=st2_s[:, col:col + 1], in_=red2[hs, 0:nreal],
                                axis=mybir.AxisListType.X, op=OP.add)
                            sqt = bp.tile([F, TA], F32, tag="sqt")
                            nc.scalar.activation(
                                out=sqt[:, 0:nreal], in_=red2[hs, 0:nreal],
                                func=AF.Square,
                                accum_out=st2_q[:, col:col + 1])

                if not live((l, 5)):
                    break
                # ---- stats2 AllReduce + affine2 + pass C ----
                ncols = NT
                pack2 = s1p.tile([F, 2], F32, tag="pack2")
                nc.vector.tensor_reduce(out=pack2[:, 0:1],
                                        in_=st2_s[:, 0:ncols],
                                        axis=mybir.AxisListType.X, op=OP.add)
                nc.vector.tensor_reduce(out=pack2[:, 1:2],
                                        in_=st2_q[:, 0:ncols],
                                        axis=mybir.AxisListType.X, op=OP.add)
                nc.sync.dma_start(out=ar2_in[:, :], in_=pack2[:])
                tc.strict_bb_all_engine_barrier()
                nc.gpsimd.collective_compute(
                    "AllReduce", OP.add, replica_groups=rg,
                    ins=[ar2_in[:, :]], outs=[ar2_out[:, :]])
                tc.strict_bb_all_engine_barrier()

                red2 = s1p.tile([F, 2], F32, tag="red2")
                nc.sync.dma_start(out=red2[:], in_=ar2_out[:, :])
                g2_t = s1p.tile([F, 1], F32, tag="g2t")
                nc.sync.dma_start(out=g2_t[:], in_=g2[l, :, :])
                be2_t = s1p.tile([F, 1], F32, tag="be2t")
                nc.sync.dma_start(out=be2_t[:], in_=be2[l, :, :])

                invN = 1.0 / float(N)
                mt2 = s1p.tile([F, 8], F32, tag="mt2")
                m2c = mt2[:, 0:1]
                nc.vector.tensor_scalar(out=m2c, in0=red2[:, 0:1], scalar1=invN,
                                        scalar2=None, op0=OP.mult)
                e2c = mt2[:, 1:2]
                nc.vector.tensor_scalar(out=e2c, in0=red2[:, 1:2], scalar1=invN,
                                        scalar2=None, op0=OP.mult)
                ms2 = mt2[:, 2:3]
                nc.vector.tensor_tensor(out=ms2, in0=m2c, in1=m2c, op=OP.mult)
                v2 = mt2[:, 3:4]
                nc.vector.tensor_tensor(out=v2, in0=e2c, in1=ms2, op=OP.subtract)
                lv2 = mt2[:, 4:5]
                nc.scalar.activation(out=lv2, in_=v2, func=AF.Ln,
                                     bias=eps_t[0:F, :], scale=1.0)
                r02 = mt2[:, 5:6]
                nc.scalar.activation(out=r02, in_=lv2, func=AF.Exp, scale=-0.5)
                vpe2 = mt2[:, 6:7]
                nc.vector.tensor_scalar(out=vpe2, in0=v2, scalar1=eps_t[0:F, :],
                                        scalar2=0.5, op0=OP.add, op1=OP.mult)
                r0q2 = mt2[:, 7:8]
                nc.vector.tensor_tensor(out=r0q2, in0=r02, in1=r02, op=OP.mult)
                nc.vector.tensor_tensor(out=r0q2, in0=r0q2, in1=vpe2, op=OP.mult)
                nc.vector.tensor_scalar(out=r0q2, in0=r0q2, scalar1=-1.0,
                                        scalar2=1.5, op0=OP.mult, op1=OP.add)
                r12 = mt2[:, 6:7]
                nc.vector.tensor_tensor(out=r12, in0=r02, in1=r0q2, op=OP.mult)
                s2c = s1p.tile([F, 1], F32, tag="s2c")
                nc.vector.tensor_tensor(out=s2c[:], in0=g2_t[:], in1=r12,
                                        op=OP.mult)
                t2c = s1p.tile([F, 1], F32, tag="t2c")
                nc.vector.tensor_tensor(out=t2c[:], in0=m2c, in1=s2c[:],
                                        op=OP.mult)
                nc.vector.scalar_tensor_tensor(out=t2c[:], in0=t2c[:],
                                               scalar=-1.0, in1=be2_t[:],
                                               op0=OP.mult, op1=OP.add)

                if not live((l, 6)):
                    break
                # pass C: x = softplus(x + s2*summed + t2) via ln(1+exp)
                with tc.tile_pool(name="pc", bufs=2) as pcp:
                    CW = 2048
                    for j in range(0, AP_, CW):
                        w = min(CW, AP_ - j)
                        pre = pcp.tile([F, CW], F32, tag="pre")
                        nc.vector.scalar_tensor_tensor(
                            out=pre[:, :w], in0=summed[:, j:j + w],
                            scalar=s2c[:, 0:1], in1=x_cm[:, j:j + w],
                            op0=OP.mult, op1=OP.add)
                        pex = pcp.tile([F, CW], F32, tag="pex")
                        nc.scalar.activation(out=pex[:, :w], in_=pre[:, :w],
                                             func=AF.Exp, bias=t2c[:], scale=1.0)
                        nc.scalar.activation(out=x_cm[:, j:j + w], in_=pex[:, :w],
                                             func=AF.Ln, bias=1.0, scale=1.0)
                if dbg_dump:
                    nc.gpsimd.dma_start(out=dbgsum[l, :, :], in_=summed[:, :])
                    nc.sync.dma_start(out=dbgx[l, :, :], in_=x_cm[:, :])

        for _ in range(1 if live((L, 0)) else 0):
            # ---------------- crystal pooling ----------------
            with tc.tile_pool(name="pool", bufs=4) as pp_, \
                 tc.tile_pool(name="poolacc", bufs=1, space="PSUM") as pacc, \
                 tc.tile_pool(name="poolps", bufs=2, space="PSUM") as pps:
                acc = pacc.tile([F, 512], F32, tag="pacc")
                for t in range(NT):
                    xps = pps.tile([TA, F], F32, tag="xps")
                    nc.tensor.transpose(out=xps[:], in_=x_cm[:, t * TA:(t + 1) * TA],
                                        identity=id_t[0:F, 0:F])
                    xrm = pp_.tile([TA, F], F16, tag="xrm")
                    nc.scalar.activation(out=xrm[:], in_=xps[:], func=AF.Copy)
                    oht = pp_.tile([128, 512], F16, tag="oht")
                    nc.sync.dma_start(out=oht[:], in_=pone[t, :, :])
                    nc.tensor.matmul(acc[:], lhsT=xrm[:], rhs=oht[:],
                                     start=(t == 0), stop=(t == NT - 1))
                # zero pool_in, then scatter our window rows
                for r_ in range(NCB // 128):
                    nc.sync.dma_start(out=pool_in[128 * r_:128 * (r_ + 1), :],
                                      in_=zt128[:, 0:F])
                sci = pp_.tile([128, 32], I16, tag="sci")
                nc.sync.dma_start(out=sci[:], in_=scidx[:, :])
                tc.strict_bb_all_engine_barrier()
                accS = pp_.tile([F, 512], F32, tag="accS")
                nc.vector.tensor_copy(out=accS[:], in_=acc[:])
                asb4 = pp_.tile([128, 4, F], F32, tag="asb4")
                for ci in range(4):
                    # [F, 128] crystal block -> row-major [128, F]
                    tps = pps.tile([128, F], F32, tag="tps")
                    nc.tensor.transpose(out=tps[:],
                                        in_=accS[:, ci * 128:(ci + 1) * 128],
                                        identity=id_t[0:F, 0:F])
                    nc.vector.tensor_copy(out=asb4[:, ci, :], in_=tps[:])
                # one SWDGE scatter-add of the 512-crystal window (row i of
                # the window lives at asb4[i%128, i//128, :]); queue follows
                # the global Pool-DMA lane counter like the gathers
                nc.gpsimd.dma_scatter_add(
                    pool_in[:, :], asb4[:], sci[:, :], 512, 512, F,
                    single_packet=False, queue_num=gather_q())
                tc.strict_bb_all_engine_barrier()
                nc.gpsimd.collective_compute(
                    "AllReduce", OP.add, replica_groups=rg,
                    ins=[pool_in[:, :]], outs=[pool_out[:, :]])
                tc.strict_bb_all_engine_barrier()

            if dbg_dump:
                nc.sync.dma_start(out=dbgpool[:, :], in_=pool_out[:, :])


        for _ in range(1 if live((L, 1)) else 0):
            # ---------------- head ----------------
            with tc.tile_pool(name="head", bufs=3) as hp, \
                 tc.tile_pool(name="headps", bufs=2, space="PSUM") as hps:
                crys = nc.alloc_sbuf_tensor("crys", [F, NCB], F16)
                wfc_t = hp.tile([F, H], F16, tag="wfc")
                nc.sync.dma_start(out=wfc_t[:], in_=w_fc[:, :])
                bfc_t = hp.tile([H, 1], F32, tag="bfc")
                nc.sync.dma_start(out=bfc_t[:], in_=b_fc[:, :])
                wout_t = hp.tile([H, 1], F16, tag="wout")
                nc.sync.dma_start(out=wout_t[:], in_=w_out[:, :])
                bout_t = hp.tile([1, 1], F32, tag="bout")
                nc.sync.dma_start(out=bout_t[:], in_=b_out[:, :])

                for r_ in range(NCB // 128):
                    pt = hp.tile([128, F], F32, tag="pt")
                    nc.sync.dma_start(out=pt[:], in_=pool_out[128 * r_:128 * (r_ + 1), :])
                    ic = hp.tile([128, 1], F32, tag="ic")
                    nc.sync.dma_start(out=ic[:], in_=invcnt[128 * r_:128 * (r_ + 1), :])
                    nc.vector.tensor_scalar(out=pt[:], in0=pt[:], scalar1=ic[:, 0:1],
                                            scalar2=None, op0=OP.mult)
                    pex2 = hp.tile([128, F], F32, tag="pex2")
                    nc.scalar.activation(out=pex2[:], in_=pt[:], func=AF.Exp)
                    spt = hp.tile([128, F], F32, tag="spt")
                    nc.scalar.activation(out=spt[:], in_=pex2[:], func=AF.Ln,
                                         bias=1.0, scale=1.0)
                    tps = hps.tile([F, 128], F32, tag="tps")
                    nc.tensor.transpose(out=tps[:], in_=spt[:], identity=id_t[:, :])
                    nc.scalar.activation(out=crys[:, 128 * r_:128 * (r_ + 1)],
                                         in_=tps[:], func=AF.Copy)

                hc = nc.alloc_sbuf_tensor("hc", [H, NCB], F16)
                for j in range(0, NCB, 512):
                    psh = hps.tile([H, 512], F32, tag="psh")
                    nc.tensor.matmul(psh[:], lhsT=wfc_t[:], rhs=crys[:, j:j + 512],
                                     start=True, stop=True)
                    hex_ = hp.tile([H, 512], F32, tag="hex")
                    nc.scalar.activation(out=hex_[:], in_=psh[:],
                                         func=AF.Exp, bias=bfc_t[:], scale=1.0)
                    nc.scalar.activation(out=hc[:, j:j + 512], in_=hex_[:],
                                         func=AF.Ln, bias=1.0, scale=1.0)
                ofin = hp.tile([1, NCB], F32, tag="ofin")
                for j in range(0, NCB, 512):
                    pso = hps.tile([1, 512], F32, tag="pso")
                    nc.tensor.matmul(pso[:], lhsT=wout_t[:], rhs=hc[:, j:j + 512],
                                     start=True, stop=True)
                    nc.scalar.activation(out=ofin[:, j:j + 512], in_=pso[:],
                                         func=AF.Identity, bias=bout_t[:], scale=1.0)
                nc.sync.dma_start(out=out_t[:, :], in_=ofin[:])

    nc.compile()
    return nc


# --------------------------------------------------------------------------
# host-side input preparation
# --------------------------------------------------------------------------

def prepare_inputs(c, atom_fea, nbr_fea, nbr_fea_idx, crystal_atom_idx,
                   W_emb, b_emb, W_full, b_full, g1, be1, g2, be2,
                   W_fc, b_fc, W_out, b_out):
    N, M, F0, FB, F, H, NC, L = (c["N"], c["M"], c["F0"], c["FB"], c["F"],
                                 c["H"], c["NC"], c["NCONV"])
    G, TA, NT, AP_, ET, EL = (c["G"], c["TA"], c["ntile"], c["A_pad"],
                              c["E_tile"], c["E_loc"])
    W16, NCB, K = c["W16"], c["NCB"], c["NCORES"]
    AS = c["A_shard"]

    atom_fea = np.asarray(atom_fea, np.float32)
    nbr_fea = np.asarray(nbr_fea, np.float32)
    nbr_fea_idx = np.asarray(nbr_fea_idx, np.int64)
    crystal_atom_idx = np.asarray(crystal_atom_idx, np.int64)

    # shared (replicated) tensors
    oh = np.zeros((128, ET), np.float16)
    for j in range(ET):
        oh[j // M, j] = 1.0
    shared = {
        "oh_self": oh,
        "w_emb": np.asarray(W_emb, np.float16),
        "b_emb": np.asarray(b_emb, np.float32).reshape(F, 1),
        "w_self": np.asarray(W_full[:, :F, :], np.float16),
        "w_nbr": np.asarray(W_full[:, F:2 * F, :], np.float16),
        "w_b": np.asarray(W_full[:, 2 * F:, :]).astype(ml_dtypes.float8_e4m3),
        "g1": np.asarray(g1, np.float32).reshape(L, G, 1),
        "be1": np.asarray(be1, np.float32).reshape(L, G, 1),
        "g2": np.asarray(g2, np.float32).reshape(L, F, 1),
        "be2": np.asarray(be2, np.float32).reshape(L, F, 1),
        "w_fc": np.asarray(W_fc, np.float16),
        "b_fc": np.asarray(b_fc, np.float32).reshape(H, 1),
        "w_out": np.asarray(W_out, np.float16),
        "b_out": np.asarray(b_out, np.float32).reshape(1, 1),
        "ident": np.eye(128, dtype=np.float32),
        "identh": np.eye(128, dtype=np.float16),
    }
    # crystal counts (global, from index data only)
    cnt = np.bincount(crystal_atom_idx, minlength=NC).astype(np.float32)
    icnt = np.zeros((NCB, 1), np.float32)
    icnt[:NC, 0] = 1.0 / np.maximum(cnt, 1.0)
    shared["invcnt"] = icnt

    # b_full is mathematically irrelevant (cancelled by training-mode BN)

    in_maps = []
    for k in range(K):
        a0 = k * AS
        af = np.zeros((F0, AP_), np.float16)
        af[:, :AS] = atom_fea[a0:a0 + AS].T
        # edge ordering: e = a*M + m within each tile of TA atoms
        gi = np.zeros((NT * TA, M), np.int64)
        gi_raw = nbr_fea_idx[a0:a0 + AS]
        gi[:AS] = gi_raw
        valid = np.zeros((NT * TA, M), bool)
        valid[:AS] = True
        jj = (gi // AS) * AP_ + (gi % AS)          # padded-global atom index
        rows = np.where(valid, 1 + jj // 2, 0).astype(np.int64)
        par = np.where(valid, jj & 1, 0).astype(np.int64)
        idxw = np.zeros((128, NT, W16), np.int16)
        mparr = np.zeros((128, NT, M), np.int8)
        j = np.arange(ET)
        for t in range(NT):
            fl = rows[t * TA:(t + 1) * TA].reshape(ET)
            wrap = np.zeros((16, W16), np.int16)
            wrap[j % 16, j // 16] = fl
            idxw[:, t, :] = np.tile(wrap, (8, 1))
            # ge[p, i, :] holds edge i*128+p -> mask[p, i]
            mparr[:, t, :] = par[t * TA:(t + 1) * TA].reshape(ET)[
                (np.arange(M)[None, :] * 128 + np.arange(128)[:, None])]
        nb = np.zeros((FB, EL), ml_dtypes.float8_e4m3)
        nb_l = nbr_fea[a0:a0 + AS].reshape(AS * M, FB)
        src = np.zeros((NT * TA * M, FB), np.float32)
        src[:AS * M] = nb_l
        # src is already in (a, m) order; tiles are contiguous runs of ET
        nb[:, :] = src.T.astype(ml_dtypes.float8_e4m3)

        cry = np.zeros(NT * TA, np.int64)
        cry[:AS] = crystal_atom_idx[a0:a0 + AS]
        cb = int(crystal_atom_idx[a0:a0 + AS].min())
        cmax = int(crystal_atom_idx[a0:a0 + AS].max())
        assert cmax - cb < 512, f"crystal window too wide: {cmax - cb}"
        # pool one-hot: [atom (partition), crystal-window col]
        pone = np.zeros((NT, TA, 512), np.float16)
        for t in range(NT):
            for a in range(TA):
                ga = t * TA + a
                if ga >= AS:
                    continue
                pone[t, a, int(cry[ga]) - cb] = 1.0
        # scatter-add indices: window row j -> crystal cb+j, wrapped in 16
        # partitions and replicated (SWDGE idx convention)
        sw = np.zeros((16, 32), np.int16)
        jj512 = np.arange(512)
        sw[jj512 % 16, jj512 // 16] = (cb + jj512).astype(np.int16)
        scidx = np.tile(sw, (8, 1))
        assert cb + 512 <= NCB

        in_maps.append(dict(shared,
                            afT=af, idxw=idxw, mparr=mparr, nbrT=nb, pone=pone,
                            scidx=scidx))
    return in_maps


# --------------------------------------------------------------------------
# public entry point
# --------------------------------------------------------------------------

_PROG_CACHE = {}


def _get_program(c):
    key = tuple(sorted((k, v) for k, v in c.items()))
    if key not in _PROG_CACHE:
        _PROG_CACHE[key] = build_program(c)
    return _PROG_CACHE[key]


def kernel(atom_fea, nbr_fea, nbr_fea_idx, crystal_atom_idx, W_emb, b_emb,
           W_full, b_full, g1, be1, g2, be2, W_fc, b_fc, W_out, b_out,
           _trace=False):
    from concourse import bass_utils
    c = CFG
    nc = _get_program(c)
    in_maps = prepare_inputs(c, atom_fea, nbr_fea, nbr_fea_idx,
                             crystal_atom_idx, W_emb, b_emb, W_full, b_full,
                             g1, be1, g2, be2, W_fc, b_fc, W_out, b_out)
    res = bass_utils.run_bass_kernel_spmd(
        nc, in_maps, core_ids=list(range(c["NCORES"])), trace=_trace)
    out = np.asarray(res.results[0]["out"], np.float32)
    ret = out[0, :c["NC"]].reshape(c["NC"], 1)
    if _trace:
        return ret, res
    return ret



# revision 26
# speedup vs baseline: 1.8277x; 1.0657x over previous
"""CrystalGraphConvNet forward pass as a distributed Bass/Tile kernel on 8 TRN2
NeuronCores.

Strategy (graph/data parallel, per sharding hint):
  - Atoms sharded contiguously across 8 cores (7500 each, padded to 7552).
  - Per conv layer, each core computes Y_self = x @ Wf[:F], Y_nbr = x @ Wf[F:2F]
    for its atom shard; Y_nbr shards are AllGathered into a replicated f16
    table viewed as PAIR rows (two atoms = 512 B per row, plus a leading zero
    row), so a single int16-indexed dma_gather per tile fetches both parity
    candidates of every edge in one 512 B packet (row index = 1 + j//2).
  - The gather runs un-transposed (contiguous SBUF writes, edge-major):
    ge[p, i, 0:128] / [128:256] hold the even/odd atom of edge i*128+p.  A
    single copy_predicated with a tiny resident per-edge parity mask
    (broadcast along channels) selects the right atom; 12 accumulating
    transpose-matmuls against an f16 identity then fold the selected rows
    into the channel-major PSUM accumulator on the TensorEngine, on top of
    the nbr_fea projection and the one-hot self term.
  - Training-mode batchnorm needs global stats, so gated values are staged to
    DRAM scratch in f16 while per-channel sum/sumsq accumulate (scalar-engine
    copy/square with accum); a tiny AllReduce yields the affine.  The second
    pass is a SINGLE pass per tile pair: sigmoid is computed as
    1/(1+exp(-a)) with a DVE reciprocal, so every activation in the program
    lives in the one exp/ln table set and no ACT table reloads occur.
  - Crystal mean-pooling is one accumulating one-hot matmul per tile into a
    512-crystal window, scattered by int32 indirect DMA into a global crystal
    array and AllReduced; the tiny MLP head runs redundantly on every core.
"""

import math
import os
import ml_dtypes
import numpy as np

import concourse.bass as bass
import concourse.bacc as bacc
import concourse.tile as tile
from concourse import mybir
from contextlib import ExitStack


def _single_act_table(orig):
    """Route exp/ln to the one ACT table set that contains BOTH (the default
    chooser splits them across exp_and_others/natural_log, thrashing table
    reloads on every exp<->ln transition in the batchnorm affines).  Copy/
    Square/Identity stay in EVERY set so pass A and the sigmoid/softplus
    batches never force a reload for them; Sigmoid and Softplus live in their
    own sets and are batch-switched in pass B."""
    AF = mybir.ActivationFunctionType
    both = {AF.Exp, AF.Ln}
    shared = {AF.Exp, AF.Ln}
    home = None
    for name, funcs in orig.items():
        if both <= funcs:
            home = name
            break
    if home is None:
        return orig
    out = {}
    for name, funcs in orig.items():
        out[name] = funcs if name == home else (funcs - shared)
    return out


_orig_get_activation_tables = bacc.get_activation_tables


def _patched_get_activation_tables(arch):
    return _single_act_table(_orig_get_activation_tables(arch))


bacc.get_activation_tables = _patched_get_activation_tables

F16 = mybir.dt.float16
F32 = mybir.dt.float32
F8 = mybir.dt.float8e4
I8 = mybir.dt.int8
I16 = mybir.dt.int16
I32 = mybir.dt.int32


def make_cfg(N=60000, M=12, F0=92, FB=41, F=64, H=128, NC=2000, NCONV=3,
             EPS=1e-5, NCORES=8, TA=128):
    c = dict(N=N, M=M, F0=F0, FB=FB, F=F, H=H, NC=NC, NCONV=NCONV, EPS=EPS,
             NCORES=NCORES, TA=TA)
    assert N % NCORES == 0
    c["A_shard"] = N // NCORES
    c["ntile"] = (c["A_shard"] + TA - 1) // TA
    c["A_pad"] = c["ntile"] * TA
    c["E_tile"] = TA * M
    c["E_loc"] = c["ntile"] * c["E_tile"]
    assert (NCORES * c["A_pad"]) % 2 == 0
    c["R2"] = 1 + NCORES * c["A_pad"] // 2      # pair-table rows (zero row at 0)
    assert c["R2"] <= 32768, "pair table must stay int16-addressable"
    c["W16"] = c["E_tile"] // 16
    c["NCB"] = 512 * ((NC + 511) // 512) + 512  # crystal bounce rows
    c["G"] = 2 * F                              # gated channels
    return c


CFG = make_cfg()


# --------------------------------------------------------------------------
# program builder
# --------------------------------------------------------------------------

def build_program(c, debug=False, dbg_dump=False, stop=None):
    # stop: optional (layer, stage) tuple for bisection; stages within a layer:
    # 0=Y, 1=AG, 2=A, 3=AR1, 4=B, 5=AR2, 6=C; (L,0)=pool, (L,1)=head
    nc = bacc.Bacc("TRN2", target_bir_lowering=False, debug=debug,
                   num_devices=c["NCORES"], num_swdge_queues=4)

    N, M, F0, FB, F, H, NC, L = (c["N"], c["M"], c["F0"], c["FB"], c["F"],
                                 c["H"], c["NC"], c["NCONV"])
    G, TA, NT, AP_, ET, EL = (c["G"], c["TA"], c["ntile"], c["A_pad"],
                              c["E_tile"], c["E_loc"])
    R2, W16, NCB, EPS = c["R2"], c["W16"], c["NCB"], c["EPS"]
    NPAIR = (NT + 1) // 2
    NCHUNK = ET // TA                           # 128-edge chunks per tile

    # ---------------- inputs ----------------
    afT = nc.dram_tensor("afT", [F0, AP_], F16, kind="ExternalInput")
    idxw = nc.dram_tensor("idxw", [128, NT, W16], I16, kind="ExternalInput")
    mparr = nc.dram_tensor("mparr", [128, NT, M], I8, kind="ExternalInput")
    nbrT = nc.dram_tensor("nbrT", [FB, EL], F8, kind="ExternalInput")
    oh_self = nc.dram_tensor("oh_self", [128, ET], F16, kind="ExternalInput")
    pone = nc.dram_tensor("pone", [NT, 128, 512], F16, kind="ExternalInput")
    scidx = nc.dram_tensor("scidx", [128, 32], I16, kind="ExternalInput")
    invcnt = nc.dram_tensor("invcnt", [NCB, 1], F32, kind="ExternalInput")
    w_emb = nc.dram_tensor("w_emb", [F0, F], F16, kind="ExternalInput")
    b_emb = nc.dram_tensor("b_emb", [F, 1], F32, kind="ExternalInput")
    w_self = nc.dram_tensor("w_self", [L, F, G], F16, kind="ExternalInput")
    w_nbr = nc.dram_tensor("w_nbr", [L, F, G], F16, kind="ExternalInput")
    w_b = nc.dram_tensor("w_b", [L, FB, G], F8, kind="ExternalInput")
    g1 = nc.dram_tensor("g1", [L, G, 1], F32, kind="ExternalInput")
    be1 = nc.dram_tensor("be1", [L, G, 1], F32, kind="ExternalInput")
    g2 = nc.dram_tensor("g2", [L, F, 1], F32, kind="ExternalInput")
    be2 = nc.dram_tensor("be2", [L, F, 1], F32, kind="ExternalInput")
    w_fc = nc.dram_tensor("w_fc", [F, H], F16, kind="ExternalInput")
    b_fc = nc.dram_tensor("b_fc", [H, 1], F32, kind="ExternalInput")
    w_out = nc.dram_tensor("w_out", [H, 1], F16, kind="ExternalInput")
    b_out = nc.dram_tensor("b_out", [1, 1], F32, kind="ExternalInput")
    ident = nc.dram_tensor("ident", [128, 128], F32, kind="ExternalInput")
    identh = nc.dram_tensor("identh", [128, 128], F16, kind="ExternalInput")

    out_t = nc.dram_tensor("out", [1, NCB], F32, kind="ExternalOutput")
    if dbg_dump:
        L_, F_, G_, AP2, EL_ = c["NCONV"], c["F"], c["G"], c["A_pad"], c["E_loc"]
        dbgx0 = nc.dram_tensor("dbgx0", [F_, AP2], F32, kind="ExternalOutput")
        dbgx = nc.dram_tensor("dbgx", [L_, F_, AP2], F32, kind="ExternalOutput")
        dbgsum = nc.dram_tensor("dbgsum", [L_, F_, AP2], F32, kind="ExternalOutput")
        dbgst1 = nc.dram_tensor("dbgst1", [L_, G_, 2], F32, kind="ExternalOutput")
        dbggat = nc.dram_tensor("dbggat", [L_, NT, 128, ET], F8,
                                kind="ExternalOutput")
        dbgpool = nc.dram_tensor("dbgpool", [NCB, F_], F32, kind="ExternalOutput")

    # ---------------- internal DRAM ----------------
    yb = nc.dram_tensor("yb", [AP_, G], F16)                        # AG input bounce
    tbl = nc.dram_tensor("tbl", [R2, 2 * G], F16, addr_space="Shared")
    scr = nc.dram_tensor("scr", [NT, 128, ET], F8)
    ar1_in = nc.dram_tensor("ar1_in", [G, 2], F32)
    ar1_out = nc.dram_tensor("ar1_out", [G, 2], F32, addr_space="Shared")
    ar2_in = nc.dram_tensor("ar2_in", [F, 2], F32)
    ar2_out = nc.dram_tensor("ar2_out", [F, 2], F32, addr_space="Shared")
    pool_in = nc.dram_tensor("pool_in", [NCB, F], F32)
    pool_out = nc.dram_tensor("pool_out", [NCB, F], F32, addr_space="Shared")

    rg = [list(range(c["NCORES"]))]
    AF = mybir.ActivationFunctionType
    OP = mybir.AluOpType
    if stop is None:
        stop = (L, 9)

    # Tile cycles DMASW lane sems per Pool DMA in GLOBAL program order (%8);
    # each lane sem is queue-locked, so the SWDGE queue must follow the same
    # global counter (%4), not the per-layer tile index.
    gq = [0]

    def gather_q():
        q = gq[0] % 4
        gq[0] += 1
        return q

    def live(key):
        return key <= stop

    with tile.TileContext(nc) as tc, ExitStack() as top:
        # Tile assigns each Pool-engine DMA inst a DMASW lane (round-robin in
        # program order) and points consumer waits at that lane's semaphore.
        # A prepare_only gather bakes its completion sem into the descriptors
        # (pass 2 attaches no then_inc), so sem= MUST be the very lane sem the
        # tick pass will pick, tracked here with a mirror counter.
        swsems = tc.sems.swdge_block()
        pool_dma_lane = [0]

        def prep_slot():
            """(lane sem, queue) for the next prepared Pool DMA.  queue =
            lane % nqueues keeps every lane sem on one fixed queue (SWDGE
            sems are queue-locked)."""
            j = pool_dma_lane[0]
            pool_dma_lane[0] += 1
            return swsems[j % len(swsems)], (j % len(swsems)) % 4

        # persistent SBUF state
        x_cm = nc.alloc_sbuf_tensor("x_cm", [F, AP_], F32)
        summed = nc.alloc_sbuf_tensor("summed", [F, AP_], F16)
        ysr = nc.alloc_sbuf_tensor("ysr", [128, NT, G], F16)      # Y_self row-major
        idx_all = nc.alloc_sbuf_tensor("idx_all", [128, NT, W16], I16)
        mpar = nc.alloc_sbuf_tensor("mpar", [128, NT, M, 1], I8)

        const = top.enter_context(tc.tile_pool(name="const", bufs=1))
        stats = top.enter_context(tc.tile_pool(name="stats", bufs=1))

        # constants resident all kernel
        ohs_t = const.tile([128, ET], F16)
        nc.sync.dma_start(out=ohs_t[:], in_=oh_self[:, :])
        wemb_t = const.tile([F0, F], F16)
        nc.sync.dma_start(out=wemb_t[:], in_=w_emb[:, :])
        bemb_t = const.tile([F, 1], F32)
        nc.sync.dma_start(out=bemb_t[:], in_=b_emb[:, :])
        id_t = const.tile([128, 128], F32)
        nc.sync.dma_start(out=id_t[:], in_=ident[:, :])
        idh_t = const.tile([128, 128], F16)
        nc.sync.dma_start(out=idh_t[:], in_=identh[:, :])
        eps_t = const.tile([128, 1], F32)
        nc.vector.memset(eps_t[:], EPS)
        zrow = const.tile([1, 2 * G], F16)
        nc.vector.memset(zrow[:], 0.0)
        zt128 = const.tile([128, F], F32)
        nc.vector.memset(zt128[:], 0.0)

        # layer-invariant gather indices and parity masks
        nc.sync.dma_start(out=idx_all[:, :, :], in_=idxw[:, :, :])
        nc.sync.dma_start(out=mpar[:, :, :, 0], in_=mparr[:, :, :])

        wS = []
        wN = []
        wB = []
        for l in range(L):
            t1 = const.tile([F, G], F16, tag=f"wS{l}")
            nc.sync.dma_start(out=t1[:], in_=w_self[l, :, :])
            t2 = const.tile([F, G], F16, tag=f"wN{l}")
            nc.sync.dma_start(out=t2[:], in_=w_nbr[l, :, :])
            t3 = const.tile([FB, G], F8, tag=f"wB{l}")
            nc.sync.dma_start(out=t3[:], in_=w_b[l, :, :])
            wS.append(t1)
            wN.append(t2)
            wB.append(t3)

        # stats buffers
        st1_s = stats.tile([G, NT], F32, tag="st1s")
        st1_q = stats.tile([G, NT], F32, tag="st1q")
        st2_s = stats.tile([F, 2 * NPAIR], F32, tag="st2s")
        st2_q = stats.tile([F, 2 * NPAIR], F32, tag="st2q")

        # zero table guard row + summed pads
        nc.sync.dma_start(out=tbl[0:1, :], in_=zrow[:])
        nc.vector.memset(summed[:, :], 0.0)

        # ---------------- embedding: x = atom_fea @ W_emb + b_emb ----------
        with tc.tile_pool(name="emb", bufs=3) as embp, \
             tc.tile_pool(name="embps", bufs=2, space="PSUM") as embps:
            CH = 512
            for j in range(0, AP_, CH):
                w = min(CH, AP_ - j)
                rhs = embp.tile([F0, CH], F16, tag="embr")
                nc.sync.dma_start(out=rhs[:, :w], in_=afT[:, j:j + w])
                ps = embps.tile([F, CH], F32, tag="embp")
                nc.tensor.matmul(ps[:, :w], lhsT=wemb_t[:], rhs=rhs[:, :w],
                                 start=True, stop=True)
                nc.scalar.activation(out=x_cm[:, j:j + w], in_=ps[:, :w],
                                     func=AF.Identity, bias=bemb_t[:], scale=1.0)
        if dbg_dump:
            nc.sync.dma_start(out=dbgx0[:, :], in_=x_cm[:, :])

        # ---------------- conv layers ----------------
        for l in range(L):
            if not live((l, 0)):
                break
            # ---- phase Y: Y_self (SBUF) / Y_nbr (-> bounce -> AllGather) ----
            with tc.tile_pool(name="yph", bufs=3) as yp, \
                 tc.tile_pool(name="yps", bufs=2, space="PSUM") as yps:
                lastreal = c["A_shard"] - (NT - 1) * TA
                for t in range(NT):
                    xa = yp.tile([F, TA], F16, tag="xa")
                    nc.scalar.activation(out=xa[:], in_=x_cm[:, t * TA:(t + 1) * TA],
                                         func=AF.Copy)
                    psS = yps.tile([TA, G], F32, tag="psS")
                    nc.tensor.matmul(psS[:], lhsT=xa[:], rhs=wS[l][:],
                                     start=True, stop=True)
                    # pad atoms of the last tile must contribute exactly zero
                    # through the self one-hot matmul
                    nreal = TA if t < NT - 1 else lastreal
                    if nreal < TA:
                        nc.vector.memset(ysr[:, t, :], 0.0)
                    nc.scalar.activation(out=ysr[0:nreal, t, :],
                                         in_=psS[0:nreal, :], func=AF.Copy)
                    psN = yps.tile([TA, G], F32, tag="psN")
                    nc.tensor.matmul(psN[:], lhsT=xa[:], rhs=wN[l][:],
                                     start=True, stop=True)
                    yn = yp.tile([TA, G], F16, tag="yn")
                    nc.scalar.activation(out=yn[:], in_=psN[:], func=AF.Copy)
                    nc.sync.dma_start(out=yb[t * TA:(t + 1) * TA, :], in_=yn[:])

            if not live((l, 1)):
                break
            tc.strict_bb_all_engine_barrier()
            nc.gpsimd.collective_compute(
                "AllGather", OP.bypass, replica_groups=rg,
                ins=[yb[:, :]], outs=[tbl[1:R2, :]])
            tc.strict_bb_all_engine_barrier()

            if not live((l, 2)):
                break
            # ---- pass A: edges -> gated scratch + stats1 ----
            with tc.tile_pool(name="pa", bufs=6) as pa, \
                 tc.tile_pool(name="paps", bufs=2, space="PSUM") as paps:
                for t in range(NT):
                    nbt = pa.tile([FB, ET], F8, tag="nbt")
                    nc.sync.dma_start(out=nbt[:], in_=nbrT[:, t * ET:(t + 1) * ET])

                    ge = pa.tile([128, NCHUNK, 2 * G], F16, tag="ge")
                    # prepare_only decouples Q7 desc-gen from the transfer:
                    # the gather DMA fires at trigger time, so transfers on
                    # the 4 SWDGE queues overlap each other and compute
                    # instead of serializing on the Pool engine.
                    nc.gpsimd.dma_gather(ge[:], tbl[:, :], idx_all[:, t, :],
                                         ET, ET, 2 * G, single_packet=False,
                                         queue_num=gather_q())
                    # parity select: overwrite even-atom slab with odd-atom
                    # slab wherever the edge's target index is odd
                    nc.vector.copy_predicated(
                        out=ge[:, :, 0:G],
                        mask=mpar[:, t, :, :].broadcast_to([128, NCHUNK, G]),
                        data=ge[:, :, G:2 * G])

                    # every 128-col region: chunk transpose (start) -> wB ->
                    # one-hot self term (stop)
                    ps = paps.tile([G, ET], F32, tag="aps")
                    for i in range(NCHUNK):
                        cs = slice(i * 128, (i + 1) * 128)
                        nc.tensor.matmul(ps[:, cs], lhsT=ge[:, i, 0:G],
                                         rhs=idh_t[:], start=(i % 4 == 0),
                                         stop=False)
                    for s in range(ET // 512):
                        sl = slice(s * 512, (s + 1) * 512)
                        nc.tensor.matmul(ps[:, sl], lhsT=wB[l][:], rhs=nbt[:, sl],
                                         start=False, stop=False)
                        nc.tensor.matmul(ps[:, sl], lhsT=ysr[:, t, :],
                                         rhs=ohs_t[:, sl], start=False, stop=True)

                    gat = pa.tile([128, ET], F8, tag="gat")
                    nc.scalar.activation(out=gat[:], in_=ps[:], func=AF.Copy,
                                         accum_out=st1_s[:, t:t + 1])
                    sqd = pa.tile([128, ET], F16, tag="sqd")
                    nc.scalar.activation(out=sqd[:], in_=ps[:], func=AF.Square,
                                         accum_out=st1_q[:, t:t + 1])
                    nc.sync.dma_start(out=scr[t, :, :], in_=gat[:])

            if not live((l, 3)):
                break
            # ---- stats1 reduce + AllReduce + affine ----
            with tc.tile_pool(name="s1", bufs=1) as s1p:
                pack1 = s1p.tile([G, 2], F32, tag="pack1")
                nc.vector.tensor_reduce(out=pack1[:, 0:1], in_=st1_s[:],
                                        axis=mybir.AxisListType.X, op=OP.add)
                nc.vector.tensor_reduce(out=pack1[:, 1:2], in_=st1_q[:],
                                        axis=mybir.AxisListType.X, op=OP.add)
                nc.sync.dma_start(out=ar1_in[:, :], in_=pack1[:])
                tc.strict_bb_all_engine_barrier()
                nc.gpsimd.collective_compute(
                    "AllReduce", OP.add, replica_groups=rg,
                    ins=[ar1_in[:, :]], outs=[ar1_out[:, :]])
                tc.strict_bb_all_engine_barrier()

                red1 = s1p.tile([G, 2], F32, tag="red1")
                nc.sync.dma_start(out=red1[:], in_=ar1_out[:, :])
                if dbg_dump:
                    nc.sync.dma_start(out=dbgst1[l, :, :], in_=red1[:])
                    for t in range(NT):
                        nc.gpsimd.dma_start(out=dbggat[l, t, :, :],
                                            in_=scr[t, :, :])
                g1_t = s1p.tile([G, 1], F32, tag="g1t")
                nc.sync.dma_start(out=g1_t[:], in_=g1[l, :, :])
                be1_t = s1p.tile([G, 1], F32, tag="be1t")
                nc.sync.dma_start(out=be1_t[:], in_=be1[l, :, :])

                # rsqrt(var+eps) = exp(-0.5*ln(var+eps)) + one Newton step
                # (no sqrt table needed; Ln/Exp share one ACT table set)
                invE = 1.0 / float(N * M)
                mmt = s1p.tile([G, 8], F32, tag="mmt")
                mcol = mmt[:, 0:1]
                nc.vector.tensor_scalar(out=mcol, in0=red1[:, 0:1], scalar1=invE,
                                        scalar2=None, op0=OP.mult)
                ex2 = mmt[:, 1:2]
                nc.vector.tensor_scalar(out=ex2, in0=red1[:, 1:2], scalar1=invE,
                                        scalar2=None, op0=OP.mult)
                msq = mmt[:, 2:3]
                nc.vector.tensor_tensor(out=msq, in0=mcol, in1=mcol, op=OP.mult)
                var = mmt[:, 3:4]
                nc.vector.tensor_tensor(out=var, in0=ex2, in1=msq, op=OP.subtract)
                lv = mmt[:, 4:5]
                nc.scalar.activation(out=lv, in_=var, func=AF.Ln,
                                     bias=eps_t[0:G, :], scale=1.0)
                r0 = mmt[:, 5:6]
                nc.scalar.activation(out=r0, in_=lv, func=AF.Exp, scale=-0.5)
                # one Newton step: r1 = r0*(1.5 - 0.5*(var+eps)*r0^2)
                vpe = mmt[:, 6:7]
                nc.vector.tensor_scalar(out=vpe, in0=var, scalar1=eps_t[0:G, :],
                                        scalar2=0.5, op0=OP.add, op1=OP.mult)
                r0q = mmt[:, 7:8]
                nc.vector.tensor_tensor(out=r0q, in0=r0, in1=r0, op=OP.mult)
                nc.vector.tensor_tensor(out=r0q, in0=r0q, in1=vpe, op=OP.mult)
                nc.vector.tensor_scalar(out=r0q, in0=r0q, scalar1=-1.0,
                                        scalar2=1.5, op0=OP.mult, op1=OP.add)
                r1 = mmt[:, 6:7]
                nc.vector.tensor_tensor(out=r1, in0=r0, in1=r0q, op=OP.mult)

                s1c_ = s1p.tile([G, 1], F32, tag="s1c")
                nc.vector.tensor_tensor(out=s1c_[:], in0=g1_t[:], in1=r1,
                                        op=OP.mult)
                t1c_ = s1p.tile([G, 1], F32, tag="t1c")
                nc.vector.tensor_tensor(out=t1c_[:], in0=mcol, in1=s1c_[:],
                                        op=OP.mult)
                nc.vector.scalar_tensor_tensor(out=t1c_[:], in0=t1c_[:],
                                               scalar=-1.0, in1=be1_t[:],
                                               op0=OP.mult, op1=OP.add)
                # replicated (packed-pair) scale/bias
                sF = s1p.tile([128, 1], F32, tag="sF")
                tF = s1p.tile([128, 1], F32, tag="tF")
                sC = s1p.tile([128, 1], F32, tag="sC")
                tC = s1p.tile([128, 1], F32, tag="tC")
                for half in range(2):
                    hp = slice(half * F, half * F + F)
                    nc.sync.dma_start(out=sF[hp, :], in_=s1c_[0:F, :])
                    nc.sync.dma_start(out=tF[hp, :], in_=t1c_[0:F, :])
                    nc.sync.dma_start(out=sC[hp, :], in_=s1c_[F:G, :])
                    nc.sync.dma_start(out=tC[hp, :], in_=t1c_[F:G, :])

                if not live((l, 4)):
                    break
                # ---- pass B: sigmoid*softplus, neighbor-sum, stats2 ----
                # K pairs per chunk share each ACT table load: all K sigmoids
                # run under sigmoid_and_others, then all K softplus under
                # softplus_and_others (2 reloads per chunk instead of 2/pair)
                tc.strict_bb_all_engine_barrier()
                KP = 4
                with tc.tile_pool(name="pb", bufs=KP + 2) as bp:
                    for pc in range(0, NPAIR, KP):
                        chunk = range(pc, min(pc + KP, NPAIR))
                        zfs, zcs, sgs, sps = {}, {}, {}, {}
                        for p in chunk:
                            t0, t1_ = 2 * p, min(2 * p + 1, NT - 1)
                            single = (2 * p + 1 > NT - 1)
                            zf = bp.tile([128, ET], F8, tag="zf")
                            zc = bp.tile([128, ET], F8, tag="zc")
                            nc.sync.dma_start(out=zf[0:F, :], in_=scr[t0, 0:F, :])
                            nc.sync.dma_start(out=zc[0:F, :], in_=scr[t0, F:G, :])
                            if not single:
                                nc.sync.dma_start(out=zf[F:G, :],
                                                  in_=scr[t1_, 0:F, :])
                                nc.sync.dma_start(out=zc[F:G, :],
                                                  in_=scr[t1_, F:G, :])
                            zfs[p], zcs[p] = zf, zc
                        for p in chunk:
                            single = (2 * p + 1 > NT - 1)
                            pp = 128 if not single else F
                            sg = bp.tile([128, ET], F16, tag="sg")
                            nc.scalar.activation(out=sg[0:pp, :],
                                                 in_=zfs[p][0:pp, :],
                                                 func=AF.Sigmoid,
                                                 bias=tF[0:pp, :],
                                                 scale=sF[0:pp, :])
                            sgs[p] = sg
                        for p in chunk:
                            single = (2 * p + 1 > NT - 1)
                            pp = 128 if not single else F
                            ec = bp.tile([128, ET], F32, tag="ec")
                            nc.scalar.activation(out=ec[0:pp, :],
                                                 in_=zcs[p][0:pp, :],
                                                 func=AF.Exp,
                                                 bias=tC[0:pp, :],
                                                 scale=sC[0:pp, :])
                            sp = bp.tile([128, ET], F16, tag="sp")
                            nc.scalar.activation(out=sp[0:pp, :],
                                                 in_=ec[0:pp, :],
                                                 func=AF.Ln, bias=1.0,
                                                 scale=1.0)
                            sps[p] = sp
                        for p in chunk:
                            t0, t1_ = 2 * p, min(2 * p + 1, NT - 1)
                            single = (2 * p + 1 > NT - 1)
                            pp = 128 if not single else F
                            sp = sps[p]
                            nc.vector.tensor_tensor(out=sp[0:pp, :],
                                                    in0=sp[0:pp, :],
                                                    in1=sgs[p][0:pp, :],
                                                    op=OP.mult)
                            # one full-width M-reduce for both pair halves
                            red2 = bp.tile([128, TA], F32, tag="red2")
                            nc.vector.tensor_reduce(
                                out=red2[0:pp, :],
                                in_=sp[0:pp, :].rearrange("p (a m) -> p a m",
                                                          m=M),
                                axis=mybir.AxisListType.X, op=OP.add)
                            halves = 1 if single else 2
                            for hh in range(halves):
                                tt = t0 if hh == 0 else t1_
                                hs = slice(hh * F, hh * F + F)
                                # aligned copy rides the DVE; the shifted one
                                # hits a DVE slow path, keep on scalar
                                if hh == 0:
                                    nc.vector.tensor_copy(
                                        out=summed[:, tt * TA:(tt + 1) * TA],
                                        in_=red2[hs, :])
                                else:
                                    nc.scalar.activation(
                                        out=summed[:, tt * TA:(tt + 1) * TA],
                                        in_=red2[hs, :], func=AF.Copy)
                                # stats over real atoms only
                                nreal = min(c["A_shard"] - tt * TA, TA)
                                col = 2 * p + hh
                                nc.vector.tensor_reduce(
                                    out=st2_s[:, col:col + 1],
                                    in_=red2[hs, 0:nreal],
                                    axis=mybir.AxisListType.X, op=OP.add)
                                sqt = bp.tile([F, TA], F32, tag="sqt")
                                nc.scalar.activation(
                                    out=sqt[:, 0:nreal], in_=red2[hs, 0:nreal],
                                    func=AF.Square,
                                    accum_out=st2_q[:, col:col + 1])

                if not live((l, 5)):
                    break
                # ---- stats2 AllReduce + affine2 + pass C ----
                ncols = NT
                pack2 = s1p.tile([F, 2], F32, tag="pack2")
                nc.vector.tensor_reduce(out=pack2[:, 0:1],
                                        in_=st2_s[:, 0:ncols],
                                        axis=mybir.AxisListType.X, op=OP.add)
                nc.vector.tensor_reduce(out=pack2[:, 1:2],
                                        in_=st2_q[:, 0:ncols],
                                        axis=mybir.AxisListType.X, op=OP.add)
                nc.sync.dma_start(out=ar2_in[:, :], in_=pack2[:])
                tc.strict_bb_all_engine_barrier()
                nc.gpsimd.collective_compute(
                    "AllReduce", OP.add, replica_groups=rg,
                    ins=[ar2_in[:, :]], outs=[ar2_out[:, :]])
                tc.strict_bb_all_engine_barrier()

                red2 = s1p.tile([F, 2], F32, tag="red2")
                nc.sync.dma_start(out=red2[:], in_=ar2_out[:, :])
                g2_t = s1p.tile([F, 1], F32, tag="g2t")
                nc.sync.dma_start(out=g2_t[:], in_=g2[l, :, :])
                be2_t = s1p.tile([F, 1], F32, tag="be2t")
                nc.sync.dma_start(out=be2_t[:], in_=be2[l, :, :])

                invN = 1.0 / float(N)
                mt2 = s1p.tile([F, 8], F32, tag="mt2")
                m2c = mt2[:, 0:1]
                nc.vector.tensor_scalar(out=m2c, in0=red2[:, 0:1], scalar1=invN,
                                        scalar2=None, op0=OP.mult)
                e2c = mt2[:, 1:2]
                nc.vector.tensor_scalar(out=e2c, in0=red2[:, 1:2], scalar1=invN,
                                        scalar2=None, op0=OP.mult)
                ms2 = mt2[:, 2:3]
                nc.vector.tensor_tensor(out=ms2, in0=m2c, in1=m2c, op=OP.mult)
                v2 = mt2[:, 3:4]
                nc.vector.tensor_tensor(out=v2, in0=e2c, in1=ms2, op=OP.subtract)
                lv2 = mt2[:, 4:5]
                nc.scalar.activation(out=lv2, in_=v2, func=AF.Ln,
                                     bias=eps_t[0:F, :], scale=1.0)
                r02 = mt2[:, 5:6]
                nc.scalar.activation(out=r02, in_=lv2, func=AF.Exp, scale=-0.5)
                vpe2 = mt2[:, 6:7]
                nc.vector.tensor_scalar(out=vpe2, in0=v2, scalar1=eps_t[0:F, :],
                                        scalar2=0.5, op0=OP.add, op1=OP.mult)
                r0q2 = mt2[:, 7:8]
                nc.vector.tensor_tensor(out=r0q2, in0=r02, in1=r02, op=OP.mult)
                nc.vector.tensor_tensor(out=r0q2, in0=r0q2, in1=vpe2, op=OP.mult)
                nc.vector.tensor_scalar(out=r0q2, in0=r0q2, scalar1=-1.0,
                                        scalar2=1.5, op0=OP.mult, op1=OP.add)
                r12 = mt2[:, 6:7]
                nc.vector.tensor_tensor(out=r12, in0=r02, in1=r0q2, op=OP.mult)
                s2c = s1p.tile([F, 1], F32, tag="s2c")
                nc.vector.tensor_tensor(out=s2c[:], in0=g2_t[:], in1=r12,
                                        op=OP.mult)
                t2c = s1p.tile([F, 1], F32, tag="t2c")
                nc.vector.tensor_tensor(out=t2c[:], in0=m2c, in1=s2c[:],
                                        op=OP.mult)
                nc.vector.scalar_tensor_tensor(out=t2c[:], in0=t2c[:],
                                               scalar=-1.0, in1=be2_t[:],
                                               op0=OP.mult, op1=OP.add)

                if not live((l, 6)):
                    break
                # pass C: x = softplus(x + s2*summed + t2) via ln(1+exp)
                with tc.tile_pool(name="pc", bufs=2) as pcp:
                    CW = 2048
                    for j in range(0, AP_, CW):
                        w = min(CW, AP_ - j)
                        pre = pcp.tile([F, CW], F32, tag="pre")
                        nc.vector.scalar_tensor_tensor(
                            out=pre[:, :w], in0=summed[:, j:j + w],
                            scalar=s2c[:, 0:1], in1=x_cm[:, j:j + w],
                            op0=OP.mult, op1=OP.add)
                        pex = pcp.tile([F, CW], F32, tag="pex")
                        nc.scalar.activation(out=pex[:, :w], in_=pre[:, :w],
                                             func=AF.Exp, bias=t2c[:], scale=1.0)
                        nc.scalar.activation(out=x_cm[:, j:j + w], in_=pex[:, :w],
                                             func=AF.Ln, bias=1.0, scale=1.0)
                if dbg_dump:
                    nc.gpsimd.dma_start(out=dbgsum[l, :, :], in_=summed[:, :])
                    nc.sync.dma_start(out=dbgx[l, :, :], in_=x_cm[:, :])

        for _ in range(1 if live((L, 0)) else 0):
            # ---------------- crystal pooling ----------------
            with tc.tile_pool(name="pool", bufs=4) as pp_, \
                 tc.tile_pool(name="poolacc", bufs=1, space="PSUM") as pacc, \
                 tc.tile_pool(name="poolps", bufs=2, space="PSUM") as pps:
                acc = pacc.tile([F, 512], F32, tag="pacc")
                for t in range(NT):
                    xps = pps.tile([TA, F], F32, tag="xps")
                    nc.tensor.transpose(out=xps[:], in_=x_cm[:, t * TA:(t + 1) * TA],
                                        identity=id_t[0:F, 0:F])
                    xrm = pp_.tile([TA, F], F16, tag="xrm")
                    nc.scalar.activation(out=xrm[:], in_=xps[:], func=AF.Copy)
                    oht = pp_.tile([128, 512], F16, tag="oht")
                    nc.sync.dma_start(out=oht[:], in_=pone[t, :, :])
                    nc.tensor.matmul(acc[:], lhsT=xrm[:], rhs=oht[:],
                                     start=(t == 0), stop=(t == NT - 1))
                # zero pool_in, then scatter our window rows
                for r_ in range(NCB // 128):
                    nc.sync.dma_start(out=pool_in[128 * r_:128 * (r_ + 1), :],
                                      in_=zt128[:, 0:F])
                sci = pp_.tile([128, 32], I16, tag="sci")
                nc.sync.dma_start(out=sci[:], in_=scidx[:, :])
                tc.strict_bb_all_engine_barrier()
                accS = pp_.tile([F, 512], F32, tag="accS")
                nc.vector.tensor_copy(out=accS[:], in_=acc[:])
                asb4 = pp_.tile([128, 4, F], F32, tag="asb4")
                for ci in range(4):
                    # [F, 128] crystal block -> row-major [128, F]
                    tps = pps.tile([128, F], F32, tag="tps")
                    nc.tensor.transpose(out=tps[:],
                                        in_=accS[:, ci * 128:(ci + 1) * 128],
                                        identity=id_t[0:F, 0:F])
                    nc.vector.tensor_copy(out=asb4[:, ci, :], in_=tps[:])
                # one SWDGE scatter-add of the 512-crystal window (row i of
                # the window lives at asb4[i%128, i//128, :]); queue follows
                # the global Pool-DMA lane counter like the gathers
                nc.gpsimd.dma_scatter_add(
                    pool_in[:, :], asb4[:], sci[:, :], 512, 512, F,
                    single_packet=False, queue_num=gather_q())
                tc.strict_bb_all_engine_barrier()
                nc.gpsimd.collective_compute(
                    "AllReduce", OP.add, replica_groups=rg,
                    ins=[pool_in[:, :]], outs=[pool_out[:, :]])
                tc.strict_bb_all_engine_barrier()

            if dbg_dump:
                nc.sync.dma_start(out=dbgpool[:, :], in_=pool_out[:, :])


        for _ in range(1 if live((L, 1)) else 0):
            # ---------------- head ----------------
            with tc.tile_pool(name="head", bufs=3) as hp, \
                 tc.tile_pool(name="headps", bufs=2, space="PSUM") as hps:
                crys = nc.alloc_sbuf_tensor("crys", [F, NCB], F16)
                wfc_t = hp.tile([F, H], F16, tag="wfc")
                nc.sync.dma_start(out=wfc_t[:], in_=w_fc[:, :])
                bfc_t = hp.tile([H, 1], F32, tag="bfc")
                nc.sync.dma_start(out=bfc_t[:], in_=b_fc[:, :])
                wout_t = hp.tile([H, 1], F16, tag="wout")
                nc.sync.dma_start(out=wout_t[:], in_=w_out[:, :])
                bout_t = hp.tile([1, 1], F32, tag="bout")
                nc.sync.dma_start(out=bout_t[:], in_=b_out[:, :])

                for r_ in range(NCB // 128):
                    pt = hp.tile([128, F], F32, tag="pt")
                    nc.sync.dma_start(out=pt[:], in_=pool_out[128 * r_:128 * (r_ + 1), :])
                    ic = hp.tile([128, 1], F32, tag="ic")
                    nc.sync.dma_start(out=ic[:], in_=invcnt[128 * r_:128 * (r_ + 1), :])
                    nc.vector.tensor_scalar(out=pt[:], in0=pt[:], scalar1=ic[:, 0:1],
                                            scalar2=None, op0=OP.mult)
                    pex2 = hp.tile([128, F], F32, tag="pex2")
                    nc.scalar.activation(out=pex2[:], in_=pt[:], func=AF.Exp)
                    spt = hp.tile([128, F], F32, tag="spt")
                    nc.scalar.activation(out=spt[:], in_=pex2[:], func=AF.Ln,
                                         bias=1.0, scale=1.0)
                    tps = hps.tile([F, 128], F32, tag="tps")
                    nc.tensor.transpose(out=tps[:], in_=spt[:], identity=id_t[:, :])
                    nc.scalar.activation(out=crys[:, 128 * r_:128 * (r_ + 1)],
                                         in_=tps[:], func=AF.Copy)

                hc = nc.alloc_sbuf_tensor("hc", [H, NCB], F16)
                for j in range(0, NCB, 512):
                    psh = hps.tile([H, 512], F32, tag="psh")
                    nc.tensor.matmul(psh[:], lhsT=wfc_t[:], rhs=crys[:, j:j + 512],
                                     start=True, stop=True)
                    hex_ = hp.tile([H, 512], F32, tag="hex")
                    nc.scalar.activation(out=hex_[:], in_=psh[:],
                                         func=AF.Exp, bias=bfc_t[:], scale=1.0)
                    nc.scalar.activation(out=hc[:, j:j + 512], in_=hex_[:],
                                         func=AF.Ln, bias=1.0, scale=1.0)
                ofin = hp.tile([1, NCB], F32, tag="ofin")
                for j in range(0, NCB, 512):
                    pso = hps.tile([1, 512], F32, tag="pso")
                    nc.tensor.matmul(pso[:], lhsT=wout_t[:], rhs=hc[:, j:j + 512],
                                     start=True, stop=True)
                    nc.scalar.activation(out=ofin[:, j:j + 512], in_=pso[:],
                                         func=AF.Identity, bias=bout_t[:], scale=1.0)
                nc.sync.dma_start(out=out_t[:, :], in_=ofin[:])

    nc.compile()
    return nc


# --------------------------------------------------------------------------
# host-side input preparation
# --------------------------------------------------------------------------

def prepare_inputs(c, atom_fea, nbr_fea, nbr_fea_idx, crystal_atom_idx,
                   W_emb, b_emb, W_full, b_full, g1, be1, g2, be2,
                   W_fc, b_fc, W_out, b_out):
    N, M, F0, FB, F, H, NC, L = (c["N"], c["M"], c["F0"], c["FB"], c["F"],
                                 c["H"], c["NC"], c["NCONV"])
    G, TA, NT, AP_, ET, EL = (c["G"], c["TA"], c["ntile"], c["A_pad"],
                              c["E_tile"], c["E_loc"])
    W16, NCB, K = c["W16"], c["NCB"], c["NCORES"]
    AS = c["A_shard"]

    atom_fea = np.asarray(atom_fea, np.float32)
    nbr_fea = np.asarray(nbr_fea, np.float32)
    nbr_fea_idx = np.asarray(nbr_fea_idx, np.int64)
    crystal_atom_idx = np.asarray(crystal_atom_idx, np.int64)

    # shared (replicated) tensors
    oh = np.zeros((128, ET), np.float16)
    for j in range(ET):
        oh[j // M, j] = 1.0
    shared = {
        "oh_self": oh,
        "w_emb": np.asarray(W_emb, np.float16),
        "b_emb": np.asarray(b_emb, np.float32).reshape(F, 1),
        "w_self": np.asarray(W_full[:, :F, :], np.float16),
        "w_nbr": np.asarray(W_full[:, F:2 * F, :], np.float16),
        "w_b": np.asarray(W_full[:, 2 * F:, :]).astype(ml_dtypes.float8_e4m3),
        "g1": np.asarray(g1, np.float32).reshape(L, G, 1),
        "be1": np.asarray(be1, np.float32).reshape(L, G, 1),
        "g2": np.asarray(g2, np.float32).reshape(L, F, 1),
        "be2": np.asarray(be2, np.float32).reshape(L, F, 1),
        "w_fc": np.asarray(W_fc, np.float16),
        "b_fc": np.asarray(b_fc, np.float32).reshape(H, 1),
        "w_out": np.asarray(W_out, np.float16),
        "b_out": np.asarray(b_out, np.float32).reshape(1, 1),
        "ident": np.eye(128, dtype=np.float32),
        "identh": np.eye(128, dtype=np.float16),
    }
    # crystal counts (global, from index data only)
    cnt = np.bincount(crystal_atom_idx, minlength=NC).astype(np.float32)
    icnt = np.zeros((NCB, 1), np.float32)
    icnt[:NC, 0] = 1.0 / np.maximum(cnt, 1.0)
    shared["invcnt"] = icnt

    # b_full is mathematically irrelevant (cancelled by training-mode BN)

    in_maps = []
    for k in range(K):
        a0 = k * AS
        af = np.zeros((F0, AP_), np.float16)
        af[:, :AS] = atom_fea[a0:a0 + AS].T
        # edge ordering: e = a*M + m within each tile of TA atoms
        gi = np.zeros((NT * TA, M), np.int64)
        gi_raw = nbr_fea_idx[a0:a0 + AS]
        gi[:AS] = gi_raw
        valid = np.zeros((NT * TA, M), bool)
        valid[:AS] = True
        jj = (gi // AS) * AP_ + (gi % AS)          # padded-global atom index
        rows = np.where(valid, 1 + jj // 2, 0).astype(np.int64)
        par = np.where(valid, jj & 1, 0).astype(np.int64)
        idxw = np.zeros((128, NT, W16), np.int16)
        mparr = np.zeros((128, NT, M), np.int8)
        j = np.arange(ET)
        for t in range(NT):
            fl = rows[t * TA:(t + 1) * TA].reshape(ET)
            wrap = np.zeros((16, W16), np.int16)
            wrap[j % 16, j // 16] = fl
            idxw[:, t, :] = np.tile(wrap, (8, 1))
            # ge[p, i, :] holds edge i*128+p -> mask[p, i]
            mparr[:, t, :] = par[t * TA:(t + 1) * TA].reshape(ET)[
                (np.arange(M)[None, :] * 128 + np.arange(128)[:, None])]
        nb = np.zeros((FB, EL), ml_dtypes.float8_e4m3)
        nb_l = nbr_fea[a0:a0 + AS].reshape(AS * M, FB)
        src = np.zeros((NT * TA * M, FB), np.float32)
        src[:AS * M] = nb_l
        # src is already in (a, m) order; tiles are contiguous runs of ET
        nb[:, :] = src.T.astype(ml_dtypes.float8_e4m3)

        cry = np.zeros(NT * TA, np.int64)
        cry[:AS] = crystal_atom_idx[a0:a0 + AS]
        cb = int(crystal_atom_idx[a0:a0 + AS].min())
        cmax = int(crystal_atom_idx[a0:a0 + AS].max())
        assert cmax - cb < 512, f"crystal window too wide: {cmax - cb}"
        # pool one-hot: [atom (partition), crystal-window col]
        pone = np.zeros((NT, TA, 512), np.float16)
        for t in range(NT):
            for a in range(TA):
                ga = t * TA + a
                if ga >= AS:
                    continue
                pone[t, a, int(cry[ga]) - cb] = 1.0
        # scatter-add indices: window row j -> crystal cb+j, wrapped in 16
        # partitions and replicated (SWDGE idx convention)
        sw = np.zeros((16, 32), np.int16)
        jj512 = np.arange(512)
        sw[jj512 % 16, jj512 // 16] = (cb + jj512).astype(np.int16)
        scidx = np.tile(sw, (8, 1))
        assert cb + 512 <= NCB

        in_maps.append(dict(shared,
                            afT=af, idxw=idxw, mparr=mparr, nbrT=nb, pone=pone,
                            scidx=scidx))
    return in_maps


# --------------------------------------------------------------------------
# public entry point
# --------------------------------------------------------------------------

_PROG_CACHE = {}


def _get_program(c):
    key = tuple(sorted((k, v) for k, v in c.items()))
    if key not in _PROG_CACHE:
        _PROG_CACHE[key] = build_program(c)
    return _PROG_CACHE[key]


def kernel(atom_fea, nbr_fea, nbr_fea_idx, crystal_atom_idx, W_emb, b_emb,
           W_full, b_full, g1, be1, g2, be2, W_fc, b_fc, W_out, b_out,
           _trace=False):
    from concourse import bass_utils
    c = CFG
    nc = _get_program(c)
    in_maps = prepare_inputs(c, atom_fea, nbr_fea, nbr_fea_idx,
                             crystal_atom_idx, W_emb, b_emb, W_full, b_full,
                             g1, be1, g2, be2, W_fc, b_fc, W_out, b_out)
    res = bass_utils.run_bass_kernel_spmd(
        nc, in_maps, core_ids=list(range(c["NCORES"])), trace=_trace)
    out = np.asarray(res.results[0]["out"], np.float32)
    ret = out[0, :c["NC"]].reshape(c["NC"], 1)
    if _trace:
        return ret, res
    return ret



# revision 27
# speedup vs baseline: 1.9164x; 1.0485x over previous
"""CrystalGraphConvNet forward pass as a distributed Bass/Tile kernel on 8 TRN2
NeuronCores.

Strategy (graph/data parallel, per sharding hint):
  - Atoms sharded contiguously across 8 cores (7500 each, padded to 7552).
  - Per conv layer, each core computes Y_self = x @ Wf[:F], Y_nbr = x @ Wf[F:2F]
    for its atom shard; Y_nbr shards are AllGathered into a replicated f16
    table viewed as PAIR rows (two atoms = 512 B per row, plus a leading zero
    row), so a single int16-indexed dma_gather per tile fetches both parity
    candidates of every edge in one 512 B packet (row index = 1 + j//2).
  - The gather runs un-transposed (contiguous SBUF writes, edge-major):
    ge[p, i, 0:128] / [128:256] hold the even/odd atom of edge i*128+p.  A
    single copy_predicated with a tiny resident per-edge parity mask
    (broadcast along channels) selects the right atom; 12 accumulating
    transpose-matmuls against an f16 identity then fold the selected rows
    into the channel-major PSUM accumulator on the TensorEngine, on top of
    the nbr_fea projection and the one-hot self term.
  - Training-mode batchnorm needs global stats, so gated values are staged to
    DRAM scratch in f16 while per-channel sum/sumsq accumulate (scalar-engine
    copy/square with accum); a tiny AllReduce yields the affine.  The second
    pass is a SINGLE pass per tile pair: sigmoid is computed as
    1/(1+exp(-a)) with a DVE reciprocal, so every activation in the program
    lives in the one exp/ln table set and no ACT table reloads occur.
  - Crystal mean-pooling is one accumulating one-hot matmul per tile into a
    512-crystal window, scattered by int32 indirect DMA into a global crystal
    array and AllReduced; the tiny MLP head runs redundantly on every core.
"""

import math
import os
import ml_dtypes
import numpy as np

import concourse.bass as bass
import concourse.bacc as bacc
import concourse.tile as tile
from concourse import mybir
from contextlib import ExitStack


def _single_act_table(orig):
    """Route exp/ln to the one ACT table set that contains BOTH (the default
    chooser splits them across exp_and_others/natural_log, thrashing table
    reloads on every exp<->ln transition in the batchnorm affines).  Copy/
    Square/Identity stay in EVERY set so pass A and the sigmoid/softplus
    batches never force a reload for them; Sigmoid and Softplus live in their
    own sets and are batch-switched in pass B."""
    AF = mybir.ActivationFunctionType
    both = {AF.Exp, AF.Ln}
    shared = {AF.Exp, AF.Ln}
    home = None
    for name, funcs in orig.items():
        if both <= funcs:
            home = name
            break
    if home is None:
        return orig
    out = {}
    for name, funcs in orig.items():
        out[name] = funcs if name == home else (funcs - shared)
    return out


_orig_get_activation_tables = bacc.get_activation_tables


def _patched_get_activation_tables(arch):
    return _single_act_table(_orig_get_activation_tables(arch))


bacc.get_activation_tables = _patched_get_activation_tables

F16 = mybir.dt.float16
F32 = mybir.dt.float32
F8 = mybir.dt.float8e4
I8 = mybir.dt.int8
I16 = mybir.dt.int16
I32 = mybir.dt.int32


def make_cfg(N=60000, M=12, F0=92, FB=41, F=64, H=128, NC=2000, NCONV=3,
             EPS=1e-5, NCORES=8, TA=128):
    c = dict(N=N, M=M, F0=F0, FB=FB, F=F, H=H, NC=NC, NCONV=NCONV, EPS=EPS,
             NCORES=NCORES, TA=TA)
    assert N % NCORES == 0
    c["A_shard"] = N // NCORES
    c["ntile"] = (c["A_shard"] + TA - 1) // TA
    c["A_pad"] = c["ntile"] * TA
    c["E_tile"] = TA * M
    c["E_loc"] = c["ntile"] * c["E_tile"]
    assert (NCORES * c["A_pad"]) % 2 == 0
    c["R2"] = 1 + NCORES * c["A_pad"] // 2      # pair-table rows (zero row at 0)
    assert c["R2"] <= 32768, "pair table must stay int16-addressable"
    c["W16"] = c["E_tile"] // 16
    c["NCB"] = 512 * ((NC + 511) // 512) + 512  # crystal bounce rows
    c["G"] = 2 * F                              # gated channels
    return c


CFG = make_cfg()


# --------------------------------------------------------------------------
# program builder
# --------------------------------------------------------------------------

def build_program(c, debug=False, dbg_dump=False, stop=None):
    # stop: optional (layer, stage) tuple for bisection; stages within a layer:
    # 0=Y, 1=AG, 2=A, 3=AR1, 4=B, 5=AR2, 6=C; (L,0)=pool, (L,1)=head
    nc = bacc.Bacc("TRN2", target_bir_lowering=False, debug=debug,
                   num_devices=c["NCORES"], num_swdge_queues=4)

    N, M, F0, FB, F, H, NC, L = (c["N"], c["M"], c["F0"], c["FB"], c["F"],
                                 c["H"], c["NC"], c["NCONV"])
    G, TA, NT, AP_, ET, EL = (c["G"], c["TA"], c["ntile"], c["A_pad"],
                              c["E_tile"], c["E_loc"])
    R2, W16, NCB, EPS = c["R2"], c["W16"], c["NCB"], c["EPS"]
    NPAIR = (NT + 1) // 2
    NCHUNK = ET // TA                           # 128-edge chunks per tile

    # ---------------- inputs ----------------
    afT = nc.dram_tensor("afT", [F0, AP_], F16, kind="ExternalInput")
    idxw = nc.dram_tensor("idxw", [128, NT, W16], I16, kind="ExternalInput")
    mparr = nc.dram_tensor("mparr", [128, NT, M], I8, kind="ExternalInput")
    nbrT = nc.dram_tensor("nbrT", [FB, EL], F8, kind="ExternalInput")
    oh_self = nc.dram_tensor("oh_self", [128, ET], F16, kind="ExternalInput")
    pone = nc.dram_tensor("pone", [NT, 128, 512], F16, kind="ExternalInput")
    scidx = nc.dram_tensor("scidx", [128, 32], I16, kind="ExternalInput")
    invcnt = nc.dram_tensor("invcnt", [NCB, 1], F32, kind="ExternalInput")
    w_emb = nc.dram_tensor("w_emb", [F0, F], F16, kind="ExternalInput")
    b_emb = nc.dram_tensor("b_emb", [F, 1], F32, kind="ExternalInput")
    w_self = nc.dram_tensor("w_self", [L, F, G], F16, kind="ExternalInput")
    w_nbr = nc.dram_tensor("w_nbr", [L, F, G], F16, kind="ExternalInput")
    w_b = nc.dram_tensor("w_b", [L, FB, G], F8, kind="ExternalInput")
    g1 = nc.dram_tensor("g1", [L, G, 1], F32, kind="ExternalInput")
    be1 = nc.dram_tensor("be1", [L, G, 1], F32, kind="ExternalInput")
    g2 = nc.dram_tensor("g2", [L, F, 1], F32, kind="ExternalInput")
    be2 = nc.dram_tensor("be2", [L, F, 1], F32, kind="ExternalInput")
    w_fc = nc.dram_tensor("w_fc", [F, H], F16, kind="ExternalInput")
    b_fc = nc.dram_tensor("b_fc", [H, 1], F32, kind="ExternalInput")
    w_out = nc.dram_tensor("w_out", [H, 1], F16, kind="ExternalInput")
    b_out = nc.dram_tensor("b_out", [1, 1], F32, kind="ExternalInput")
    ident = nc.dram_tensor("ident", [128, 128], F32, kind="ExternalInput")
    identh = nc.dram_tensor("identh", [128, 128], F16, kind="ExternalInput")

    out_t = nc.dram_tensor("out", [1, NCB], F32, kind="ExternalOutput")
    if dbg_dump:
        L_, F_, G_, AP2, EL_ = c["NCONV"], c["F"], c["G"], c["A_pad"], c["E_loc"]
        dbgx0 = nc.dram_tensor("dbgx0", [F_, AP2], F32, kind="ExternalOutput")
        dbgx = nc.dram_tensor("dbgx", [L_, F_, AP2], F32, kind="ExternalOutput")
        dbgsum = nc.dram_tensor("dbgsum", [L_, F_, AP2], F32, kind="ExternalOutput")
        dbgst1 = nc.dram_tensor("dbgst1", [L_, G_, 2], F32, kind="ExternalOutput")
        dbggat = nc.dram_tensor("dbggat", [L_, NT, 128, ET], F8,
                                kind="ExternalOutput")
        dbgpool = nc.dram_tensor("dbgpool", [NCB, F_], F32, kind="ExternalOutput")

    # ---------------- internal DRAM ----------------
    yb = nc.dram_tensor("yb", [AP_, G], F16)                        # AG input bounce
    tbl = nc.dram_tensor("tbl", [R2, 2 * G], F16, addr_space="Shared")
    scr = nc.dram_tensor("scr", [NT, 128, ET], F8)
    ar1_in = nc.dram_tensor("ar1_in", [G, 2], F32)
    ar1_out = nc.dram_tensor("ar1_out", [G, 2], F32, addr_space="Shared")
    ar2_in = nc.dram_tensor("ar2_in", [F, 2], F32)
    ar2_out = nc.dram_tensor("ar2_out", [F, 2], F32, addr_space="Shared")
    pool_in = nc.dram_tensor("pool_in", [NCB, F], F32)
    pool_out = nc.dram_tensor("pool_out", [NCB, F], F32, addr_space="Shared")

    rg = [list(range(c["NCORES"]))]
    AF = mybir.ActivationFunctionType
    OP = mybir.AluOpType
    if stop is None:
        stop = (L, 9)

    # Tile cycles DMASW lane sems per Pool DMA in GLOBAL program order (%8);
    # each lane sem is queue-locked, so the SWDGE queue must follow the same
    # global counter (%4), not the per-layer tile index.
    gq = [0]

    def gather_q():
        q = gq[0] % 4
        gq[0] += 1
        return q

    def live(key):
        return key <= stop

    with tile.TileContext(nc) as tc, ExitStack() as top:
        # Tile assigns each Pool-engine DMA inst a DMASW lane (round-robin in
        # program order) and points consumer waits at that lane's semaphore.
        # A prepare_only gather bakes its completion sem into the descriptors
        # (pass 2 attaches no then_inc), so sem= MUST be the very lane sem the
        # tick pass will pick, tracked here with a mirror counter.
        swsems = tc.sems.swdge_block()
        pool_dma_lane = [0]

        def prep_slot():
            """(lane sem, queue) for the next prepared Pool DMA.  queue =
            lane % nqueues keeps every lane sem on one fixed queue (SWDGE
            sems are queue-locked)."""
            j = pool_dma_lane[0]
            pool_dma_lane[0] += 1
            return swsems[j % len(swsems)], (j % len(swsems)) % 4

        # persistent SBUF state
        x_cm = nc.alloc_sbuf_tensor("x_cm", [F, AP_], F32)
        summed = nc.alloc_sbuf_tensor("summed", [F, AP_], F16)
        ysr = nc.alloc_sbuf_tensor("ysr", [128, NT, G], F16)      # Y_self row-major
        idx_all = nc.alloc_sbuf_tensor("idx_all", [128, NT, W16], I16)
        mpar = nc.alloc_sbuf_tensor("mpar", [128, NT, M, 1], I8)

        const = top.enter_context(tc.tile_pool(name="const", bufs=1))
        stats = top.enter_context(tc.tile_pool(name="stats", bufs=1))

        # constants resident all kernel
        ohs_t = const.tile([128, ET], F16)
        nc.sync.dma_start(out=ohs_t[:], in_=oh_self[:, :])
        wemb_t = const.tile([F0, F], F16)
        nc.sync.dma_start(out=wemb_t[:], in_=w_emb[:, :])
        bemb_t = const.tile([F, 1], F32)
        nc.sync.dma_start(out=bemb_t[:], in_=b_emb[:, :])
        id_t = const.tile([128, 128], F32)
        nc.sync.dma_start(out=id_t[:], in_=ident[:, :])
        idh_t = const.tile([128, 128], F16)
        nc.sync.dma_start(out=idh_t[:], in_=identh[:, :])
        eps_t = const.tile([128, 1], F32)
        nc.vector.memset(eps_t[:], EPS)
        zrow = const.tile([1, 2 * G], F16)
        nc.vector.memset(zrow[:], 0.0)
        zt128 = const.tile([128, F], F32)
        nc.vector.memset(zt128[:], 0.0)

        # layer-invariant gather indices and parity masks
        nc.sync.dma_start(out=idx_all[:, :, :], in_=idxw[:, :, :])
        nc.sync.dma_start(out=mpar[:, :, :, 0], in_=mparr[:, :, :])

        wS = []
        wN = []
        wB = []
        for l in range(L):
            t1 = const.tile([F, G], F16, tag=f"wS{l}")
            nc.sync.dma_start(out=t1[:], in_=w_self[l, :, :])
            t2 = const.tile([F, G], F16, tag=f"wN{l}")
            nc.sync.dma_start(out=t2[:], in_=w_nbr[l, :, :])
            t3 = const.tile([FB, G], F8, tag=f"wB{l}")
            nc.sync.dma_start(out=t3[:], in_=w_b[l, :, :])
            wS.append(t1)
            wN.append(t2)
            wB.append(t3)

        # stats buffers
        st1_s = stats.tile([G, NT], F32, tag="st1s")
        st1_q = stats.tile([G, NT], F32, tag="st1q")
        st2_s = stats.tile([F, 2 * NPAIR], F32, tag="st2s")
        st2_q = stats.tile([F, 2 * NPAIR], F32, tag="st2q")

        # zero table guard row + summed pads
        nc.sync.dma_start(out=tbl[0:1, :], in_=zrow[:])
        nc.vector.memset(summed[:, :], 0.0)

        # ---------------- embedding: x = atom_fea @ W_emb + b_emb ----------
        with tc.tile_pool(name="emb", bufs=3) as embp, \
             tc.tile_pool(name="embps", bufs=2, space="PSUM") as embps:
            CH = 512
            for j in range(0, AP_, CH):
                w = min(CH, AP_ - j)
                rhs = embp.tile([F0, CH], F16, tag="embr")
                nc.sync.dma_start(out=rhs[:, :w], in_=afT[:, j:j + w])
                ps = embps.tile([F, CH], F32, tag="embp")
                nc.tensor.matmul(ps[:, :w], lhsT=wemb_t[:], rhs=rhs[:, :w],
                                 start=True, stop=True)
                nc.scalar.activation(out=x_cm[:, j:j + w], in_=ps[:, :w],
                                     func=AF.Identity, bias=bemb_t[:], scale=1.0)
        if dbg_dump:
            nc.sync.dma_start(out=dbgx0[:, :], in_=x_cm[:, :])

        # ---------------- conv layers ----------------
        for l in range(L):
            if not live((l, 0)):
                break
            # ---- phase Y: Y_self (SBUF) / Y_nbr (-> bounce -> AllGather) ----
            with tc.tile_pool(name="yph", bufs=3) as yp, \
                 tc.tile_pool(name="yps", bufs=2, space="PSUM") as yps:
                lastreal = c["A_shard"] - (NT - 1) * TA
                for t in range(NT):
                    xa = yp.tile([F, TA], F16, tag="xa")
                    nc.scalar.activation(out=xa[:], in_=x_cm[:, t * TA:(t + 1) * TA],
                                         func=AF.Copy)
                    psS = yps.tile([TA, G], F32, tag="psS")
                    nc.tensor.matmul(psS[:], lhsT=xa[:], rhs=wS[l][:],
                                     start=True, stop=True)
                    # pad atoms of the last tile must contribute exactly zero
                    # through the self one-hot matmul
                    nreal = TA if t < NT - 1 else lastreal
                    if nreal < TA:
                        nc.vector.memset(ysr[:, t, :], 0.0)
                    nc.scalar.activation(out=ysr[0:nreal, t, :],
                                         in_=psS[0:nreal, :], func=AF.Copy)
                    psN = yps.tile([TA, G], F32, tag="psN")
                    nc.tensor.matmul(psN[:], lhsT=xa[:], rhs=wN[l][:],
                                     start=True, stop=True)
                    yn = yp.tile([TA, G], F16, tag="yn")
                    nc.scalar.activation(out=yn[:], in_=psN[:], func=AF.Copy)
                    nc.sync.dma_start(out=yb[t * TA:(t + 1) * TA, :], in_=yn[:])

            if not live((l, 1)):
                break
            tc.strict_bb_all_engine_barrier()
            nc.gpsimd.collective_compute(
                "AllGather", OP.bypass, replica_groups=rg,
                ins=[yb[:, :]], outs=[tbl[1:R2, :]])
            tc.strict_bb_all_engine_barrier()

            if not live((l, 2)):
                break
            # ---- pass A: edges -> gated scratch + stats1 ----
            with tc.tile_pool(name="pa", bufs=6) as pa, \
                 tc.tile_pool(name="paps", bufs=2, space="PSUM") as paps:
                for t in range(NT):
                    nbt = pa.tile([FB, ET], F8, tag="nbt")
                    nc.sync.dma_start(out=nbt[:], in_=nbrT[:, t * ET:(t + 1) * ET])

                    ge = pa.tile([128, NCHUNK, 2 * G], F16, tag="ge")
                    # prepare_only decouples Q7 desc-gen from the transfer:
                    # the gather DMA fires at trigger time, so transfers on
                    # the 4 SWDGE queues overlap each other and compute
                    # instead of serializing on the Pool engine.
                    nc.gpsimd.dma_gather(ge[:], tbl[:, :], idx_all[:, t, :],
                                         ET, ET, 2 * G, single_packet=False,
                                         queue_num=gather_q())
                    # parity select: overwrite even-atom slab with odd-atom
                    # slab wherever the edge's target index is odd
                    nc.vector.copy_predicated(
                        out=ge[:, :, 0:G],
                        mask=mpar[:, t, :, :].broadcast_to([128, NCHUNK, G]),
                        data=ge[:, :, G:2 * G])

                    # every 128-col region: chunk transpose (start) -> wB ->
                    # one-hot self term (stop)
                    ps = paps.tile([G, ET], F32, tag="aps")
                    for i in range(NCHUNK):
                        cs = slice(i * 128, (i + 1) * 128)
                        nc.tensor.matmul(ps[:, cs], lhsT=ge[:, i, 0:G],
                                         rhs=idh_t[:], start=(i % 4 == 0),
                                         stop=False)
                    for s in range(ET // 512):
                        sl = slice(s * 512, (s + 1) * 512)
                        nc.tensor.matmul(ps[:, sl], lhsT=wB[l][:], rhs=nbt[:, sl],
                                         start=False, stop=False)
                        nc.tensor.matmul(ps[:, sl], lhsT=ysr[:, t, :],
                                         rhs=ohs_t[:, sl], start=False, stop=True)

                    gat = pa.tile([128, ET], F8, tag="gat")
                    nc.scalar.activation(out=gat[:], in_=ps[:], func=AF.Copy,
                                         accum_out=st1_s[:, t:t + 1])
                    sqd = pa.tile([128, ET], F16, tag="sqd")
                    nc.scalar.activation(out=sqd[:], in_=ps[:], func=AF.Square,
                                         accum_out=st1_q[:, t:t + 1])
                    nc.sync.dma_start(out=scr[t, :, :], in_=gat[:])

            if not live((l, 3)):
                break
            # ---- stats1 reduce + AllReduce + affine ----
            with tc.tile_pool(name="s1", bufs=1) as s1p:
                pack1 = s1p.tile([G, 2], F32, tag="pack1")
                nc.vector.tensor_reduce(out=pack1[:, 0:1], in_=st1_s[:],
                                        axis=mybir.AxisListType.X, op=OP.add)
                nc.vector.tensor_reduce(out=pack1[:, 1:2], in_=st1_q[:],
                                        axis=mybir.AxisListType.X, op=OP.add)
                nc.sync.dma_start(out=ar1_in[:, :], in_=pack1[:])
                tc.strict_bb_all_engine_barrier()
                nc.gpsimd.collective_compute(
                    "AllReduce", OP.add, replica_groups=rg,
                    ins=[ar1_in[:, :]], outs=[ar1_out[:, :]])
                tc.strict_bb_all_engine_barrier()

                red1 = s1p.tile([G, 2], F32, tag="red1")
                nc.sync.dma_start(out=red1[:], in_=ar1_out[:, :])
                if dbg_dump:
                    nc.sync.dma_start(out=dbgst1[l, :, :], in_=red1[:])
                    for t in range(NT):
                        nc.gpsimd.dma_start(out=dbggat[l, t, :, :],
                                            in_=scr[t, :, :])
                g1_t = s1p.tile([G, 1], F32, tag="g1t")
                nc.sync.dma_start(out=g1_t[:], in_=g1[l, :, :])
                be1_t = s1p.tile([G, 1], F32, tag="be1t")
                nc.sync.dma_start(out=be1_t[:], in_=be1[l, :, :])

                # rsqrt(var+eps) = exp(-0.5*ln(var+eps)) + one Newton step
                # (no sqrt table needed; Ln/Exp share one ACT table set)
                invE = 1.0 / float(N * M)
                mmt = s1p.tile([G, 8], F32, tag="mmt")
                mcol = mmt[:, 0:1]
                nc.vector.tensor_scalar(out=mcol, in0=red1[:, 0:1], scalar1=invE,
                                        scalar2=None, op0=OP.mult)
                ex2 = mmt[:, 1:2]
                nc.vector.tensor_scalar(out=ex2, in0=red1[:, 1:2], scalar1=invE,
                                        scalar2=None, op0=OP.mult)
                msq = mmt[:, 2:3]
                nc.vector.tensor_tensor(out=msq, in0=mcol, in1=mcol, op=OP.mult)
                var = mmt[:, 3:4]
                nc.vector.tensor_tensor(out=var, in0=ex2, in1=msq, op=OP.subtract)
                lv = mmt[:, 4:5]
                nc.scalar.activation(out=lv, in_=var, func=AF.Ln,
                                     bias=eps_t[0:G, :], scale=1.0)
                r0 = mmt[:, 5:6]
                nc.scalar.activation(out=r0, in_=lv, func=AF.Exp, scale=-0.5)
                # one Newton step: r1 = r0*(1.5 - 0.5*(var+eps)*r0^2)
                vpe = mmt[:, 6:7]
                nc.vector.tensor_scalar(out=vpe, in0=var, scalar1=eps_t[0:G, :],
                                        scalar2=0.5, op0=OP.add, op1=OP.mult)
                r0q = mmt[:, 7:8]
                nc.vector.tensor_tensor(out=r0q, in0=r0, in1=r0, op=OP.mult)
                nc.vector.tensor_tensor(out=r0q, in0=r0q, in1=vpe, op=OP.mult)
                nc.vector.tensor_scalar(out=r0q, in0=r0q, scalar1=-1.0,
                                        scalar2=1.5, op0=OP.mult, op1=OP.add)
                r1 = mmt[:, 6:7]
                nc.vector.tensor_tensor(out=r1, in0=r0, in1=r0q, op=OP.mult)

                s1c_ = s1p.tile([G, 1], F32, tag="s1c")
                nc.vector.tensor_tensor(out=s1c_[:], in0=g1_t[:], in1=r1,
                                        op=OP.mult)
                t1c_ = s1p.tile([G, 1], F32, tag="t1c")
                nc.vector.tensor_tensor(out=t1c_[:], in0=mcol, in1=s1c_[:],
                                        op=OP.mult)
                nc.vector.scalar_tensor_tensor(out=t1c_[:], in0=t1c_[:],
                                               scalar=-1.0, in1=be1_t[:],
                                               op0=OP.mult, op1=OP.add)
                # replicated (packed-pair) scale/bias
                sF = s1p.tile([128, 1], F32, tag="sF")
                tF = s1p.tile([128, 1], F32, tag="tF")
                sC = s1p.tile([128, 1], F32, tag="sC")
                tC = s1p.tile([128, 1], F32, tag="tC")
                for half in range(2):
                    hp = slice(half * F, half * F + F)
                    nc.sync.dma_start(out=sF[hp, :], in_=s1c_[0:F, :])
                    nc.sync.dma_start(out=tF[hp, :], in_=t1c_[0:F, :])
                    nc.sync.dma_start(out=sC[hp, :], in_=s1c_[F:G, :])
                    nc.sync.dma_start(out=tC[hp, :], in_=t1c_[F:G, :])

                if not live((l, 4)):
                    break
                # ---- pass B: sigmoid*softplus, neighbor-sum, stats2 ----
                # K pairs per chunk share each ACT table load: all K sigmoids
                # run under sigmoid_and_others, then all K softplus under
                # softplus_and_others (2 reloads per chunk instead of 2/pair)
                tc.strict_bb_all_engine_barrier()
                KP = 4
                with tc.tile_pool(name="pb", bufs=KP + 2) as bp:
                    for pc in range(0, NPAIR, KP):
                        chunk = range(pc, min(pc + KP, NPAIR))
                        zfs, zcs, sgs, sps = {}, {}, {}, {}
                        for p in chunk:
                            t0, t1_ = 2 * p, min(2 * p + 1, NT - 1)
                            single = (2 * p + 1 > NT - 1)
                            zf = bp.tile([128, ET], F8, tag="zf")
                            zc = bp.tile([128, ET], F8, tag="zc")
                            nc.sync.dma_start(out=zf[0:F, :], in_=scr[t0, 0:F, :])
                            nc.sync.dma_start(out=zc[0:F, :], in_=scr[t0, F:G, :])
                            if not single:
                                nc.sync.dma_start(out=zf[F:G, :],
                                                  in_=scr[t1_, 0:F, :])
                                nc.sync.dma_start(out=zc[F:G, :],
                                                  in_=scr[t1_, F:G, :])
                            zfs[p], zcs[p] = zf, zc
                        for p in chunk:
                            single = (2 * p + 1 > NT - 1)
                            pp = 128 if not single else F
                            sg = bp.tile([128, ET], F16, tag="sg")
                            nc.scalar.activation(out=sg[0:pp, :],
                                                 in_=zfs[p][0:pp, :],
                                                 func=AF.Sigmoid,
                                                 bias=tF[0:pp, :],
                                                 scale=sF[0:pp, :])
                            sgs[p] = sg
                        for p in chunk:
                            single = (2 * p + 1 > NT - 1)
                            pp = 128 if not single else F
                            ec = bp.tile([128, ET], F32, tag="ec")
                            nc.scalar.activation(out=ec[0:pp, :],
                                                 in_=zcs[p][0:pp, :],
                                                 func=AF.Exp,
                                                 bias=tC[0:pp, :],
                                                 scale=sC[0:pp, :])
                            sp = bp.tile([128, ET], F16, tag="sp")
                            nc.scalar.activation(out=sp[0:pp, :],
                                                 in_=ec[0:pp, :],
                                                 func=AF.Ln, bias=1.0,
                                                 scale=1.0)
                            sps[p] = sp
                        for p in chunk:
                            t0, t1_ = 2 * p, min(2 * p + 1, NT - 1)
                            single = (2 * p + 1 > NT - 1)
                            pp = 128 if not single else F
                            sp = sps[p]
                            nc.vector.tensor_tensor(out=sp[0:pp, :],
                                                    in0=sp[0:pp, :],
                                                    in1=sgs[p][0:pp, :],
                                                    op=OP.mult)
                            # one full-width M-reduce for both pair halves
                            red2 = bp.tile([128, TA], F32, tag="red2")
                            nc.vector.tensor_reduce(
                                out=red2[0:pp, :],
                                in_=sp[0:pp, :].rearrange("p (a m) -> p a m",
                                                          m=M),
                                axis=mybir.AxisListType.X, op=OP.add)
                            halves = 1 if single else 2
                            for hh in range(halves):
                                tt = t0 if hh == 0 else t1_
                                hs = slice(hh * F, hh * F + F)
                                # aligned copy rides the DVE; the shifted one
                                # hits a DVE slow path, keep on scalar
                                if hh == 0:
                                    nc.vector.tensor_copy(
                                        out=summed[:, tt * TA:(tt + 1) * TA],
                                        in_=red2[hs, :])
                                else:
                                    nc.scalar.activation(
                                        out=summed[:, tt * TA:(tt + 1) * TA],
                                        in_=red2[hs, :], func=AF.Copy)
                                # stats over real atoms only
                                nreal = min(c["A_shard"] - tt * TA, TA)
                                col = 2 * p + hh
                                nc.vector.tensor_reduce(
                                    out=st2_s[:, col:col + 1],
                                    in_=red2[hs, 0:nreal],
                                    axis=mybir.AxisListType.X, op=OP.add)
                                sqt = bp.tile([F, TA], F32, tag="sqt")
                                nc.vector.tensor_tensor(
                                    out=sqt[:, 0:nreal], in0=red2[hs, 0:nreal],
                                    in1=red2[hs, 0:nreal], op=OP.mult)
                                nc.vector.tensor_reduce(
                                    out=st2_q[:, col:col + 1],
                                    in_=sqt[:, 0:nreal],
                                    axis=mybir.AxisListType.X, op=OP.add)

                if not live((l, 5)):
                    break
                # ---- stats2 AllReduce + affine2 + pass C ----
                ncols = NT
                pack2 = s1p.tile([F, 2], F32, tag="pack2")
                nc.vector.tensor_reduce(out=pack2[:, 0:1],
                                        in_=st2_s[:, 0:ncols],
                                        axis=mybir.AxisListType.X, op=OP.add)
                nc.vector.tensor_reduce(out=pack2[:, 1:2],
                                        in_=st2_q[:, 0:ncols],
                                        axis=mybir.AxisListType.X, op=OP.add)
                nc.sync.dma_start(out=ar2_in[:, :], in_=pack2[:])
                tc.strict_bb_all_engine_barrier()
                nc.gpsimd.collective_compute(
                    "AllReduce", OP.add, replica_groups=rg,
                    ins=[ar2_in[:, :]], outs=[ar2_out[:, :]])
                tc.strict_bb_all_engine_barrier()

                red2 = s1p.tile([F, 2], F32, tag="red2")
                nc.sync.dma_start(out=red2[:], in_=ar2_out[:, :])
                g2_t = s1p.tile([F, 1], F32, tag="g2t")
                nc.sync.dma_start(out=g2_t[:], in_=g2[l, :, :])
                be2_t = s1p.tile([F, 1], F32, tag="be2t")
                nc.sync.dma_start(out=be2_t[:], in_=be2[l, :, :])

                invN = 1.0 / float(N)
                mt2 = s1p.tile([F, 8], F32, tag="mt2")
                m2c = mt2[:, 0:1]
                nc.vector.tensor_scalar(out=m2c, in0=red2[:, 0:1], scalar1=invN,
                                        scalar2=None, op0=OP.mult)
                e2c = mt2[:, 1:2]
                nc.vector.tensor_scalar(out=e2c, in0=red2[:, 1:2], scalar1=invN,
                                        scalar2=None, op0=OP.mult)
                ms2 = mt2[:, 2:3]
                nc.vector.tensor_tensor(out=ms2, in0=m2c, in1=m2c, op=OP.mult)
                v2 = mt2[:, 3:4]
                nc.vector.tensor_tensor(out=v2, in0=e2c, in1=ms2, op=OP.subtract)
                lv2 = mt2[:, 4:5]
                nc.scalar.activation(out=lv2, in_=v2, func=AF.Ln,
                                     bias=eps_t[0:F, :], scale=1.0)
                r02 = mt2[:, 5:6]
                nc.scalar.activation(out=r02, in_=lv2, func=AF.Exp, scale=-0.5)
                vpe2 = mt2[:, 6:7]
                nc.vector.tensor_scalar(out=vpe2, in0=v2, scalar1=eps_t[0:F, :],
                                        scalar2=0.5, op0=OP.add, op1=OP.mult)
                r0q2 = mt2[:, 7:8]
                nc.vector.tensor_tensor(out=r0q2, in0=r02, in1=r02, op=OP.mult)
                nc.vector.tensor_tensor(out=r0q2, in0=r0q2, in1=vpe2, op=OP.mult)
                nc.vector.tensor_scalar(out=r0q2, in0=r0q2, scalar1=-1.0,
                                        scalar2=1.5, op0=OP.mult, op1=OP.add)
                r12 = mt2[:, 6:7]
                nc.vector.tensor_tensor(out=r12, in0=r02, in1=r0q2, op=OP.mult)
                s2c = s1p.tile([F, 1], F32, tag="s2c")
                nc.vector.tensor_tensor(out=s2c[:], in0=g2_t[:], in1=r12,
                                        op=OP.mult)
                t2c = s1p.tile([F, 1], F32, tag="t2c")
                nc.vector.tensor_tensor(out=t2c[:], in0=m2c, in1=s2c[:],
                                        op=OP.mult)
                nc.vector.scalar_tensor_tensor(out=t2c[:], in0=t2c[:],
                                               scalar=-1.0, in1=be2_t[:],
                                               op0=OP.mult, op1=OP.add)

                if not live((l, 6)):
                    break
                # pass C: x = softplus(x + s2*summed + t2) via ln(1+exp)
                with tc.tile_pool(name="pc", bufs=2) as pcp:
                    CW = 2048
                    for j in range(0, AP_, CW):
                        w = min(CW, AP_ - j)
                        pre = pcp.tile([F, CW], F32, tag="pre")
                        nc.vector.scalar_tensor_tensor(
                            out=pre[:, :w], in0=summed[:, j:j + w],
                            scalar=s2c[:, 0:1], in1=x_cm[:, j:j + w],
                            op0=OP.mult, op1=OP.add)
                        pex = pcp.tile([F, CW], F32, tag="pex")
                        nc.scalar.activation(out=pex[:, :w], in_=pre[:, :w],
                                             func=AF.Exp, bias=t2c[:], scale=1.0)
                        nc.scalar.activation(out=x_cm[:, j:j + w], in_=pex[:, :w],
                                             func=AF.Ln, bias=1.0, scale=1.0)
                if dbg_dump:
                    nc.gpsimd.dma_start(out=dbgsum[l, :, :], in_=summed[:, :])
                    nc.sync.dma_start(out=dbgx[l, :, :], in_=x_cm[:, :])

        for _ in range(1 if live((L, 0)) else 0):
            # ---------------- crystal pooling ----------------
            with tc.tile_pool(name="pool", bufs=4) as pp_, \
                 tc.tile_pool(name="poolacc", bufs=1, space="PSUM") as pacc, \
                 tc.tile_pool(name="poolps", bufs=2, space="PSUM") as pps:
                acc = pacc.tile([F, 512], F32, tag="pacc")
                for t in range(NT):
                    xps = pps.tile([TA, F], F32, tag="xps")
                    nc.tensor.transpose(out=xps[:], in_=x_cm[:, t * TA:(t + 1) * TA],
                                        identity=id_t[0:F, 0:F])
                    xrm = pp_.tile([TA, F], F16, tag="xrm")
                    nc.scalar.activation(out=xrm[:], in_=xps[:], func=AF.Copy)
                    oht = pp_.tile([128, 512], F16, tag="oht")
                    nc.sync.dma_start(out=oht[:], in_=pone[t, :, :])
                    nc.tensor.matmul(acc[:], lhsT=xrm[:], rhs=oht[:],
                                     start=(t == 0), stop=(t == NT - 1))
                # zero pool_in, then scatter our window rows
                for r_ in range(NCB // 128):
                    nc.sync.dma_start(out=pool_in[128 * r_:128 * (r_ + 1), :],
                                      in_=zt128[:, 0:F])
                sci = pp_.tile([128, 32], I16, tag="sci")
                nc.sync.dma_start(out=sci[:], in_=scidx[:, :])
                tc.strict_bb_all_engine_barrier()
                accS = pp_.tile([F, 512], F32, tag="accS")
                nc.vector.tensor_copy(out=accS[:], in_=acc[:])
                asb4 = pp_.tile([128, 4, F], F32, tag="asb4")
                for ci in range(4):
                    # [F, 128] crystal block -> row-major [128, F]
                    tps = pps.tile([128, F], F32, tag="tps")
                    nc.tensor.transpose(out=tps[:],
                                        in_=accS[:, ci * 128:(ci + 1) * 128],
                                        identity=id_t[0:F, 0:F])
                    nc.vector.tensor_copy(out=asb4[:, ci, :], in_=tps[:])
                # one SWDGE scatter-add of the 512-crystal window (row i of
                # the window lives at asb4[i%128, i//128, :]); queue follows
                # the global Pool-DMA lane counter like the gathers
                nc.gpsimd.dma_scatter_add(
                    pool_in[:, :], asb4[:], sci[:, :], 512, 512, F,
                    single_packet=False, queue_num=gather_q())
                tc.strict_bb_all_engine_barrier()
                nc.gpsimd.collective_compute(
                    "AllReduce", OP.add, replica_groups=rg,
                    ins=[pool_in[:, :]], outs=[pool_out[:, :]])
                tc.strict_bb_all_engine_barrier()

            if dbg_dump:
                nc.sync.dma_start(out=dbgpool[:, :], in_=pool_out[:, :])


        for _ in range(1 if live((L, 1)) else 0):
            # ---------------- head ----------------
            with tc.tile_pool(name="head", bufs=3) as hp, \
                 tc.tile_pool(name="headps", bufs=2, space="PSUM") as hps:
                crys = nc.alloc_sbuf_tensor("crys", [F, NCB], F16)
                wfc_t = hp.tile([F, H], F16, tag="wfc")
                nc.sync.dma_start(out=wfc_t[:], in_=w_fc[:, :])
                bfc_t = hp.tile([H, 1], F32, tag="bfc")
                nc.sync.dma_start(out=bfc_t[:], in_=b_fc[:, :])
                wout_t = hp.tile([H, 1], F16, tag="wout")
                nc.sync.dma_start(out=wout_t[:], in_=w_out[:, :])
                bout_t = hp.tile([1, 1], F32, tag="bout")
                nc.sync.dma_start(out=bout_t[:], in_=b_out[:, :])

                for r_ in range(NCB // 128):
                    pt = hp.tile([128, F], F32, tag="pt")
                    nc.sync.dma_start(out=pt[:], in_=pool_out[128 * r_:128 * (r_ + 1), :])
                    ic = hp.tile([128, 1], F32, tag="ic")
                    nc.sync.dma_start(out=ic[:], in_=invcnt[128 * r_:128 * (r_ + 1), :])
                    nc.vector.tensor_scalar(out=pt[:], in0=pt[:], scalar1=ic[:, 0:1],
                                            scalar2=None, op0=OP.mult)
                    pex2 = hp.tile([128, F], F32, tag="pex2")
                    nc.scalar.activation(out=pex2[:], in_=pt[:], func=AF.Exp)
                    spt = hp.tile([128, F], F32, tag="spt")
                    nc.scalar.activation(out=spt[:], in_=pex2[:], func=AF.Ln,
                                         bias=1.0, scale=1.0)
                    tps = hps.tile([F, 128], F32, tag="tps")
                    nc.tensor.transpose(out=tps[:], in_=spt[:], identity=id_t[:, :])
                    nc.scalar.activation(out=crys[:, 128 * r_:128 * (r_ + 1)],
                                         in_=tps[:], func=AF.Copy)

                hc = nc.alloc_sbuf_tensor("hc", [H, NCB], F16)
                for j in range(0, NCB, 512):
                    psh = hps.tile([H, 512], F32, tag="psh")
                    nc.tensor.matmul(psh[:], lhsT=wfc_t[:], rhs=crys[:, j:j + 512],
                                     start=True, stop=True)
                    hex_ = hp.tile([H, 512], F32, tag="hex")
                    nc.scalar.activation(out=hex_[:], in_=psh[:],
                                         func=AF.Exp, bias=bfc_t[:], scale=1.0)
                    nc.scalar.activation(out=hc[:, j:j + 512], in_=hex_[:],
                                         func=AF.Ln, bias=1.0, scale=1.0)
                ofin = hp.tile([1, NCB], F32, tag="ofin")
                for j in range(0, NCB, 512):
                    pso = hps.tile([1, 512], F32, tag="pso")
                    nc.tensor.matmul(pso[:], lhsT=wout_t[:], rhs=hc[:, j:j + 512],
                                     start=True, stop=True)
                    nc.scalar.activation(out=ofin[:, j:j + 512], in_=pso[:],
                                         func=AF.Identity, bias=bout_t[:], scale=1.0)
                nc.sync.dma_start(out=out_t[:, :], in_=ofin[:])

    nc.compile()
    return nc


# --------------------------------------------------------------------------
# host-side input preparation
# --------------------------------------------------------------------------

def prepare_inputs(c, atom_fea, nbr_fea, nbr_fea_idx, crystal_atom_idx,
                   W_emb, b_emb, W_full, b_full, g1, be1, g2, be2,
                   W_fc, b_fc, W_out, b_out):
    N, M, F0, FB, F, H, NC, L = (c["N"], c["M"], c["F0"], c["FB"], c["F"],
                                 c["H"], c["NC"], c["NCONV"])
    G, TA, NT, AP_, ET, EL = (c["G"], c["TA"], c["ntile"], c["A_pad"],
                              c["E_tile"], c["E_loc"])
    W16, NCB, K = c["W16"], c["NCB"], c["NCORES"]
    AS = c["A_shard"]

    atom_fea = np.asarray(atom_fea, np.float32)
    nbr_fea = np.asarray(nbr_fea, np.float32)
    nbr_fea_idx = np.asarray(nbr_fea_idx, np.int64)
    crystal_atom_idx = np.asarray(crystal_atom_idx, np.int64)

    # shared (replicated) tensors
    oh = np.zeros((128, ET), np.float16)
    for j in range(ET):
        oh[j // M, j] = 1.0
    shared = {
        "oh_self": oh,
        "w_emb": np.asarray(W_emb, np.float16),
        "b_emb": np.asarray(b_emb, np.float32).reshape(F, 1),
        "w_self": np.asarray(W_full[:, :F, :], np.float16),
        "w_nbr": np.asarray(W_full[:, F:2 * F, :], np.float16),
        "w_b": np.asarray(W_full[:, 2 * F:, :]).astype(ml_dtypes.float8_e4m3),
        "g1": np.asarray(g1, np.float32).reshape(L, G, 1),
        "be1": np.asarray(be1, np.float32).reshape(L, G, 1),
        "g2": np.asarray(g2, np.float32).reshape(L, F, 1),
        "be2": np.asarray(be2, np.float32).reshape(L, F, 1),
        "w_fc": np.asarray(W_fc, np.float16),
        "b_fc": np.asarray(b_fc, np.float32).reshape(H, 1),
        "w_out": np.asarray(W_out, np.float16),
        "b_out": np.asarray(b_out, np.float32).reshape(1, 1),
        "ident": np.eye(128, dtype=np.float32),
        "identh": np.eye(128, dtype=np.float16),
    }
    # crystal counts (global, from index data only)
    cnt = np.bincount(crystal_atom_idx, minlength=NC).astype(np.float32)
    icnt = np.zeros((NCB, 1), np.float32)
    icnt[:NC, 0] = 1.0 / np.maximum(cnt, 1.0)
    shared["invcnt"] = icnt

    # b_full is mathematically irrelevant (cancelled by training-mode BN)

    in_maps = []
    for k in range(K):
        a0 = k * AS
        af = np.zeros((F0, AP_), np.float16)
        af[:, :AS] = atom_fea[a0:a0 + AS].T
        # edge ordering: e = a*M + m within each tile of TA atoms
        gi = np.zeros((NT * TA, M), np.int64)
        gi_raw = nbr_fea_idx[a0:a0 + AS]
        gi[:AS] = gi_raw
        valid = np.zeros((NT * TA, M), bool)
        valid[:AS] = True
        jj = (gi // AS) * AP_ + (gi % AS)          # padded-global atom index
        rows = np.where(valid, 1 + jj // 2, 0).astype(np.int64)
        par = np.where(valid, jj & 1, 0).astype(np.int64)
        idxw = np.zeros((128, NT, W16), np.int16)
        mparr = np.zeros((128, NT, M), np.int8)
        j = np.arange(ET)
        for t in range(NT):
            fl = rows[t * TA:(t + 1) * TA].reshape(ET)
            wrap = np.zeros((16, W16), np.int16)
            wrap[j % 16, j // 16] = fl
            idxw[:, t, :] = np.tile(wrap, (8, 1))
            # ge[p, i, :] holds edge i*128+p -> mask[p, i]
            mparr[:, t, :] = par[t * TA:(t + 1) * TA].reshape(ET)[
                (np.arange(M)[None, :] * 128 + np.arange(128)[:, None])]
        nb = np.zeros((FB, EL), ml_dtypes.float8_e4m3)
        nb_l = nbr_fea[a0:a0 + AS].reshape(AS * M, FB)
        src = np.zeros((NT * TA * M, FB), np.float32)
        src[:AS * M] = nb_l
        # src is already in (a, m) order; tiles are contiguous runs of ET
        nb[:, :] = src.T.astype(ml_dtypes.float8_e4m3)

        cry = np.zeros(NT * TA, np.int64)
        cry[:AS] = crystal_atom_idx[a0:a0 + AS]
        cb = int(crystal_atom_idx[a0:a0 + AS].min())
        cmax = int(crystal_atom_idx[a0:a0 + AS].max())
        assert cmax - cb < 512, f"crystal window too wide: {cmax - cb}"
        # pool one-hot: [atom (partition), crystal-window col]
        pone = np.zeros((NT, TA, 512), np.float16)
        for t in range(NT):
            for a in range(TA):
                ga = t * TA + a
                if ga >= AS:
                    continue
                pone[t, a, int(cry[ga]) - cb] = 1.0
        # scatter-add indices: window row j -> crystal cb+j, wrapped in 16
        # partitions and replicated (SWDGE idx convention)
        sw = np.zeros((16, 32), np.int16)
        jj512 = np.arange(512)
        sw[jj512 % 16, jj512 // 16] = (cb + jj512).astype(np.int16)
        scidx = np.tile(sw, (8, 1))
        assert cb + 512 <= NCB

        in_maps.append(dict(shared,
                            afT=af, idxw=idxw, mparr=mparr, nbrT=nb, pone=pone,
                            scidx=scidx))
    return in_maps


# --------------------------------------------------------------------------
# public entry point
# --------------------------------------------------------------------------

_PROG_CACHE = {}


def _get_program(c):
    key = tuple(sorted((k, v) for k, v in c.items()))
    if key not in _PROG_CACHE:
        _PROG_CACHE[key] = build_program(c)
    return _PROG_CACHE[key]


def kernel(atom_fea, nbr_fea, nbr_fea_idx, crystal_atom_idx, W_emb, b_emb,
           W_full, b_full, g1, be1, g2, be2, W_fc, b_fc, W_out, b_out,
           _trace=False):
    from concourse import bass_utils
    c = CFG
    nc = _get_program(c)
    in_maps = prepare_inputs(c, atom_fea, nbr_fea, nbr_fea_idx,
                             crystal_atom_idx, W_emb, b_emb, W_full, b_full,
                             g1, be1, g2, be2, W_fc, b_fc, W_out, b_out)
    res = bass_utils.run_bass_kernel_spmd(
        nc, in_maps, core_ids=list(range(c["NCORES"])), trace=_trace)
    out = np.asarray(res.results[0]["out"], np.float32)
    ret = out[0, :c["NC"]].reshape(c["NC"], 1)
    if _trace:
        return ret, res
    return ret



# revision 28
# speedup vs baseline: 2.1367x; 1.1150x over previous
"""CrystalGraphConvNet forward pass as a distributed Bass/Tile kernel on 8 TRN2
NeuronCores.

Strategy (graph/data parallel, per sharding hint):
  - Atoms sharded contiguously across 8 cores (7500 each, padded to 7552).
  - Per conv layer, each core computes Y_self = x @ Wf[:F], Y_nbr = x @ Wf[F:2F]
    for its atom shard; Y_nbr shards are AllGathered into a replicated f16
    table viewed as PAIR rows (two atoms = 512 B per row, plus a leading zero
    row), so a single int16-indexed dma_gather per tile fetches both parity
    candidates of every edge in one 512 B packet (row index = 1 + j//2).
  - The gather runs un-transposed (contiguous SBUF writes, edge-major):
    ge[p, i, 0:128] / [128:256] hold the even/odd atom of edge i*128+p.  A
    single copy_predicated with a tiny resident per-edge parity mask
    (broadcast along channels) selects the right atom; 12 accumulating
    transpose-matmuls against an f16 identity then fold the selected rows
    into the channel-major PSUM accumulator on the TensorEngine, on top of
    the nbr_fea projection and the one-hot self term.
  - Training-mode batchnorm needs global stats, so gated values are staged to
    DRAM scratch in f16 while per-channel sum/sumsq accumulate (scalar-engine
    copy/square with accum); a tiny AllReduce yields the affine.  The second
    pass is a SINGLE pass per tile pair: sigmoid is computed as
    1/(1+exp(-a)) with a DVE reciprocal, so every activation in the program
    lives in the one exp/ln table set and no ACT table reloads occur.
  - Crystal mean-pooling is one accumulating one-hot matmul per tile into a
    512-crystal window, scattered by int32 indirect DMA into a global crystal
    array and AllReduced; the tiny MLP head runs redundantly on every core.
"""

import math
import os
import ml_dtypes
import numpy as np

import concourse.bass as bass
import concourse.bacc as bacc
import concourse.tile as tile
from concourse import mybir
from contextlib import ExitStack


def _single_act_table(orig):
    """Route exp/ln to the one ACT table set that contains BOTH (the default
    chooser splits them across exp_and_others/natural_log, thrashing table
    reloads on every exp<->ln transition in the batchnorm affines).  Copy/
    Square/Identity stay in EVERY set so pass A and the sigmoid/softplus
    batches never force a reload for them; Sigmoid and Softplus live in their
    own sets and are batch-switched in pass B."""
    AF = mybir.ActivationFunctionType
    both = {AF.Exp, AF.Ln}
    shared = {AF.Exp, AF.Ln}
    home = None
    for name, funcs in orig.items():
        if both <= funcs:
            home = name
            break
    if home is None:
        return orig
    out = {}
    for name, funcs in orig.items():
        out[name] = funcs if name == home else (funcs - shared)
    return out


_orig_get_activation_tables = bacc.get_activation_tables


def _patched_get_activation_tables(arch):
    return _single_act_table(_orig_get_activation_tables(arch))


bacc.get_activation_tables = _patched_get_activation_tables

F16 = mybir.dt.float16
F32 = mybir.dt.float32
F8 = mybir.dt.float8e4
I8 = mybir.dt.int8
I16 = mybir.dt.int16
I32 = mybir.dt.int32


def make_cfg(N=60000, M=12, F0=92, FB=41, F=64, H=128, NC=2000, NCONV=3,
             EPS=1e-5, NCORES=8, TA=128):
    c = dict(N=N, M=M, F0=F0, FB=FB, F=F, H=H, NC=NC, NCONV=NCONV, EPS=EPS,
             NCORES=NCORES, TA=TA)
    assert N % NCORES == 0
    c["A_shard"] = N // NCORES
    c["ntile"] = (c["A_shard"] + TA - 1) // TA
    c["A_pad"] = c["ntile"] * TA
    c["E_tile"] = TA * M
    c["E_loc"] = c["ntile"] * c["E_tile"]
    assert (NCORES * c["A_pad"]) % 2 == 0
    c["R2"] = 1 + NCORES * c["A_pad"] // 2      # pair-table rows (zero row at 0)
    assert c["R2"] <= 32768, "pair table must stay int16-addressable"
    c["W16"] = c["E_tile"] // 16
    c["NCB"] = 512 * ((NC + 511) // 512) + 512  # crystal bounce rows
    c["G"] = 2 * F                              # gated channels
    return c


CFG = make_cfg()


# --------------------------------------------------------------------------
# program builder
# --------------------------------------------------------------------------

def build_program(c, debug=False, dbg_dump=False, stop=None):
    # stop: optional (layer, stage) tuple for bisection; stages within a layer:
    # 0=Y, 1=AG, 2=A, 3=AR1, 4=B, 5=AR2, 6=C; (L,0)=pool, (L,1)=head
    nc = bacc.Bacc("TRN2", target_bir_lowering=False, debug=debug,
                   num_devices=c["NCORES"], num_swdge_queues=4)

    N, M, F0, FB, F, H, NC, L = (c["N"], c["M"], c["F0"], c["FB"], c["F"],
                                 c["H"], c["NC"], c["NCONV"])
    G, TA, NT, AP_, ET, EL = (c["G"], c["TA"], c["ntile"], c["A_pad"],
                              c["E_tile"], c["E_loc"])
    R2, W16, NCB, EPS = c["R2"], c["W16"], c["NCB"], c["EPS"]
    NPAIR = (NT + 1) // 2
    NCHUNK = ET // TA                           # 128-edge chunks per tile

    # ---------------- inputs ----------------
    afT = nc.dram_tensor("afT", [F0, AP_], F16, kind="ExternalInput")
    idxw = nc.dram_tensor("idxw", [128, NT, W16], I16, kind="ExternalInput")
    mparr = nc.dram_tensor("mparr", [128, NT, M], I8, kind="ExternalInput")
    nbrT = nc.dram_tensor("nbrT", [FB, EL], F8, kind="ExternalInput")
    oh_self = nc.dram_tensor("oh_self", [128, ET], F16, kind="ExternalInput")
    pone = nc.dram_tensor("pone", [NT, 128, 512], F16, kind="ExternalInput")
    scidx = nc.dram_tensor("scidx", [128, 32], I16, kind="ExternalInput")
    invcnt = nc.dram_tensor("invcnt", [NCB, 1], F32, kind="ExternalInput")
    w_emb = nc.dram_tensor("w_emb", [F0, F], F16, kind="ExternalInput")
    b_emb = nc.dram_tensor("b_emb", [F, 1], F32, kind="ExternalInput")
    w_self = nc.dram_tensor("w_self", [L, F, G], F16, kind="ExternalInput")
    w_nbr = nc.dram_tensor("w_nbr", [L, F, G], F16, kind="ExternalInput")
    w_b = nc.dram_tensor("w_b", [L, FB, G], F8, kind="ExternalInput")
    g1 = nc.dram_tensor("g1", [L, G, 1], F32, kind="ExternalInput")
    be1 = nc.dram_tensor("be1", [L, G, 1], F32, kind="ExternalInput")
    g2 = nc.dram_tensor("g2", [L, F, 1], F32, kind="ExternalInput")
    be2 = nc.dram_tensor("be2", [L, F, 1], F32, kind="ExternalInput")
    w_fc = nc.dram_tensor("w_fc", [F, H], F16, kind="ExternalInput")
    b_fc = nc.dram_tensor("b_fc", [H, 1], F32, kind="ExternalInput")
    w_out = nc.dram_tensor("w_out", [H, 1], F16, kind="ExternalInput")
    b_out = nc.dram_tensor("b_out", [1, 1], F32, kind="ExternalInput")
    ident = nc.dram_tensor("ident", [128, 128], F32, kind="ExternalInput")
    identh = nc.dram_tensor("identh", [128, 128], F16, kind="ExternalInput")

    out_t = nc.dram_tensor("out", [1, NCB], F32, kind="ExternalOutput")
    if dbg_dump:
        L_, F_, G_, AP2, EL_ = c["NCONV"], c["F"], c["G"], c["A_pad"], c["E_loc"]
        dbgx0 = nc.dram_tensor("dbgx0", [F_, AP2], F32, kind="ExternalOutput")
        dbgx = nc.dram_tensor("dbgx", [L_, F_, AP2], F32, kind="ExternalOutput")
        dbgsum = nc.dram_tensor("dbgsum", [L_, F_, AP2], F32, kind="ExternalOutput")
        dbgst1 = nc.dram_tensor("dbgst1", [L_, G_, 2], F32, kind="ExternalOutput")
        dbggat = nc.dram_tensor("dbggat", [L_, NT, 128, ET], F8,
                                kind="ExternalOutput")
        dbgpool = nc.dram_tensor("dbgpool", [NCB, F_], F32, kind="ExternalOutput")

    # ---------------- internal DRAM ----------------
    yb = nc.dram_tensor("yb", [AP_, G], F8)                        # AG input bounce
    tbl = nc.dram_tensor("tbl", [R2, 2 * G], F8, addr_space="Shared")
    scr = nc.dram_tensor("scr", [NT, 128, ET], F8)
    ar1_in = nc.dram_tensor("ar1_in", [G, 2], F32)
    ar1_out = nc.dram_tensor("ar1_out", [G, 2], F32, addr_space="Shared")
    ar2_in = nc.dram_tensor("ar2_in", [F, 2], F32)
    ar2_out = nc.dram_tensor("ar2_out", [F, 2], F32, addr_space="Shared")
    pool_in = nc.dram_tensor("pool_in", [NCB, F], F32)
    pool_out = nc.dram_tensor("pool_out", [NCB, F], F32, addr_space="Shared")

    rg = [list(range(c["NCORES"]))]
    AF = mybir.ActivationFunctionType
    OP = mybir.AluOpType
    if stop is None:
        stop = (L, 9)

    # Tile cycles DMASW lane sems per Pool DMA in GLOBAL program order (%8);
    # each lane sem is queue-locked, so the SWDGE queue must follow the same
    # global counter (%4), not the per-layer tile index.
    gq = [0]

    def gather_q():
        q = gq[0] % 4
        gq[0] += 1
        return q

    def live(key):
        return key <= stop

    with tile.TileContext(nc) as tc, ExitStack() as top:
        # Tile assigns each Pool-engine DMA inst a DMASW lane (round-robin in
        # program order) and points consumer waits at that lane's semaphore.
        # A prepare_only gather bakes its completion sem into the descriptors
        # (pass 2 attaches no then_inc), so sem= MUST be the very lane sem the
        # tick pass will pick, tracked here with a mirror counter.
        swsems = tc.sems.swdge_block()
        pool_dma_lane = [0]

        def prep_slot():
            """(lane sem, queue) for the next prepared Pool DMA.  queue =
            lane % nqueues keeps every lane sem on one fixed queue (SWDGE
            sems are queue-locked)."""
            j = pool_dma_lane[0]
            pool_dma_lane[0] += 1
            return swsems[j % len(swsems)], (j % len(swsems)) % 4

        # persistent SBUF state
        x_cm = nc.alloc_sbuf_tensor("x_cm", [F, AP_], F32)
        summed = nc.alloc_sbuf_tensor("summed", [F, AP_], F16)
        ysr = nc.alloc_sbuf_tensor("ysr", [128, NT, G], F16)      # Y_self row-major
        idx_all = nc.alloc_sbuf_tensor("idx_all", [128, NT, W16], I16)
        mpar = nc.alloc_sbuf_tensor("mpar", [128, NT, M, 1], I8)

        const = top.enter_context(tc.tile_pool(name="const", bufs=1))
        stats = top.enter_context(tc.tile_pool(name="stats", bufs=1))

        # constants resident all kernel
        ohs_t = const.tile([128, ET], F16)
        nc.sync.dma_start(out=ohs_t[:], in_=oh_self[:, :])
        wemb_t = const.tile([F0, F], F16)
        nc.sync.dma_start(out=wemb_t[:], in_=w_emb[:, :])
        bemb_t = const.tile([F, 1], F32)
        nc.sync.dma_start(out=bemb_t[:], in_=b_emb[:, :])
        id_t = const.tile([128, 128], F32)
        nc.sync.dma_start(out=id_t[:], in_=ident[:, :])
        idh_t = const.tile([128, 128], F16)
        nc.sync.dma_start(out=idh_t[:], in_=identh[:, :])
        idh8 = const.tile([128, 128], F8)
        nc.vector.tensor_copy(out=idh8[:], in_=idh_t[:])
        eps_t = const.tile([128, 1], F32)
        nc.vector.memset(eps_t[:], EPS)
        zrow = const.tile([1, 2 * G], F8)
        nc.vector.memset(zrow[:], 0.0)
        zt128 = const.tile([128, F], F32)
        nc.vector.memset(zt128[:], 0.0)

        # layer-invariant gather indices and parity masks
        nc.sync.dma_start(out=idx_all[:, :, :], in_=idxw[:, :, :])
        nc.sync.dma_start(out=mpar[:, :, :, 0], in_=mparr[:, :, :])

        wS = []
        wN = []
        wB = []
        for l in range(L):
            t1 = const.tile([F, G], F16, tag=f"wS{l}")
            nc.sync.dma_start(out=t1[:], in_=w_self[l, :, :])
            t2 = const.tile([F, G], F16, tag=f"wN{l}")
            nc.sync.dma_start(out=t2[:], in_=w_nbr[l, :, :])
            t3 = const.tile([FB, G], F8, tag=f"wB{l}")
            nc.sync.dma_start(out=t3[:], in_=w_b[l, :, :])
            wS.append(t1)
            wN.append(t2)
            wB.append(t3)

        # stats buffers
        st1_s = stats.tile([G, NT], F32, tag="st1s")
        st1_q = stats.tile([G, NT], F32, tag="st1q")
        st2_s = stats.tile([F, 2 * NPAIR], F32, tag="st2s")
        st2_q = stats.tile([F, 2 * NPAIR], F32, tag="st2q")

        # zero table guard row + summed pads
        nc.sync.dma_start(out=tbl[0:1, :], in_=zrow[:])
        nc.vector.memset(summed[:, :], 0.0)

        # ---------------- embedding: x = atom_fea @ W_emb + b_emb ----------
        with tc.tile_pool(name="emb", bufs=3) as embp, \
             tc.tile_pool(name="embps", bufs=2, space="PSUM") as embps:
            CH = 512
            for j in range(0, AP_, CH):
                w = min(CH, AP_ - j)
                rhs = embp.tile([F0, CH], F16, tag="embr")
                nc.sync.dma_start(out=rhs[:, :w], in_=afT[:, j:j + w])
                ps = embps.tile([F, CH], F32, tag="embp")
                nc.tensor.matmul(ps[:, :w], lhsT=wemb_t[:], rhs=rhs[:, :w],
                                 start=True, stop=True)
                nc.scalar.activation(out=x_cm[:, j:j + w], in_=ps[:, :w],
                                     func=AF.Identity, bias=bemb_t[:], scale=1.0)
        if dbg_dump:
            nc.sync.dma_start(out=dbgx0[:, :], in_=x_cm[:, :])

        # ---------------- conv layers ----------------
        for l in range(L):
            if not live((l, 0)):
                break
            # ---- phase Y: Y_self (SBUF) / Y_nbr (-> bounce -> AllGather) ----
            with tc.tile_pool(name="yph", bufs=3) as yp, \
                 tc.tile_pool(name="yps", bufs=2, space="PSUM") as yps:
                lastreal = c["A_shard"] - (NT - 1) * TA
                for t in range(NT):
                    xa = yp.tile([F, TA], F16, tag="xa")
                    nc.scalar.activation(out=xa[:], in_=x_cm[:, t * TA:(t + 1) * TA],
                                         func=AF.Copy)
                    psS = yps.tile([TA, G], F32, tag="psS")
                    nc.tensor.matmul(psS[:], lhsT=xa[:], rhs=wS[l][:],
                                     start=True, stop=True)
                    # pad atoms of the last tile must contribute exactly zero
                    # through the self one-hot matmul
                    nreal = TA if t < NT - 1 else lastreal
                    if nreal < TA:
                        nc.vector.memset(ysr[:, t, :], 0.0)
                    nc.scalar.activation(out=ysr[0:nreal, t, :],
                                         in_=psS[0:nreal, :], func=AF.Copy)
                    psN = yps.tile([TA, G], F32, tag="psN")
                    nc.tensor.matmul(psN[:], lhsT=xa[:], rhs=wN[l][:],
                                     start=True, stop=True)
                    yn = yp.tile([TA, G], F8, tag="yn")
                    nc.scalar.activation(out=yn[:], in_=psN[:], func=AF.Copy)
                    nc.sync.dma_start(out=yb[t * TA:(t + 1) * TA, :], in_=yn[:])

            if not live((l, 1)):
                break
            tc.strict_bb_all_engine_barrier()
            nc.gpsimd.collective_compute(
                "AllGather", OP.bypass, replica_groups=rg,
                ins=[yb[:, :]], outs=[tbl[1:R2, :]])
            tc.strict_bb_all_engine_barrier()

            if not live((l, 2)):
                break
            # ---- pass A: edges -> gated scratch + stats1 ----
            with tc.tile_pool(name="pa", bufs=6) as pa, \
                 tc.tile_pool(name="paps", bufs=2, space="PSUM") as paps:
                for t in range(NT):
                    nbt = pa.tile([FB, ET], F8, tag="nbt")
                    nc.sync.dma_start(out=nbt[:], in_=nbrT[:, t * ET:(t + 1) * ET])

                    ge = pa.tile([128, NCHUNK, 2 * G], F8, tag="ge")
                    # prepare_only decouples Q7 desc-gen from the transfer:
                    # the gather DMA fires at trigger time, so transfers on
                    # the 4 SWDGE queues overlap each other and compute
                    # instead of serializing on the Pool engine.
                    nc.gpsimd.dma_gather(ge[:], tbl[:, :], idx_all[:, t, :],
                                         ET, ET, 2 * G, single_packet=False,
                                         queue_num=gather_q())
                    # parity select: overwrite even-atom slab with odd-atom
                    # slab wherever the edge's target index is odd
                    nc.vector.copy_predicated(
                        out=ge[:, :, 0:G],
                        mask=mpar[:, t, :, :].broadcast_to([128, NCHUNK, G]),
                        data=ge[:, :, G:2 * G])

                    # every 128-col region: chunk transpose (start) -> wB ->
                    # one-hot self term (stop)
                    ps = paps.tile([G, ET], F32, tag="aps")
                    for i in range(NCHUNK):
                        cs = slice(i * 128, (i + 1) * 128)
                        nc.tensor.matmul(ps[:, cs], lhsT=ge[:, i, 0:G],
                                         rhs=idh8[:], start=(i % 4 == 0),
                                         stop=False)
                    for s in range(ET // 512):
                        sl = slice(s * 512, (s + 1) * 512)
                        nc.tensor.matmul(ps[:, sl], lhsT=wB[l][:], rhs=nbt[:, sl],
                                         start=False, stop=False)
                        nc.tensor.matmul(ps[:, sl], lhsT=ysr[:, t, :],
                                         rhs=ohs_t[:, sl], start=False, stop=True)

                    gat = pa.tile([128, ET], F8, tag="gat")
                    nc.scalar.activation(out=gat[:], in_=ps[:], func=AF.Copy,
                                         accum_out=st1_s[:, t:t + 1])
                    sqd = pa.tile([128, ET], F16, tag="sqd")
                    nc.scalar.activation(out=sqd[:], in_=ps[:], func=AF.Square,
                                         accum_out=st1_q[:, t:t + 1])
                    nc.sync.dma_start(out=scr[t, :, :], in_=gat[:])

            if not live((l, 3)):
                break
            # ---- stats1 reduce + AllReduce + affine ----
            with tc.tile_pool(name="s1", bufs=1) as s1p:
                pack1 = s1p.tile([G, 2], F32, tag="pack1")
                nc.vector.tensor_reduce(out=pack1[:, 0:1], in_=st1_s[:],
                                        axis=mybir.AxisListType.X, op=OP.add)
                nc.vector.tensor_reduce(out=pack1[:, 1:2], in_=st1_q[:],
                                        axis=mybir.AxisListType.X, op=OP.add)
                nc.sync.dma_start(out=ar1_in[:, :], in_=pack1[:])
                tc.strict_bb_all_engine_barrier()
                nc.gpsimd.collective_compute(
                    "AllReduce", OP.add, replica_groups=rg,
                    ins=[ar1_in[:, :]], outs=[ar1_out[:, :]])
                tc.strict_bb_all_engine_barrier()

                red1 = s1p.tile([G, 2], F32, tag="red1")
                nc.sync.dma_start(out=red1[:], in_=ar1_out[:, :])
                if dbg_dump:
                    nc.sync.dma_start(out=dbgst1[l, :, :], in_=red1[:])
                    for t in range(NT):
                        nc.gpsimd.dma_start(out=dbggat[l, t, :, :],
                                            in_=scr[t, :, :])
                g1_t = s1p.tile([G, 1], F32, tag="g1t")
                nc.sync.dma_start(out=g1_t[:], in_=g1[l, :, :])
                be1_t = s1p.tile([G, 1], F32, tag="be1t")
                nc.sync.dma_start(out=be1_t[:], in_=be1[l, :, :])

                # rsqrt(var+eps) = exp(-0.5*ln(var+eps)) + one Newton step
                # (no sqrt table needed; Ln/Exp share one ACT table set)
                invE = 1.0 / float(N * M)
                mmt = s1p.tile([G, 8], F32, tag="mmt")
                mcol = mmt[:, 0:1]
                nc.vector.tensor_scalar(out=mcol, in0=red1[:, 0:1], scalar1=invE,
                                        scalar2=None, op0=OP.mult)
                ex2 = mmt[:, 1:2]
                nc.vector.tensor_scalar(out=ex2, in0=red1[:, 1:2], scalar1=invE,
                                        scalar2=None, op0=OP.mult)
                msq = mmt[:, 2:3]
                nc.vector.tensor_tensor(out=msq, in0=mcol, in1=mcol, op=OP.mult)
                var = mmt[:, 3:4]
                nc.vector.tensor_tensor(out=var, in0=ex2, in1=msq, op=OP.subtract)
                lv = mmt[:, 4:5]
                nc.scalar.activation(out=lv, in_=var, func=AF.Ln,
                                     bias=eps_t[0:G, :], scale=1.0)
                r0 = mmt[:, 5:6]
                nc.scalar.activation(out=r0, in_=lv, func=AF.Exp, scale=-0.5)
                # one Newton step: r1 = r0*(1.5 - 0.5*(var+eps)*r0^2)
                vpe = mmt[:, 6:7]
                nc.vector.tensor_scalar(out=vpe, in0=var, scalar1=eps_t[0:G, :],
                                        scalar2=0.5, op0=OP.add, op1=OP.mult)
                r0q = mmt[:, 7:8]
                nc.vector.tensor_tensor(out=r0q, in0=r0, in1=r0, op=OP.mult)
                nc.vector.tensor_tensor(out=r0q, in0=r0q, in1=vpe, op=OP.mult)
                nc.vector.tensor_scalar(out=r0q, in0=r0q, scalar1=-1.0,
                                        scalar2=1.5, op0=OP.mult, op1=OP.add)
                r1 = mmt[:, 6:7]
                nc.vector.tensor_tensor(out=r1, in0=r0, in1=r0q, op=OP.mult)

                s1c_ = s1p.tile([G, 1], F32, tag="s1c")
                nc.vector.tensor_tensor(out=s1c_[:], in0=g1_t[:], in1=r1,
                                        op=OP.mult)
                t1c_ = s1p.tile([G, 1], F32, tag="t1c")
                nc.vector.tensor_tensor(out=t1c_[:], in0=mcol, in1=s1c_[:],
                                        op=OP.mult)
                nc.vector.scalar_tensor_tensor(out=t1c_[:], in0=t1c_[:],
                                               scalar=-1.0, in1=be1_t[:],
                                               op0=OP.mult, op1=OP.add)
                # replicated (packed-pair) scale/bias
                sF = s1p.tile([128, 1], F32, tag="sF")
                tF = s1p.tile([128, 1], F32, tag="tF")
                sC = s1p.tile([128, 1], F32, tag="sC")
                tC = s1p.tile([128, 1], F32, tag="tC")
                for half in range(2):
                    hp = slice(half * F, half * F + F)
                    nc.sync.dma_start(out=sF[hp, :], in_=s1c_[0:F, :])
                    nc.sync.dma_start(out=tF[hp, :], in_=t1c_[0:F, :])
                    nc.sync.dma_start(out=sC[hp, :], in_=s1c_[F:G, :])
                    nc.sync.dma_start(out=tC[hp, :], in_=t1c_[F:G, :])

                if not live((l, 4)):
                    break
                # ---- pass B: sigmoid*softplus, neighbor-sum, stats2 ----
                # K pairs per chunk share each ACT table load: all K sigmoids
                # run under sigmoid_and_others, then all K softplus under
                # softplus_and_others (2 reloads per chunk instead of 2/pair)
                tc.strict_bb_all_engine_barrier()
                KP = 4
                with tc.tile_pool(name="pb", bufs=KP + 2) as bp:
                    for pc in range(0, NPAIR, KP):
                        chunk = range(pc, min(pc + KP, NPAIR))
                        zfs, zcs, sgs, sps = {}, {}, {}, {}
                        for p in chunk:
                            t0, t1_ = 2 * p, min(2 * p + 1, NT - 1)
                            single = (2 * p + 1 > NT - 1)
                            zf = bp.tile([128, ET], F8, tag="zf")
                            zc = bp.tile([128, ET], F8, tag="zc")
                            nc.sync.dma_start(out=zf[0:F, :], in_=scr[t0, 0:F, :])
                            nc.sync.dma_start(out=zc[0:F, :], in_=scr[t0, F:G, :])
                            if not single:
                                nc.sync.dma_start(out=zf[F:G, :],
                                                  in_=scr[t1_, 0:F, :])
                                nc.sync.dma_start(out=zc[F:G, :],
                                                  in_=scr[t1_, F:G, :])
                            zfs[p], zcs[p] = zf, zc
                        for p in chunk:
                            single = (2 * p + 1 > NT - 1)
                            pp = 128 if not single else F
                            sg = bp.tile([128, ET], F16, tag="sg")
                            nc.scalar.activation(out=sg[0:pp, :],
                                                 in_=zfs[p][0:pp, :],
                                                 func=AF.Sigmoid,
                                                 bias=tF[0:pp, :],
                                                 scale=sF[0:pp, :])
                            sgs[p] = sg
                        for p in chunk:
                            single = (2 * p + 1 > NT - 1)
                            pp = 128 if not single else F
                            ec = bp.tile([128, ET], F32, tag="ec")
                            nc.scalar.activation(out=ec[0:pp, :],
                                                 in_=zcs[p][0:pp, :],
                                                 func=AF.Exp,
                                                 bias=tC[0:pp, :],
                                                 scale=sC[0:pp, :])
                            sp = bp.tile([128, ET], F16, tag="sp")
                            nc.scalar.activation(out=sp[0:pp, :],
                                                 in_=ec[0:pp, :],
                                                 func=AF.Ln, bias=1.0,
                                                 scale=1.0)
                            sps[p] = sp
                        for p in chunk:
                            t0, t1_ = 2 * p, min(2 * p + 1, NT - 1)
                            single = (2 * p + 1 > NT - 1)
                            pp = 128 if not single else F
                            sp = sps[p]
                            nc.vector.tensor_tensor(out=sp[0:pp, :],
                                                    in0=sp[0:pp, :],
                                                    in1=sgs[p][0:pp, :],
                                                    op=OP.mult)
                            # one full-width M-reduce for both pair halves
                            red2 = bp.tile([128, TA], F32, tag="red2")
                            nc.vector.tensor_reduce(
                                out=red2[0:pp, :],
                                in_=sp[0:pp, :].rearrange("p (a m) -> p a m",
                                                          m=M),
                                axis=mybir.AxisListType.X, op=OP.add)
                            halves = 1 if single else 2
                            for hh in range(halves):
                                tt = t0 if hh == 0 else t1_
                                hs = slice(hh * F, hh * F + F)
                                # aligned copy rides the DVE; the shifted one
                                # hits a DVE slow path, keep on scalar
                                if hh == 0:
                                    nc.vector.tensor_copy(
                                        out=summed[:, tt * TA:(tt + 1) * TA],
                                        in_=red2[hs, :])
                                else:
                                    nc.scalar.activation(
                                        out=summed[:, tt * TA:(tt + 1) * TA],
                                        in_=red2[hs, :], func=AF.Copy)
                                # stats over real atoms only
                                nreal = min(c["A_shard"] - tt * TA, TA)
                                col = 2 * p + hh
                                nc.vector.tensor_reduce(
                                    out=st2_s[:, col:col + 1],
                                    in_=red2[hs, 0:nreal],
                                    axis=mybir.AxisListType.X, op=OP.add)
                                sqt = bp.tile([F, TA], F32, tag="sqt")
                                nc.vector.tensor_tensor(
                                    out=sqt[:, 0:nreal], in0=red2[hs, 0:nreal],
                                    in1=red2[hs, 0:nreal], op=OP.mult)
                                nc.vector.tensor_reduce(
                                    out=st2_q[:, col:col + 1],
                                    in_=sqt[:, 0:nreal],
                                    axis=mybir.AxisListType.X, op=OP.add)

                if not live((l, 5)):
                    break
                # ---- stats2 AllReduce + affine2 + pass C ----
                ncols = NT
                pack2 = s1p.tile([F, 2], F32, tag="pack2")
                nc.vector.tensor_reduce(out=pack2[:, 0:1],
                                        in_=st2_s[:, 0:ncols],
                                        axis=mybir.AxisListType.X, op=OP.add)
                nc.vector.tensor_reduce(out=pack2[:, 1:2],
                                        in_=st2_q[:, 0:ncols],
                                        axis=mybir.AxisListType.X, op=OP.add)
                nc.sync.dma_start(out=ar2_in[:, :], in_=pack2[:])
                tc.strict_bb_all_engine_barrier()
                nc.gpsimd.collective_compute(
                    "AllReduce", OP.add, replica_groups=rg,
                    ins=[ar2_in[:, :]], outs=[ar2_out[:, :]])
                tc.strict_bb_all_engine_barrier()

                red2 = s1p.tile([F, 2], F32, tag="red2")
                nc.sync.dma_start(out=red2[:], in_=ar2_out[:, :])
                g2_t = s1p.tile([F, 1], F32, tag="g2t")
                nc.sync.dma_start(out=g2_t[:], in_=g2[l, :, :])
                be2_t = s1p.tile([F, 1], F32, tag="be2t")
                nc.sync.dma_start(out=be2_t[:], in_=be2[l, :, :])

                invN = 1.0 / float(N)
                mt2 = s1p.tile([F, 8], F32, tag="mt2")
                m2c = mt2[:, 0:1]
                nc.vector.tensor_scalar(out=m2c, in0=red2[:, 0:1], scalar1=invN,
                                        scalar2=None, op0=OP.mult)
                e2c = mt2[:, 1:2]
                nc.vector.tensor_scalar(out=e2c, in0=red2[:, 1:2], scalar1=invN,
                                        scalar2=None, op0=OP.mult)
                ms2 = mt2[:, 2:3]
                nc.vector.tensor_tensor(out=ms2, in0=m2c, in1=m2c, op=OP.mult)
                v2 = mt2[:, 3:4]
                nc.vector.tensor_tensor(out=v2, in0=e2c, in1=ms2, op=OP.subtract)
                lv2 = mt2[:, 4:5]
                nc.scalar.activation(out=lv2, in_=v2, func=AF.Ln,
                                     bias=eps_t[0:F, :], scale=1.0)
                r02 = mt2[:, 5:6]
                nc.scalar.activation(out=r02, in_=lv2, func=AF.Exp, scale=-0.5)
                vpe2 = mt2[:, 6:7]
                nc.vector.tensor_scalar(out=vpe2, in0=v2, scalar1=eps_t[0:F, :],
                                        scalar2=0.5, op0=OP.add, op1=OP.mult)
                r0q2 = mt2[:, 7:8]
                nc.vector.tensor_tensor(out=r0q2, in0=r02, in1=r02, op=OP.mult)
                nc.vector.tensor_tensor(out=r0q2, in0=r0q2, in1=vpe2, op=OP.mult)
                nc.vector.tensor_scalar(out=r0q2, in0=r0q2, scalar1=-1.0,
                                        scalar2=1.5, op0=OP.mult, op1=OP.add)
                r12 = mt2[:, 6:7]
                nc.vector.tensor_tensor(out=r12, in0=r02, in1=r0q2, op=OP.mult)
                s2c = s1p.tile([F, 1], F32, tag="s2c")
                nc.vector.tensor_tensor(out=s2c[:], in0=g2_t[:], in1=r12,
                                        op=OP.mult)
                t2c = s1p.tile([F, 1], F32, tag="t2c")
                nc.vector.tensor_tensor(out=t2c[:], in0=m2c, in1=s2c[:],
                                        op=OP.mult)
                nc.vector.scalar_tensor_tensor(out=t2c[:], in0=t2c[:],
                                               scalar=-1.0, in1=be2_t[:],
                                               op0=OP.mult, op1=OP.add)

                if not live((l, 6)):
                    break
                # pass C: x = softplus(x + s2*summed + t2) via ln(1+exp)
                with tc.tile_pool(name="pc", bufs=2) as pcp:
                    CW = 2048
                    for j in range(0, AP_, CW):
                        w = min(CW, AP_ - j)
                        pre = pcp.tile([F, CW], F32, tag="pre")
                        nc.vector.scalar_tensor_tensor(
                            out=pre[:, :w], in0=summed[:, j:j + w],
                            scalar=s2c[:, 0:1], in1=x_cm[:, j:j + w],
                            op0=OP.mult, op1=OP.add)
                        pex = pcp.tile([F, CW], F32, tag="pex")
                        nc.scalar.activation(out=pex[:, :w], in_=pre[:, :w],
                                             func=AF.Exp, bias=t2c[:], scale=1.0)
                        nc.scalar.activation(out=x_cm[:, j:j + w], in_=pex[:, :w],
                                             func=AF.Ln, bias=1.0, scale=1.0)
                if dbg_dump:
                    nc.gpsimd.dma_start(out=dbgsum[l, :, :], in_=summed[:, :])
                    nc.sync.dma_start(out=dbgx[l, :, :], in_=x_cm[:, :])

        for _ in range(1 if live((L, 0)) else 0):
            # ---------------- crystal pooling ----------------
            with tc.tile_pool(name="pool", bufs=4) as pp_, \
                 tc.tile_pool(name="poolacc", bufs=1, space="PSUM") as pacc, \
                 tc.tile_pool(name="poolps", bufs=2, space="PSUM") as pps:
                acc = pacc.tile([F, 512], F32, tag="pacc")
                for t in range(NT):
                    xps = pps.tile([TA, F], F32, tag="xps")
                    nc.tensor.transpose(out=xps[:], in_=x_cm[:, t * TA:(t + 1) * TA],
                                        identity=id_t[0:F, 0:F])
                    xrm = pp_.tile([TA, F], F16, tag="xrm")
                    nc.scalar.activation(out=xrm[:], in_=xps[:], func=AF.Copy)
                    oht = pp_.tile([128, 512], F16, tag="oht")
                    nc.sync.dma_start(out=oht[:], in_=pone[t, :, :])
                    nc.tensor.matmul(acc[:], lhsT=xrm[:], rhs=oht[:],
                                     start=(t == 0), stop=(t == NT - 1))
                # zero pool_in, then scatter our window rows
                for r_ in range(NCB // 128):
                    nc.sync.dma_start(out=pool_in[128 * r_:128 * (r_ + 1), :],
                                      in_=zt128[:, 0:F])
                sci = pp_.tile([128, 32], I16, tag="sci")
                nc.sync.dma_start(out=sci[:], in_=scidx[:, :])
                tc.strict_bb_all_engine_barrier()
                accS = pp_.tile([F, 512], F32, tag="accS")
                nc.vector.tensor_copy(out=accS[:], in_=acc[:])
                asb4 = pp_.tile([128, 4, F], F32, tag="asb4")
                for ci in range(4):
                    # [F, 128] crystal block -> row-major [128, F]
                    tps = pps.tile([128, F], F32, tag="tps")
                    nc.tensor.transpose(out=tps[:],
                                        in_=accS[:, ci * 128:(ci + 1) * 128],
                                        identity=id_t[0:F, 0:F])
                    nc.vector.tensor_copy(out=asb4[:, ci, :], in_=tps[:])
                # one SWDGE scatter-add of the 512-crystal window (row i of
                # the window lives at asb4[i%128, i//128, :]); queue follows
                # the global Pool-DMA lane counter like the gathers
                nc.gpsimd.dma_scatter_add(
                    pool_in[:, :], asb4[:], sci[:, :], 512, 512, F,
                    single_packet=False, queue_num=gather_q())
                tc.strict_bb_all_engine_barrier()
                nc.gpsimd.collective_compute(
                    "AllReduce", OP.add, replica_groups=rg,
                    ins=[pool_in[:, :]], outs=[pool_out[:, :]])
                tc.strict_bb_all_engine_barrier()

            if dbg_dump:
                nc.sync.dma_start(out=dbgpool[:, :], in_=pool_out[:, :])


        for _ in range(1 if live((L, 1)) else 0):
            # ---------------- head ----------------
            with tc.tile_pool(name="head", bufs=3) as hp, \
                 tc.tile_pool(name="headps", bufs=2, space="PSUM") as hps:
                crys = nc.alloc_sbuf_tensor("crys", [F, NCB], F16)
                wfc_t = hp.tile([F, H], F16, tag="wfc")
                nc.sync.dma_start(out=wfc_t[:], in_=w_fc[:, :])
                bfc_t = hp.tile([H, 1], F32, tag="bfc")
                nc.sync.dma_start(out=bfc_t[:], in_=b_fc[:, :])
                wout_t = hp.tile([H, 1], F16, tag="wout")
                nc.sync.dma_start(out=wout_t[:], in_=w_out[:, :])
                bout_t = hp.tile([1, 1], F32, tag="bout")
                nc.sync.dma_start(out=bout_t[:], in_=b_out[:, :])

                for r_ in range(NCB // 128):
                    pt = hp.tile([128, F], F32, tag="pt")
                    nc.sync.dma_start(out=pt[:], in_=pool_out[128 * r_:128 * (r_ + 1), :])
                    ic = hp.tile([128, 1], F32, tag="ic")
                    nc.sync.dma_start(out=ic[:], in_=invcnt[128 * r_:128 * (r_ + 1), :])
                    nc.vector.tensor_scalar(out=pt[:], in0=pt[:], scalar1=ic[:, 0:1],
                                            scalar2=None, op0=OP.mult)
                    pex2 = hp.tile([128, F], F32, tag="pex2")
                    nc.scalar.activation(out=pex2[:], in_=pt[:], func=AF.Exp)
                    spt = hp.tile([128, F], F32, tag="spt")
                    nc.scalar.activation(out=spt[:], in_=pex2[:], func=AF.Ln,
                                         bias=1.0, scale=1.0)
                    tps = hps.tile([F, 128], F32, tag="tps")
                    nc.tensor.transpose(out=tps[:], in_=spt[:], identity=id_t[:, :])
                    nc.scalar.activation(out=crys[:, 128 * r_:128 * (r_ + 1)],
                                         in_=tps[:], func=AF.Copy)

                hc = nc.alloc_sbuf_tensor("hc", [H, NCB], F16)
                for j in range(0, NCB, 512):
                    psh = hps.tile([H, 512], F32, tag="psh")
                    nc.tensor.matmul(psh[:], lhsT=wfc_t[:], rhs=crys[:, j:j + 512],
                                     start=True, stop=True)
                    hex_ = hp.tile([H, 512], F32, tag="hex")
                    nc.scalar.activation(out=hex_[:], in_=psh[:],
                                         func=AF.Exp, bias=bfc_t[:], scale=1.0)
                    nc.scalar.activation(out=hc[:, j:j + 512], in_=hex_[:],
                                         func=AF.Ln, bias=1.0, scale=1.0)
                ofin = hp.tile([1, NCB], F32, tag="ofin")
                for j in range(0, NCB, 512):
                    pso = hps.tile([1, 512], F32, tag="pso")
                    nc.tensor.matmul(pso[:], lhsT=wout_t[:], rhs=hc[:, j:j + 512],
                                     start=True, stop=True)
                    nc.scalar.activation(out=ofin[:, j:j + 512], in_=pso[:],
                                         func=AF.Identity, bias=bout_t[:], scale=1.0)
                nc.sync.dma_start(out=out_t[:, :], in_=ofin[:])

    nc.compile()
    return nc


# --------------------------------------------------------------------------
# host-side input preparation
# --------------------------------------------------------------------------

def prepare_inputs(c, atom_fea, nbr_fea, nbr_fea_idx, crystal_atom_idx,
                   W_emb, b_emb, W_full, b_full, g1, be1, g2, be2,
                   W_fc, b_fc, W_out, b_out):
    N, M, F0, FB, F, H, NC, L = (c["N"], c["M"], c["F0"], c["FB"], c["F"],
                                 c["H"], c["NC"], c["NCONV"])
    G, TA, NT, AP_, ET, EL = (c["G"], c["TA"], c["ntile"], c["A_pad"],
                              c["E_tile"], c["E_loc"])
    W16, NCB, K = c["W16"], c["NCB"], c["NCORES"]
    AS = c["A_shard"]

    atom_fea = np.asarray(atom_fea, np.float32)
    nbr_fea = np.asarray(nbr_fea, np.float32)
    nbr_fea_idx = np.asarray(nbr_fea_idx, np.int64)
    crystal_atom_idx = np.asarray(crystal_atom_idx, np.int64)

    # shared (replicated) tensors
    oh = np.zeros((128, ET), np.float16)
    for j in range(ET):
        oh[j // M, j] = 1.0
    shared = {
        "oh_self": oh,
        "w_emb": np.asarray(W_emb, np.float16),
        "b_emb": np.asarray(b_emb, np.float32).reshape(F, 1),
        "w_self": np.asarray(W_full[:, :F, :], np.float16),
        "w_nbr": np.asarray(W_full[:, F:2 * F, :], np.float16),
        "w_b": np.asarray(W_full[:, 2 * F:, :]).astype(ml_dtypes.float8_e4m3),
        "g1": np.asarray(g1, np.float32).reshape(L, G, 1),
        "be1": np.asarray(be1, np.float32).reshape(L, G, 1),
        "g2": np.asarray(g2, np.float32).reshape(L, F, 1),
        "be2": np.asarray(be2, np.float32).reshape(L, F, 1),
        "w_fc": np.asarray(W_fc, np.float16),
        "b_fc": np.asarray(b_fc, np.float32).reshape(H, 1),
        "w_out": np.asarray(W_out, np.float16),
        "b_out": np.asarray(b_out, np.float32).reshape(1, 1),
        "ident": np.eye(128, dtype=np.float32),
        "identh": np.eye(128, dtype=np.float16),
    }
    # crystal counts (global, from index data only)
    cnt = np.bincount(crystal_atom_idx, minlength=NC).astype(np.float32)
    icnt = np.zeros((NCB, 1), np.float32)
    icnt[:NC, 0] = 1.0 / np.maximum(cnt, 1.0)
    shared["invcnt"] = icnt

    # b_full is mathematically irrelevant (cancelled by training-mode BN)

    in_maps = []
    for k in range(K):
        a0 = k * AS
        af = np.zeros((F0, AP_), np.float16)
        af[:, :AS] = atom_fea[a0:a0 + AS].T
        # edge ordering: e = a*M + m within each tile of TA atoms
        gi = np.zeros((NT * TA, M), np.int64)
        gi_raw = nbr_fea_idx[a0:a0 + AS]
        gi[:AS] = gi_raw
        valid = np.zeros((NT * TA, M), bool)
        valid[:AS] = True
        jj = (gi // AS) * AP_ + (gi % AS)          # padded-global atom index
        rows = np.where(valid, 1 + jj // 2, 0).astype(np.int64)
        par = np.where(valid, jj & 1, 0).astype(np.int64)
        idxw = np.zeros((128, NT, W16), np.int16)
        mparr = np.zeros((128, NT, M), np.int8)
        j = np.arange(ET)
        for t in range(NT):
            fl = rows[t * TA:(t + 1) * TA].reshape(ET)
            wrap = np.zeros((16, W16), np.int16)
            wrap[j % 16, j // 16] = fl
            idxw[:, t, :] = np.tile(wrap, (8, 1))
            # ge[p, i, :] holds edge i*128+p -> mask[p, i]
            mparr[:, t, :] = par[t * TA:(t + 1) * TA].reshape(ET)[
                (np.arange(M)[None, :] * 128 + np.arange(128)[:, None])]
        nb = np.zeros((FB, EL), ml_dtypes.float8_e4m3)
        nb_l = nbr_fea[a0:a0 + AS].reshape(AS * M, FB)
        src = np.zeros((NT * TA * M, FB), np.float32)
        src[:AS * M] = nb_l
        # src is already in (a, m) order; tiles are contiguous runs of ET
        nb[:, :] = src.T.astype(ml_dtypes.float8_e4m3)

        cry = np.zeros(NT * TA, np.int64)
        cry[:AS] = crystal_atom_idx[a0:a0 + AS]
        cb = int(crystal_atom_idx[a0:a0 + AS].min())
        cmax = int(crystal_atom_idx[a0:a0 + AS].max())
        assert cmax - cb < 512, f"crystal window too wide: {cmax - cb}"
        # pool one-hot: [atom (partition), crystal-window col]
        pone = np.zeros((NT, TA, 512), np.float16)
        for t in range(NT):
            for a in range(TA):
                ga = t * TA + a
                if ga >= AS:
                    continue
                pone[t, a, int(cry[ga]) - cb] = 1.0
        # scatter-add indices: window row j -> crystal cb+j, wrapped in 16
        # partitions and replicated (SWDGE idx convention)
        sw = np.zeros((16, 32), np.int16)
        jj512 = np.arange(512)
        sw[jj512 % 16, jj512 // 16] = (cb + jj512).astype(np.int16)
        scidx = np.tile(sw, (8, 1))
        assert cb + 512 <= NCB

        in_maps.append(dict(shared,
                            afT=af, idxw=idxw, mparr=mparr, nbrT=nb, pone=pone,
                            scidx=scidx))
    return in_maps


# --------------------------------------------------------------------------
# public entry point
# --------------------------------------------------------------------------

_PROG_CACHE = {}


def _get_program(c):
    key = tuple(sorted((k, v) for k, v in c.items()))
    if key not in _PROG_CACHE:
        _PROG_CACHE[key] = build_program(c)
    return _PROG_CACHE[key]


def kernel(atom_fea, nbr_fea, nbr_fea_idx, crystal_atom_idx, W_emb, b_emb,
           W_full, b_full, g1, be1, g2, be2, W_fc, b_fc, W_out, b_out,
           _trace=False):
    from concourse import bass_utils
    c = CFG
    nc = _get_program(c)
    in_maps = prepare_inputs(c, atom_fea, nbr_fea, nbr_fea_idx,
                             crystal_atom_idx, W_emb, b_emb, W_full, b_full,
                             g1, be1, g2, be2, W_fc, b_fc, W_out, b_out)
    res = bass_utils.run_bass_kernel_spmd(
        nc, in_maps, core_ids=list(range(c["NCORES"])), trace=_trace)
    out = np.asarray(res.results[0]["out"], np.float32)
    ret = out[0, :c["NC"]].reshape(c["NC"], 1)
    if _trace:
        return ret, res
    return ret



# revision 29
# speedup vs baseline: 2.1588x; 1.0103x over previous
"""CrystalGraphConvNet forward pass as a distributed Bass/Tile kernel on 8 TRN2
NeuronCores.

Strategy (graph/data parallel, per sharding hint):
  - Atoms sharded contiguously across 8 cores (7500 each, padded to 7552).
  - Per conv layer, each core computes Y_self = x @ Wf[:F], Y_nbr = x @ Wf[F:2F]
    for its atom shard; Y_nbr shards are AllGathered into a replicated f16
    table viewed as PAIR rows (two atoms = 512 B per row, plus a leading zero
    row), so a single int16-indexed dma_gather per tile fetches both parity
    candidates of every edge in one 512 B packet (row index = 1 + j//2).
  - The gather runs un-transposed (contiguous SBUF writes, edge-major):
    ge[p, i, 0:128] / [128:256] hold the even/odd atom of edge i*128+p.  A
    single copy_predicated with a tiny resident per-edge parity mask
    (broadcast along channels) selects the right atom; 12 accumulating
    transpose-matmuls against an f16 identity then fold the selected rows
    into the channel-major PSUM accumulator on the TensorEngine, on top of
    the nbr_fea projection and the one-hot self term.
  - Training-mode batchnorm needs global stats, so gated values are staged to
    DRAM scratch in f16 while per-channel sum/sumsq accumulate (scalar-engine
    copy/square with accum); a tiny AllReduce yields the affine.  The second
    pass is a SINGLE pass per tile pair: sigmoid is computed as
    1/(1+exp(-a)) with a DVE reciprocal, so every activation in the program
    lives in the one exp/ln table set and no ACT table reloads occur.
  - Crystal mean-pooling is one accumulating one-hot matmul per tile into a
    512-crystal window, scattered by int32 indirect DMA into a global crystal
    array and AllReduced; the tiny MLP head runs redundantly on every core.
"""

import math
import os
import ml_dtypes
import numpy as np

import concourse.bass as bass
import concourse.bacc as bacc
import concourse.tile as tile
from concourse import mybir
from contextlib import ExitStack


def _single_act_table(orig):
    """Route exp/ln to the one ACT table set that contains BOTH (the default
    chooser splits them across exp_and_others/natural_log, thrashing table
    reloads on every exp<->ln transition in the batchnorm affines).  Copy/
    Square/Identity stay in EVERY set so pass A and the sigmoid/softplus
    batches never force a reload for them; Sigmoid and Softplus live in their
    own sets and are batch-switched in pass B."""
    AF = mybir.ActivationFunctionType
    both = {AF.Exp, AF.Ln}
    shared = {AF.Exp, AF.Ln}
    home = None
    for name, funcs in orig.items():
        if both <= funcs:
            home = name
            break
    if home is None:
        return orig
    out = {}
    for name, funcs in orig.items():
        out[name] = funcs if name == home else (funcs - shared)
    return out


_orig_get_activation_tables = bacc.get_activation_tables


def _patched_get_activation_tables(arch):
    return _single_act_table(_orig_get_activation_tables(arch))


bacc.get_activation_tables = _patched_get_activation_tables

F16 = mybir.dt.float16
F32 = mybir.dt.float32
F8 = mybir.dt.float8e4
I8 = mybir.dt.int8
I16 = mybir.dt.int16
I32 = mybir.dt.int32


def make_cfg(N=60000, M=12, F0=92, FB=41, F=64, H=128, NC=2000, NCONV=3,
             EPS=1e-5, NCORES=8, TA=128):
    c = dict(N=N, M=M, F0=F0, FB=FB, F=F, H=H, NC=NC, NCONV=NCONV, EPS=EPS,
             NCORES=NCORES, TA=TA)
    assert N % NCORES == 0
    c["A_shard"] = N // NCORES
    c["ntile"] = (c["A_shard"] + TA - 1) // TA
    c["A_pad"] = c["ntile"] * TA
    c["E_tile"] = TA * M
    c["E_loc"] = c["ntile"] * c["E_tile"]
    assert (NCORES * c["A_pad"]) % 2 == 0
    c["R2"] = 1 + NCORES * c["A_pad"] // 2      # pair-table rows (zero row at 0)
    assert c["R2"] <= 32768, "pair table must stay int16-addressable"
    c["W16"] = c["E_tile"] // 16
    c["NCB"] = 512 * ((NC + 511) // 512) + 512  # crystal bounce rows
    c["G"] = 2 * F                              # gated channels
    return c


CFG = make_cfg()


# --------------------------------------------------------------------------
# program builder
# --------------------------------------------------------------------------

def build_program(c, debug=False, dbg_dump=False, stop=None):
    # stop: optional (layer, stage) tuple for bisection; stages within a layer:
    # 0=Y, 1=AG, 2=A, 3=AR1, 4=B, 5=AR2, 6=C; (L,0)=pool, (L,1)=head
    nc = bacc.Bacc("TRN2", target_bir_lowering=False, debug=debug,
                   num_devices=c["NCORES"], num_swdge_queues=4)

    N, M, F0, FB, F, H, NC, L = (c["N"], c["M"], c["F0"], c["FB"], c["F"],
                                 c["H"], c["NC"], c["NCONV"])
    G, TA, NT, AP_, ET, EL = (c["G"], c["TA"], c["ntile"], c["A_pad"],
                              c["E_tile"], c["E_loc"])
    R2, W16, NCB, EPS = c["R2"], c["W16"], c["NCB"], c["EPS"]
    NPAIR = (NT + 1) // 2
    NCHUNK = ET // TA                           # 128-edge chunks per tile

    # ---------------- inputs ----------------
    afT = nc.dram_tensor("afT", [F0, AP_], F16, kind="ExternalInput")
    idxw = nc.dram_tensor("idxw", [128, NT, W16], I16, kind="ExternalInput")
    mparr = nc.dram_tensor("mparr", [128, NT, M], I8, kind="ExternalInput")
    nbrT = nc.dram_tensor("nbrT", [FB, EL], F8, kind="ExternalInput")
    oh_self = nc.dram_tensor("oh_self", [128, ET], F16, kind="ExternalInput")
    pone = nc.dram_tensor("pone", [NT, 128, 512], F16, kind="ExternalInput")
    scidx = nc.dram_tensor("scidx", [128, 32], I16, kind="ExternalInput")
    invcnt = nc.dram_tensor("invcnt", [NCB, 1], F32, kind="ExternalInput")
    w_emb = nc.dram_tensor("w_emb", [F0, F], F16, kind="ExternalInput")
    b_emb = nc.dram_tensor("b_emb", [F, 1], F32, kind="ExternalInput")
    w_self = nc.dram_tensor("w_self", [L, F, G], F16, kind="ExternalInput")
    w_nbr = nc.dram_tensor("w_nbr", [L, F, G], F16, kind="ExternalInput")
    w_b = nc.dram_tensor("w_b", [L, FB, G], F8, kind="ExternalInput")
    g1 = nc.dram_tensor("g1", [L, G, 1], F32, kind="ExternalInput")
    be1 = nc.dram_tensor("be1", [L, G, 1], F32, kind="ExternalInput")
    g2 = nc.dram_tensor("g2", [L, F, 1], F32, kind="ExternalInput")
    be2 = nc.dram_tensor("be2", [L, F, 1], F32, kind="ExternalInput")
    w_fc = nc.dram_tensor("w_fc", [F, H], F16, kind="ExternalInput")
    b_fc = nc.dram_tensor("b_fc", [H, 1], F32, kind="ExternalInput")
    w_out = nc.dram_tensor("w_out", [H, 1], F16, kind="ExternalInput")
    b_out = nc.dram_tensor("b_out", [1, 1], F32, kind="ExternalInput")
    ident = nc.dram_tensor("ident", [128, 128], F32, kind="ExternalInput")
    identh = nc.dram_tensor("identh", [128, 128], F16, kind="ExternalInput")

    out_t = nc.dram_tensor("out", [1, NCB], F32, kind="ExternalOutput")
    if dbg_dump:
        L_, F_, G_, AP2, EL_ = c["NCONV"], c["F"], c["G"], c["A_pad"], c["E_loc"]
        dbgx0 = nc.dram_tensor("dbgx0", [F_, AP2], F32, kind="ExternalOutput")
        dbgx = nc.dram_tensor("dbgx", [L_, F_, AP2], F32, kind="ExternalOutput")
        dbgsum = nc.dram_tensor("dbgsum", [L_, F_, AP2], F32, kind="ExternalOutput")
        dbgst1 = nc.dram_tensor("dbgst1", [L_, G_, 2], F32, kind="ExternalOutput")
        dbggat = nc.dram_tensor("dbggat", [L_, NT, 128, ET], F8,
                                kind="ExternalOutput")
        dbgpool = nc.dram_tensor("dbgpool", [NCB, F_], F32, kind="ExternalOutput")

    # ---------------- internal DRAM ----------------
    yb = nc.dram_tensor("yb", [AP_, G], F8)                        # AG input bounce
    tbl = nc.dram_tensor("tbl", [R2, 2 * G], F8, addr_space="Shared")
    scr = nc.dram_tensor("scr", [NT, 128, ET], F8)
    ar1_in = nc.dram_tensor("ar1_in", [G, 2], F32)
    ar1_out = nc.dram_tensor("ar1_out", [G, 2], F32, addr_space="Shared")
    ar2_in = nc.dram_tensor("ar2_in", [F, 2], F32)
    ar2_out = nc.dram_tensor("ar2_out", [F, 2], F32, addr_space="Shared")
    pool_in = nc.dram_tensor("pool_in", [NCB, F], F32)
    pool_out = nc.dram_tensor("pool_out", [NCB, F], F32, addr_space="Shared")

    rg = [list(range(c["NCORES"]))]
    AF = mybir.ActivationFunctionType
    OP = mybir.AluOpType
    if stop is None:
        stop = (L, 9)

    # Tile cycles DMASW lane sems per Pool DMA in GLOBAL program order (%8);
    # each lane sem is queue-locked, so the SWDGE queue must follow the same
    # global counter (%4), not the per-layer tile index.
    gq = [0]

    def gather_q():
        q = gq[0] % 4
        gq[0] += 1
        return q

    def live(key):
        return key <= stop

    with tile.TileContext(nc) as tc, ExitStack() as top:
        # Tile assigns each Pool-engine DMA inst a DMASW lane (round-robin in
        # program order) and points consumer waits at that lane's semaphore.
        # A prepare_only gather bakes its completion sem into the descriptors
        # (pass 2 attaches no then_inc), so sem= MUST be the very lane sem the
        # tick pass will pick, tracked here with a mirror counter.
        swsems = tc.sems.swdge_block()
        pool_dma_lane = [0]

        def prep_slot():
            """(lane sem, queue) for the next prepared Pool DMA.  queue =
            lane % nqueues keeps every lane sem on one fixed queue (SWDGE
            sems are queue-locked)."""
            j = pool_dma_lane[0]
            pool_dma_lane[0] += 1
            return swsems[j % len(swsems)], (j % len(swsems)) % 4

        # persistent SBUF state
        x_cm = nc.alloc_sbuf_tensor("x_cm", [F, AP_], F32)
        summed = nc.alloc_sbuf_tensor("summed", [F, AP_], F16)
        ysr = nc.alloc_sbuf_tensor("ysr", [128, NT, G], F16)      # Y_self row-major
        idx_all = nc.alloc_sbuf_tensor("idx_all", [128, NT, W16], I16)
        mpar = nc.alloc_sbuf_tensor("mpar", [128, NT, M, 1], I8)

        const = top.enter_context(tc.tile_pool(name="const", bufs=1))
        stats = top.enter_context(tc.tile_pool(name="stats", bufs=1))

        # constants resident all kernel
        ohs_t = const.tile([128, ET], F16)
        nc.sync.dma_start(out=ohs_t[:], in_=oh_self[:, :])
        wemb_t = const.tile([F0, F], F16)
        nc.sync.dma_start(out=wemb_t[:], in_=w_emb[:, :])
        bemb_t = const.tile([F, 1], F32)
        nc.sync.dma_start(out=bemb_t[:], in_=b_emb[:, :])
        id_t = const.tile([128, 128], F32)
        nc.sync.dma_start(out=id_t[:], in_=ident[:, :])
        idh_t = const.tile([128, 128], F16)
        nc.sync.dma_start(out=idh_t[:], in_=identh[:, :])
        idh8 = const.tile([128, 128], F8)
        nc.vector.tensor_copy(out=idh8[:], in_=idh_t[:])
        eps_t = const.tile([128, 1], F32)
        nc.vector.memset(eps_t[:], EPS)
        zrow = const.tile([1, 2 * G], F8)
        nc.vector.memset(zrow[:], 0.0)
        zt128 = const.tile([128, F], F32)
        nc.vector.memset(zt128[:], 0.0)

        # layer-invariant gather indices and parity masks
        nc.sync.dma_start(out=idx_all[:, :, :], in_=idxw[:, :, :])
        nc.sync.dma_start(out=mpar[:, :, :, 0], in_=mparr[:, :, :])

        wS = []
        wN = []
        wB = []
        for l in range(L):
            t1 = const.tile([F, G], F16, tag=f"wS{l}")
            nc.sync.dma_start(out=t1[:], in_=w_self[l, :, :])
            t2 = const.tile([F, G], F16, tag=f"wN{l}")
            nc.sync.dma_start(out=t2[:], in_=w_nbr[l, :, :])
            t3 = const.tile([FB, G], F8, tag=f"wB{l}")
            nc.sync.dma_start(out=t3[:], in_=w_b[l, :, :])
            wS.append(t1)
            wN.append(t2)
            wB.append(t3)

        # stats buffers
        st1_s = stats.tile([G, NT], F32, tag="st1s")
        st1_q = stats.tile([G, NT], F32, tag="st1q")
        st2_s = stats.tile([F, 2 * NPAIR], F32, tag="st2s")
        st2_q = stats.tile([F, 2 * NPAIR], F32, tag="st2q")

        # zero table guard row + summed pads
        nc.sync.dma_start(out=tbl[0:1, :], in_=zrow[:])
        nc.vector.memset(summed[:, :], 0.0)

        # ---------------- embedding: x = atom_fea @ W_emb + b_emb ----------
        with tc.tile_pool(name="emb", bufs=3) as embp, \
             tc.tile_pool(name="embps", bufs=2, space="PSUM") as embps:
            CH = 512
            for j in range(0, AP_, CH):
                w = min(CH, AP_ - j)
                rhs = embp.tile([F0, CH], F16, tag="embr")
                nc.sync.dma_start(out=rhs[:, :w], in_=afT[:, j:j + w])
                ps = embps.tile([F, CH], F32, tag="embp")
                nc.tensor.matmul(ps[:, :w], lhsT=wemb_t[:], rhs=rhs[:, :w],
                                 start=True, stop=True)
                nc.scalar.activation(out=x_cm[:, j:j + w], in_=ps[:, :w],
                                     func=AF.Identity, bias=bemb_t[:], scale=1.0)
        if dbg_dump:
            nc.sync.dma_start(out=dbgx0[:, :], in_=x_cm[:, :])

        # ---------------- conv layers ----------------
        for l in range(L):
            if not live((l, 0)):
                break
            # ---- phase Y: Y_self (SBUF) / Y_nbr (-> bounce -> AllGather) ----
            with tc.tile_pool(name="yph", bufs=3) as yp, \
                 tc.tile_pool(name="yps", bufs=2, space="PSUM") as yps:
                lastreal = c["A_shard"] - (NT - 1) * TA
                for t in range(NT):
                    xa = yp.tile([F, TA], F16, tag="xa")
                    nc.scalar.activation(out=xa[:], in_=x_cm[:, t * TA:(t + 1) * TA],
                                         func=AF.Copy)
                    psS = yps.tile([TA, G], F32, tag="psS")
                    nc.tensor.matmul(psS[:], lhsT=xa[:], rhs=wS[l][:],
                                     start=True, stop=True)
                    # pad atoms of the last tile must contribute exactly zero
                    # through the self one-hot matmul
                    nreal = TA if t < NT - 1 else lastreal
                    if nreal < TA:
                        nc.vector.memset(ysr[:, t, :], 0.0)
                    nc.scalar.activation(out=ysr[0:nreal, t, :],
                                         in_=psS[0:nreal, :], func=AF.Copy)
                    psN = yps.tile([TA, G], F32, tag="psN")
                    nc.tensor.matmul(psN[:], lhsT=xa[:], rhs=wN[l][:],
                                     start=True, stop=True)
                    yn = yp.tile([TA, G], F8, tag="yn")
                    nc.scalar.activation(out=yn[:], in_=psN[:], func=AF.Copy)
                    nc.sync.dma_start(out=yb[t * TA:(t + 1) * TA, :], in_=yn[:])

            if not live((l, 1)):
                break
            tc.strict_bb_all_engine_barrier()
            nc.gpsimd.collective_compute(
                "AllGather", OP.bypass, replica_groups=rg,
                ins=[yb[:, :]], outs=[tbl[1:R2, :]])
            tc.strict_bb_all_engine_barrier()

            if not live((l, 2)):
                break
            # ---- pass A: edges -> gated scratch + stats1 ----
            with tc.tile_pool(name="pa", bufs=8) as pa, \
                 tc.tile_pool(name="paps", bufs=2, space="PSUM") as paps:
                for t in range(NT):
                    nbt = pa.tile([FB, ET], F8, tag="nbt")
                    nc.sync.dma_start(out=nbt[:], in_=nbrT[:, t * ET:(t + 1) * ET])

                    ge = pa.tile([128, NCHUNK, 2 * G], F8, tag="ge")
                    # prepare_only decouples Q7 desc-gen from the transfer:
                    # the gather DMA fires at trigger time, so transfers on
                    # the 4 SWDGE queues overlap each other and compute
                    # instead of serializing on the Pool engine.
                    nc.gpsimd.dma_gather(ge[:], tbl[:, :], idx_all[:, t, :],
                                         ET, ET, 2 * G, single_packet=False,
                                         queue_num=gather_q())
                    # parity select: overwrite even-atom slab with odd-atom
                    # slab wherever the edge's target index is odd
                    nc.vector.copy_predicated(
                        out=ge[:, :, 0:G],
                        mask=mpar[:, t, :, :].broadcast_to([128, NCHUNK, G]),
                        data=ge[:, :, G:2 * G])

                    # every 128-col region: chunk transpose (start) -> wB ->
                    # one-hot self term (stop)
                    ps = paps.tile([G, ET], F32, tag="aps")
                    for i in range(NCHUNK):
                        cs = slice(i * 128, (i + 1) * 128)
                        nc.tensor.matmul(ps[:, cs], lhsT=ge[:, i, 0:G],
                                         rhs=idh8[:], start=(i % 4 == 0),
                                         stop=False)
                    for s in range(ET // 512):
                        sl = slice(s * 512, (s + 1) * 512)
                        nc.tensor.matmul(ps[:, sl], lhsT=wB[l][:], rhs=nbt[:, sl],
                                         start=False, stop=False)
                        nc.tensor.matmul(ps[:, sl], lhsT=ysr[:, t, :],
                                         rhs=ohs_t[:, sl], start=False, stop=True)

                    gat = pa.tile([128, ET], F8, tag="gat")
                    nc.scalar.activation(out=gat[:], in_=ps[:], func=AF.Copy,
                                         accum_out=st1_s[:, t:t + 1])
                    sqd = pa.tile([128, ET], F16, tag="sqd")
                    nc.scalar.activation(out=sqd[:], in_=ps[:], func=AF.Square,
                                         accum_out=st1_q[:, t:t + 1])
                    nc.sync.dma_start(out=scr[t, :, :], in_=gat[:])

            if not live((l, 3)):
                break
            # ---- stats1 reduce + AllReduce + affine ----
            with tc.tile_pool(name="s1", bufs=1) as s1p:
                pack1 = s1p.tile([G, 2], F32, tag="pack1")
                nc.vector.tensor_reduce(out=pack1[:, 0:1], in_=st1_s[:],
                                        axis=mybir.AxisListType.X, op=OP.add)
                nc.vector.tensor_reduce(out=pack1[:, 1:2], in_=st1_q[:],
                                        axis=mybir.AxisListType.X, op=OP.add)
                nc.sync.dma_start(out=ar1_in[:, :], in_=pack1[:])
                tc.strict_bb_all_engine_barrier()
                nc.gpsimd.collective_compute(
                    "AllReduce", OP.add, replica_groups=rg,
                    ins=[ar1_in[:, :]], outs=[ar1_out[:, :]])
                tc.strict_bb_all_engine_barrier()

                red1 = s1p.tile([G, 2], F32, tag="red1")
                nc.sync.dma_start(out=red1[:], in_=ar1_out[:, :])
                if dbg_dump:
                    nc.sync.dma_start(out=dbgst1[l, :, :], in_=red1[:])
                    for t in range(NT):
                        nc.gpsimd.dma_start(out=dbggat[l, t, :, :],
                                            in_=scr[t, :, :])
                g1_t = s1p.tile([G, 1], F32, tag="g1t")
                nc.sync.dma_start(out=g1_t[:], in_=g1[l, :, :])
                be1_t = s1p.tile([G, 1], F32, tag="be1t")
                nc.sync.dma_start(out=be1_t[:], in_=be1[l, :, :])

                # rsqrt(var+eps) = exp(-0.5*ln(var+eps)) + one Newton step
                # (no sqrt table needed; Ln/Exp share one ACT table set)
                invE = 1.0 / float(N * M)
                mmt = s1p.tile([G, 8], F32, tag="mmt")
                mcol = mmt[:, 0:1]
                nc.vector.tensor_scalar(out=mcol, in0=red1[:, 0:1], scalar1=invE,
                                        scalar2=None, op0=OP.mult)
                ex2 = mmt[:, 1:2]
                nc.vector.tensor_scalar(out=ex2, in0=red1[:, 1:2], scalar1=invE,
                                        scalar2=None, op0=OP.mult)
                msq = mmt[:, 2:3]
                nc.vector.tensor_tensor(out=msq, in0=mcol, in1=mcol, op=OP.mult)
                var = mmt[:, 3:4]
                nc.vector.tensor_tensor(out=var, in0=ex2, in1=msq, op=OP.subtract)
                lv = mmt[:, 4:5]
                nc.scalar.activation(out=lv, in_=var, func=AF.Ln,
                                     bias=eps_t[0:G, :], scale=1.0)
                r0 = mmt[:, 5:6]
                nc.scalar.activation(out=r0, in_=lv, func=AF.Exp, scale=-0.5)
                # one Newton step: r1 = r0*(1.5 - 0.5*(var+eps)*r0^2)
                vpe = mmt[:, 6:7]
                nc.vector.tensor_scalar(out=vpe, in0=var, scalar1=eps_t[0:G, :],
                                        scalar2=0.5, op0=OP.add, op1=OP.mult)
                r0q = mmt[:, 7:8]
                nc.vector.tensor_tensor(out=r0q, in0=r0, in1=r0, op=OP.mult)
                nc.vector.tensor_tensor(out=r0q, in0=r0q, in1=vpe, op=OP.mult)
                nc.vector.tensor_scalar(out=r0q, in0=r0q, scalar1=-1.0,
                                        scalar2=1.5, op0=OP.mult, op1=OP.add)
                r1 = mmt[:, 6:7]
                nc.vector.tensor_tensor(out=r1, in0=r0, in1=r0q, op=OP.mult)

                s1c_ = s1p.tile([G, 1], F32, tag="s1c")
                nc.vector.tensor_tensor(out=s1c_[:], in0=g1_t[:], in1=r1,
                                        op=OP.mult)
                t1c_ = s1p.tile([G, 1], F32, tag="t1c")
                nc.vector.tensor_tensor(out=t1c_[:], in0=mcol, in1=s1c_[:],
                                        op=OP.mult)
                nc.vector.scalar_tensor_tensor(out=t1c_[:], in0=t1c_[:],
                                               scalar=-1.0, in1=be1_t[:],
                                               op0=OP.mult, op1=OP.add)
                # replicated (packed-pair) scale/bias
                sF = s1p.tile([128, 1], F32, tag="sF")
                tF = s1p.tile([128, 1], F32, tag="tF")
                sC = s1p.tile([128, 1], F32, tag="sC")
                tC = s1p.tile([128, 1], F32, tag="tC")
                for half in range(2):
                    hp = slice(half * F, half * F + F)
                    nc.sync.dma_start(out=sF[hp, :], in_=s1c_[0:F, :])
                    nc.sync.dma_start(out=tF[hp, :], in_=t1c_[0:F, :])
                    nc.sync.dma_start(out=sC[hp, :], in_=s1c_[F:G, :])
                    nc.sync.dma_start(out=tC[hp, :], in_=t1c_[F:G, :])

                if not live((l, 4)):
                    break
                # ---- pass B: sigmoid*softplus, neighbor-sum, stats2 ----
                # K pairs per chunk share each ACT table load: all K sigmoids
                # run under sigmoid_and_others, then all K softplus under
                # softplus_and_others (2 reloads per chunk instead of 2/pair)
                tc.strict_bb_all_engine_barrier()
                KP = 4
                with tc.tile_pool(name="pb", bufs=KP + 2) as bp:
                    for pc in range(0, NPAIR, KP):
                        chunk = range(pc, min(pc + KP, NPAIR))
                        zfs, zcs, sgs, sps = {}, {}, {}, {}
                        for p in chunk:
                            t0, t1_ = 2 * p, min(2 * p + 1, NT - 1)
                            single = (2 * p + 1 > NT - 1)
                            zf = bp.tile([128, ET], F8, tag="zf")
                            zc = bp.tile([128, ET], F8, tag="zc")
                            nc.sync.dma_start(out=zf[0:F, :], in_=scr[t0, 0:F, :])
                            nc.sync.dma_start(out=zc[0:F, :], in_=scr[t0, F:G, :])
                            if not single:
                                nc.sync.dma_start(out=zf[F:G, :],
                                                  in_=scr[t1_, 0:F, :])
                                nc.sync.dma_start(out=zc[F:G, :],
                                                  in_=scr[t1_, F:G, :])
                            zfs[p], zcs[p] = zf, zc
                        for p in chunk:
                            single = (2 * p + 1 > NT - 1)
                            pp = 128 if not single else F
                            sg = bp.tile([128, ET], F16, tag="sg")
                            nc.scalar.activation(out=sg[0:pp, :],
                                                 in_=zfs[p][0:pp, :],
                                                 func=AF.Sigmoid,
                                                 bias=tF[0:pp, :],
                                                 scale=sF[0:pp, :])
                            sgs[p] = sg
                        for p in chunk:
                            single = (2 * p + 1 > NT - 1)
                            pp = 128 if not single else F
                            ec = bp.tile([128, ET], F32, tag="ec")
                            nc.scalar.activation(out=ec[0:pp, :],
                                                 in_=zcs[p][0:pp, :],
                                                 func=AF.Exp,
                                                 bias=tC[0:pp, :],
                                                 scale=sC[0:pp, :])
                            sp = bp.tile([128, ET], F16, tag="sp")
                            nc.scalar.activation(out=sp[0:pp, :],
                                                 in_=ec[0:pp, :],
                                                 func=AF.Ln, bias=1.0,
                                                 scale=1.0)
                            sps[p] = sp
                        for p in chunk:
                            t0, t1_ = 2 * p, min(2 * p + 1, NT - 1)
                            single = (2 * p + 1 > NT - 1)
                            pp = 128 if not single else F
                            sp = sps[p]
                            nc.vector.tensor_tensor(out=sp[0:pp, :],
                                                    in0=sp[0:pp, :],
                                                    in1=sgs[p][0:pp, :],
                                                    op=OP.mult)
                            # one full-width M-reduce for both pair halves
                            red2 = bp.tile([128, TA], F32, tag="red2")
                            nc.vector.tensor_reduce(
                                out=red2[0:pp, :],
                                in_=sp[0:pp, :].rearrange("p (a m) -> p a m",
                                                          m=M),
                                axis=mybir.AxisListType.X, op=OP.add)
                            halves = 1 if single else 2
                            for hh in range(halves):
                                tt = t0 if hh == 0 else t1_
                                hs = slice(hh * F, hh * F + F)
                                # aligned copy rides the DVE; the shifted one
                                # hits a DVE slow path, keep on scalar
                                if hh == 0:
                                    nc.vector.tensor_copy(
                                        out=summed[:, tt * TA:(tt + 1) * TA],
                                        in_=red2[hs, :])
                                else:
                                    nc.scalar.activation(
                                        out=summed[:, tt * TA:(tt + 1) * TA],
                                        in_=red2[hs, :], func=AF.Copy)
                                # stats over real atoms only
                                nreal = min(c["A_shard"] - tt * TA, TA)
                                col = 2 * p + hh
                                nc.vector.tensor_reduce(
                                    out=st2_s[:, col:col + 1],
                                    in_=red2[hs, 0:nreal],
                                    axis=mybir.AxisListType.X, op=OP.add)
                                sqt = bp.tile([F, TA], F32, tag="sqt")
                                nc.vector.tensor_tensor(
                                    out=sqt[:, 0:nreal], in0=red2[hs, 0:nreal],
                                    in1=red2[hs, 0:nreal], op=OP.mult)
                                nc.vector.tensor_reduce(
                                    out=st2_q[:, col:col + 1],
                                    in_=sqt[:, 0:nreal],
                                    axis=mybir.AxisListType.X, op=OP.add)

                if not live((l, 5)):
                    break
                # ---- stats2 AllReduce + affine2 + pass C ----
                ncols = NT
                pack2 = s1p.tile([F, 2], F32, tag="pack2")
                nc.vector.tensor_reduce(out=pack2[:, 0:1],
                                        in_=st2_s[:, 0:ncols],
                                        axis=mybir.AxisListType.X, op=OP.add)
                nc.vector.tensor_reduce(out=pack2[:, 1:2],
                                        in_=st2_q[:, 0:ncols],
                                        axis=mybir.AxisListType.X, op=OP.add)
                nc.sync.dma_start(out=ar2_in[:, :], in_=pack2[:])
                tc.strict_bb_all_engine_barrier()
                nc.gpsimd.collective_compute(
                    "AllReduce", OP.add, replica_groups=rg,
                    ins=[ar2_in[:, :]], outs=[ar2_out[:, :]])
                tc.strict_bb_all_engine_barrier()

                red2 = s1p.tile([F, 2], F32, tag="red2")
                nc.sync.dma_start(out=red2[:], in_=ar2_out[:, :])
                g2_t = s1p.tile([F, 1], F32, tag="g2t")
                nc.sync.dma_start(out=g2_t[:], in_=g2[l, :, :])
                be2_t = s1p.tile([F, 1], F32, tag="be2t")
                nc.sync.dma_start(out=be2_t[:], in_=be2[l, :, :])

                invN = 1.0 / float(N)
                mt2 = s1p.tile([F, 8], F32, tag="mt2")
                m2c = mt2[:, 0:1]
                nc.vector.tensor_scalar(out=m2c, in0=red2[:, 0:1], scalar1=invN,
                                        scalar2=None, op0=OP.mult)
                e2c = mt2[:, 1:2]
                nc.vector.tensor_scalar(out=e2c, in0=red2[:, 1:2], scalar1=invN,
                                        scalar2=None, op0=OP.mult)
                ms2 = mt2[:, 2:3]
                nc.vector.tensor_tensor(out=ms2, in0=m2c, in1=m2c, op=OP.mult)
                v2 = mt2[:, 3:4]
                nc.vector.tensor_tensor(out=v2, in0=e2c, in1=ms2, op=OP.subtract)
                lv2 = mt2[:, 4:5]
                nc.scalar.activation(out=lv2, in_=v2, func=AF.Ln,
                                     bias=eps_t[0:F, :], scale=1.0)
                r02 = mt2[:, 5:6]
                nc.scalar.activation(out=r02, in_=lv2, func=AF.Exp, scale=-0.5)
                vpe2 = mt2[:, 6:7]
                nc.vector.tensor_scalar(out=vpe2, in0=v2, scalar1=eps_t[0:F, :],
                                        scalar2=0.5, op0=OP.add, op1=OP.mult)
                r0q2 = mt2[:, 7:8]
                nc.vector.tensor_tensor(out=r0q2, in0=r02, in1=r02, op=OP.mult)
                nc.vector.tensor_tensor(out=r0q2, in0=r0q2, in1=vpe2, op=OP.mult)
                nc.vector.tensor_scalar(out=r0q2, in0=r0q2, scalar1=-1.0,
                                        scalar2=1.5, op0=OP.mult, op1=OP.add)
                r12 = mt2[:, 6:7]
                nc.vector.tensor_tensor(out=r12, in0=r02, in1=r0q2, op=OP.mult)
                s2c = s1p.tile([F, 1], F32, tag="s2c")
                nc.vector.tensor_tensor(out=s2c[:], in0=g2_t[:], in1=r12,
                                        op=OP.mult)
                t2c = s1p.tile([F, 1], F32, tag="t2c")
                nc.vector.tensor_tensor(out=t2c[:], in0=m2c, in1=s2c[:],
                                        op=OP.mult)
                nc.vector.scalar_tensor_tensor(out=t2c[:], in0=t2c[:],
                                               scalar=-1.0, in1=be2_t[:],
                                               op0=OP.mult, op1=OP.add)

                if not live((l, 6)):
                    break
                # pass C: x = softplus(x + s2*summed + t2) via ln(1+exp)
                with tc.tile_pool(name="pc", bufs=2) as pcp:
                    CW = 2048
                    for j in range(0, AP_, CW):
                        w = min(CW, AP_ - j)
                        pre = pcp.tile([F, CW], F32, tag="pre")
                        nc.vector.scalar_tensor_tensor(
                            out=pre[:, :w], in0=summed[:, j:j + w],
                            scalar=s2c[:, 0:1], in1=x_cm[:, j:j + w],
                            op0=OP.mult, op1=OP.add)
                        pex = pcp.tile([F, CW], F32, tag="pex")
                        nc.scalar.activation(out=pex[:, :w], in_=pre[:, :w],
                                             func=AF.Exp, bias=t2c[:], scale=1.0)
                        nc.scalar.activation(out=x_cm[:, j:j + w], in_=pex[:, :w],
                                             func=AF.Ln, bias=1.0, scale=1.0)
                if dbg_dump:
                    nc.gpsimd.dma_start(out=dbgsum[l, :, :], in_=summed[:, :])
                    nc.sync.dma_start(out=dbgx[l, :, :], in_=x_cm[:, :])

        for _ in range(1 if live((L, 0)) else 0):
            # ---------------- crystal pooling ----------------
            with tc.tile_pool(name="pool", bufs=4) as pp_, \
                 tc.tile_pool(name="poolacc", bufs=1, space="PSUM") as pacc, \
                 tc.tile_pool(name="poolps", bufs=2, space="PSUM") as pps:
                acc = pacc.tile([F, 512], F32, tag="pacc")
                for t in range(NT):
                    xps = pps.tile([TA, F], F32, tag="xps")
                    nc.tensor.transpose(out=xps[:], in_=x_cm[:, t * TA:(t + 1) * TA],
                                        identity=id_t[0:F, 0:F])
                    xrm = pp_.tile([TA, F], F16, tag="xrm")
                    nc.scalar.activation(out=xrm[:], in_=xps[:], func=AF.Copy)
                    oht = pp_.tile([128, 512], F16, tag="oht")
                    nc.sync.dma_start(out=oht[:], in_=pone[t, :, :])
                    nc.tensor.matmul(acc[:], lhsT=xrm[:], rhs=oht[:],
                                     start=(t == 0), stop=(t == NT - 1))
                # zero pool_in, then scatter our window rows
                for r_ in range(NCB // 128):
                    nc.sync.dma_start(out=pool_in[128 * r_:128 * (r_ + 1), :],
                                      in_=zt128[:, 0:F])
                sci = pp_.tile([128, 32], I16, tag="sci")
                nc.sync.dma_start(out=sci[:], in_=scidx[:, :])
                tc.strict_bb_all_engine_barrier()
                accS = pp_.tile([F, 512], F32, tag="accS")
                nc.vector.tensor_copy(out=accS[:], in_=acc[:])
                asb4 = pp_.tile([128, 4, F], F32, tag="asb4")
                for ci in range(4):
                    # [F, 128] crystal block -> row-major [128, F]
                    tps = pps.tile([128, F], F32, tag="tps")
                    nc.tensor.transpose(out=tps[:],
                                        in_=accS[:, ci * 128:(ci + 1) * 128],
                                        identity=id_t[0:F, 0:F])
                    nc.vector.tensor_copy(out=asb4[:, ci, :], in_=tps[:])
                # one SWDGE scatter-add of the 512-crystal window (row i of
                # the window lives at asb4[i%128, i//128, :]); queue follows
                # the global Pool-DMA lane counter like the gathers
                nc.gpsimd.dma_scatter_add(
                    pool_in[:, :], asb4[:], sci[:, :], 512, 512, F,
                    single_packet=False, queue_num=gather_q())
                tc.strict_bb_all_engine_barrier()
                nc.gpsimd.collective_compute(
                    "AllReduce", OP.add, replica_groups=rg,
                    ins=[pool_in[:, :]], outs=[pool_out[:, :]])
                tc.strict_bb_all_engine_barrier()

            if dbg_dump:
                nc.sync.dma_start(out=dbgpool[:, :], in_=pool_out[:, :])


        for _ in range(1 if live((L, 1)) else 0):
            # ---------------- head ----------------
            with tc.tile_pool(name="head", bufs=3) as hp, \
                 tc.tile_pool(name="headps", bufs=2, space="PSUM") as hps:
                crys = nc.alloc_sbuf_tensor("crys", [F, NCB], F16)
                wfc_t = hp.tile([F, H], F16, tag="wfc")
                nc.sync.dma_start(out=wfc_t[:], in_=w_fc[:, :])
                bfc_t = hp.tile([H, 1], F32, tag="bfc")
                nc.sync.dma_start(out=bfc_t[:], in_=b_fc[:, :])
                wout_t = hp.tile([H, 1], F16, tag="wout")
                nc.sync.dma_start(out=wout_t[:], in_=w_out[:, :])
                bout_t = hp.tile([1, 1], F32, tag="bout")
                nc.sync.dma_start(out=bout_t[:], in_=b_out[:, :])

                for r_ in range(NCB // 128):
                    pt = hp.tile([128, F], F32, tag="pt")
                    nc.sync.dma_start(out=pt[:], in_=pool_out[128 * r_:128 * (r_ + 1), :])
                    ic = hp.tile([128, 1], F32, tag="ic")
                    nc.sync.dma_start(out=ic[:], in_=invcnt[128 * r_:128 * (r_ + 1), :])
                    nc.vector.tensor_scalar(out=pt[:], in0=pt[:], scalar1=ic[:, 0:1],
                                            scalar2=None, op0=OP.mult)
                    pex2 = hp.tile([128, F], F32, tag="pex2")
                    nc.scalar.activation(out=pex2[:], in_=pt[:], func=AF.Exp)
                    spt = hp.tile([128, F], F32, tag="spt")
                    nc.scalar.activation(out=spt[:], in_=pex2[:], func=AF.Ln,
                                         bias=1.0, scale=1.0)
                    tps = hps.tile([F, 128], F32, tag="tps")
                    nc.tensor.transpose(out=tps[:], in_=spt[:], identity=id_t[:, :])
                    nc.scalar.activation(out=crys[:, 128 * r_:128 * (r_ + 1)],
                                         in_=tps[:], func=AF.Copy)

                hc = nc.alloc_sbuf_tensor("hc", [H, NCB], F16)
                for j in range(0, NCB, 512):
                    psh = hps.tile([H, 512], F32, tag="psh")
                    nc.tensor.matmul(psh[:], lhsT=wfc_t[:], rhs=crys[:, j:j + 512],
                                     start=True, stop=True)
                    hex_ = hp.tile([H, 512], F32, tag="hex")
                    nc.scalar.activation(out=hex_[:], in_=psh[:],
                                         func=AF.Exp, bias=bfc_t[:], scale=1.0)
                    nc.scalar.activation(out=hc[:, j:j + 512], in_=hex_[:],
                                         func=AF.Ln, bias=1.0, scale=1.0)
                ofin = hp.tile([1, NCB], F32, tag="ofin")
                for j in range(0, NCB, 512):
                    pso = hps.tile([1, 512], F32, tag="pso")
                    nc.tensor.matmul(pso[:], lhsT=wout_t[:], rhs=hc[:, j:j + 512],
                                     start=True, stop=True)
                    nc.scalar.activation(out=ofin[:, j:j + 512], in_=pso[:],
                                         func=AF.Identity, bias=bout_t[:], scale=1.0)
                nc.sync.dma_start(out=out_t[:, :], in_=ofin[:])

    nc.compile()
    return nc


# --------------------------------------------------------------------------
# host-side input preparation
# --------------------------------------------------------------------------

def prepare_inputs(c, atom_fea, nbr_fea, nbr_fea_idx, crystal_atom_idx,
                   W_emb, b_emb, W_full, b_full, g1, be1, g2, be2,
                   W_fc, b_fc, W_out, b_out):
    N, M, F0, FB, F, H, NC, L = (c["N"], c["M"], c["F0"], c["FB"], c["F"],
                                 c["H"], c["NC"], c["NCONV"])
    G, TA, NT, AP_, ET, EL = (c["G"], c["TA"], c["ntile"], c["A_pad"],
                              c["E_tile"], c["E_loc"])
    W16, NCB, K = c["W16"], c["NCB"], c["NCORES"]
    AS = c["A_shard"]

    atom_fea = np.asarray(atom_fea, np.float32)
    nbr_fea = np.asarray(nbr_fea, np.float32)
    nbr_fea_idx = np.asarray(nbr_fea_idx, np.int64)
    crystal_atom_idx = np.asarray(crystal_atom_idx, np.int64)

    # shared (replicated) tensors
    oh = np.zeros((128, ET), np.float16)
    for j in range(ET):
        oh[j // M, j] = 1.0
    shared = {
        "oh_self": oh,
        "w_emb": np.asarray(W_emb, np.float16),
        "b_emb": np.asarray(b_emb, np.float32).reshape(F, 1),
        "w_self": np.asarray(W_full[:, :F, :], np.float16),
        "w_nbr": np.asarray(W_full[:, F:2 * F, :], np.float16),
        "w_b": np.asarray(W_full[:, 2 * F:, :]).astype(ml_dtypes.float8_e4m3),
        "g1": np.asarray(g1, np.float32).reshape(L, G, 1),
        "be1": np.asarray(be1, np.float32).reshape(L, G, 1),
        "g2": np.asarray(g2, np.float32).reshape(L, F, 1),
        "be2": np.asarray(be2, np.float32).reshape(L, F, 1),
        "w_fc": np.asarray(W_fc, np.float16),
        "b_fc": np.asarray(b_fc, np.float32).reshape(H, 1),
        "w_out": np.asarray(W_out, np.float16),
        "b_out": np.asarray(b_out, np.float32).reshape(1, 1),
        "ident": np.eye(128, dtype=np.float32),
        "identh": np.eye(128, dtype=np.float16),
    }
    # crystal counts (global, from index data only)
    cnt = np.bincount(crystal_atom_idx, minlength=NC).astype(np.float32)
    icnt = np.zeros((NCB, 1), np.float32)
    icnt[:NC, 0] = 1.0 / np.maximum(cnt, 1.0)
    shared["invcnt"] = icnt

    # b_full is mathematically irrelevant (cancelled by training-mode BN)

    in_maps = []
    for k in range(K):
        a0 = k * AS
        af = np.zeros((F0, AP_), np.float16)
        af[:, :AS] = atom_fea[a0:a0 + AS].T
        # edge ordering: e = a*M + m within each tile of TA atoms
        gi = np.zeros((NT * TA, M), np.int64)
        gi_raw = nbr_fea_idx[a0:a0 + AS]
        gi[:AS] = gi_raw
        valid = np.zeros((NT * TA, M), bool)
        valid[:AS] = True
        jj = (gi // AS) * AP_ + (gi % AS)          # padded-global atom index
        rows = np.where(valid, 1 + jj // 2, 0).astype(np.int64)
        par = np.where(valid, jj & 1, 0).astype(np.int64)
        idxw = np.zeros((128, NT, W16), np.int16)
        mparr = np.zeros((128, NT, M), np.int8)
        j = np.arange(ET)
        for t in range(NT):
            fl = rows[t * TA:(t + 1) * TA].reshape(ET)
            wrap = np.zeros((16, W16), np.int16)
            wrap[j % 16, j // 16] = fl
            idxw[:, t, :] = np.tile(wrap, (8, 1))
            # ge[p, i, :] holds edge i*128+p -> mask[p, i]
            mparr[:, t, :] = par[t * TA:(t + 1) * TA].reshape(ET)[
                (np.arange(M)[None, :] * 128 + np.arange(128)[:, None])]
        nb = np.zeros((FB, EL), ml_dtypes.float8_e4m3)
        nb_l = nbr_fea[a0:a0 + AS].reshape(AS * M, FB)
        src = np.zeros((NT * TA * M, FB), np.float32)
        src[:AS * M] = nb_l
        # src is already in (a, m) order; tiles are contiguous runs of ET
        nb[:, :] = src.T.astype(ml_dtypes.float8_e4m3)

        cry = np.zeros(NT * TA, np.int64)
        cry[:AS] = crystal_atom_idx[a0:a0 + AS]
        cb = int(crystal_atom_idx[a0:a0 + AS].min())
        cmax = int(crystal_atom_idx[a0:a0 + AS].max())
        assert cmax - cb < 512, f"crystal window too wide: {cmax - cb}"
        # pool one-hot: [atom (partition), crystal-window col]
        pone = np.zeros((NT, TA, 512), np.float16)
        for t in range(NT):
            for a in range(TA):
                ga = t * TA + a
                if ga >= AS:
                    continue
                pone[t, a, int(cry[ga]) - cb] = 1.0
        # scatter-add indices: window row j -> crystal cb+j, wrapped in 16
        # partitions and replicated (SWDGE idx convention)
        sw = np.zeros((16, 32), np.int16)
        jj512 = np.arange(512)
        sw[jj512 % 16, jj512 // 16] = (cb + jj512).astype(np.int16)
        scidx = np.tile(sw, (8, 1))
        assert cb + 512 <= NCB

        in_maps.append(dict(shared,
                            afT=af, idxw=idxw, mparr=mparr, nbrT=nb, pone=pone,
                            scidx=scidx))
    return in_maps


# --------------------------------------------------------------------------
# public entry point
# --------------------------------------------------------------------------

_PROG_CACHE = {}


def _get_program(c):
    key = tuple(sorted((k, v) for k, v in c.items()))
    if key not in _PROG_CACHE:
        _PROG_CACHE[key] = build_program(c)
    return _PROG_CACHE[key]


def kernel(atom_fea, nbr_fea, nbr_fea_idx, crystal_atom_idx, W_emb, b_emb,
           W_full, b_full, g1, be1, g2, be2, W_fc, b_fc, W_out, b_out,
           _trace=False):
    from concourse import bass_utils
    c = CFG
    nc = _get_program(c)
    in_maps = prepare_inputs(c, atom_fea, nbr_fea, nbr_fea_idx,
                             crystal_atom_idx, W_emb, b_emb, W_full, b_full,
                             g1, be1, g2, be2, W_fc, b_fc, W_out, b_out)
    res = bass_utils.run_bass_kernel_spmd(
        nc, in_maps, core_ids=list(range(c["NCORES"])), trace=_trace)
    out = np.asarray(res.results[0]["out"], np.float32)
    ret = out[0, :c["NC"]].reshape(c["NC"], 1)
    if _trace:
        return ret, res
    return ret



# revision 32
# speedup vs baseline: 2.1675x; 1.0041x over previous
"""CrystalGraphConvNet forward pass as a distributed Bass/Tile kernel on 8 TRN2
NeuronCores.

Strategy (graph/data parallel, per sharding hint):
  - Atoms sharded contiguously across 8 cores (7500 each, padded to 7552).
  - Per conv layer, each core computes Y_self/Y_nbr for its shard; Y_nbr
    shards are AllGathered into a replicated fp8 table of PAIR rows (two
    atoms = 256 B per row + a zero guard row), so one int16-indexed
    dma_gather per tile fetches both parity candidates of every edge in a
    single 256 B packet.  Gathers rotate the 4 SWDGE queues with a GLOBAL
    counter (the Tile DMASW lane sems are queue-locked, so queue must track
    the lane cycle, not the per-layer tile index).  Pass A is paced by the
    DMA packet rate (~5 us per 1536-packet gather at 4 queues), so all other
    staged traffic is fp8: nbr features + W_b (double-fp8 matmul), the gated
    scratch, and the gather table itself.
  - A copy_predicated parity select + 12 accumulating fp8 transpose-matmuls
    fold the gathered rows into the channel-major PSUM accumulator, on top
    of the nbr_fea projection and the one-hot self term.
  - Training-mode batchnorm needs global stats: gated values stage to DRAM
    in fp8 while per-channel sum/sumsq accumulate (scalar copy/square with
    accum); a tiny AllReduce yields the affine.  Pass B batches K=4 tile
    pairs per ACT table load: all sigmoids run under sigmoid_and_others
    (with the BN affine folded into scale/bias), then softplus via exp+ln
    under the combined ln/exp table (this pwp build has no Softplus entry);
    DVE does the product, the M-reduce, and the summed-stats squares.
  - Crystal mean-pooling is one accumulating one-hot matmul per tile into a
    512-crystal window, written by a single SWDGE dma_scatter_add into a
    global crystal array and AllReduced; the MLP head runs on every core.
"""

import math
import os
import ml_dtypes
import numpy as np

import concourse.bass as bass
import concourse.bacc as bacc
import concourse.tile as tile
from concourse import mybir
from contextlib import ExitStack


def _single_act_table(orig):
    """Route exp/ln to the one ACT table set that contains BOTH (the default
    chooser splits them across exp_and_others/natural_log, thrashing table
    reloads on every exp<->ln transition in the batchnorm affines).  Copy/
    Square/Identity stay in EVERY set so pass A and the sigmoid/softplus
    batches never force a reload for them; Sigmoid and Softplus live in their
    own sets and are batch-switched in pass B."""
    AF = mybir.ActivationFunctionType
    both = {AF.Exp, AF.Ln}
    shared = {AF.Exp, AF.Ln}
    home = None
    for name, funcs in orig.items():
        if both <= funcs:
            home = name
            break
    if home is None:
        return orig
    out = {}
    for name, funcs in orig.items():
        out[name] = funcs if name == home else (funcs - shared)
    return out


_orig_get_activation_tables = bacc.get_activation_tables


def _patched_get_activation_tables(arch):
    return _single_act_table(_orig_get_activation_tables(arch))


bacc.get_activation_tables = _patched_get_activation_tables

F16 = mybir.dt.float16
F32 = mybir.dt.float32
F8 = mybir.dt.float8e4
I8 = mybir.dt.int8
I16 = mybir.dt.int16
I32 = mybir.dt.int32


def make_cfg(N=60000, M=12, F0=92, FB=41, F=64, H=128, NC=2000, NCONV=3,
             EPS=1e-5, NCORES=8, TA=128):
    c = dict(N=N, M=M, F0=F0, FB=FB, F=F, H=H, NC=NC, NCONV=NCONV, EPS=EPS,
             NCORES=NCORES, TA=TA)
    assert N % NCORES == 0
    c["A_shard"] = N // NCORES
    c["ntile"] = (c["A_shard"] + TA - 1) // TA
    c["A_pad"] = c["ntile"] * TA
    c["E_tile"] = TA * M
    c["E_loc"] = c["ntile"] * c["E_tile"]
    assert (NCORES * c["A_pad"]) % 2 == 0
    c["R2"] = 1 + NCORES * c["A_pad"] // 2      # pair-table rows (zero row at 0)
    assert c["R2"] <= 32768, "pair table must stay int16-addressable"
    c["W16"] = c["E_tile"] // 16
    c["NCB"] = 512 * ((NC + 511) // 512) + 512  # crystal bounce rows
    c["G"] = 2 * F                              # gated channels
    return c


CFG = make_cfg()


# --------------------------------------------------------------------------
# program builder
# --------------------------------------------------------------------------

def build_program(c, debug=False, dbg_dump=False, stop=None):
    # stop: optional (layer, stage) tuple for bisection; stages within a layer:
    # 0=Y, 1=AG, 2=A, 3=AR1, 4=B, 5=AR2, 6=C; (L,0)=pool, (L,1)=head
    nc = bacc.Bacc("TRN2", target_bir_lowering=False, debug=debug,
                   num_devices=c["NCORES"], num_swdge_queues=4)

    N, M, F0, FB, F, H, NC, L = (c["N"], c["M"], c["F0"], c["FB"], c["F"],
                                 c["H"], c["NC"], c["NCONV"])
    G, TA, NT, AP_, ET, EL = (c["G"], c["TA"], c["ntile"], c["A_pad"],
                              c["E_tile"], c["E_loc"])
    R2, W16, NCB, EPS = c["R2"], c["W16"], c["NCB"], c["EPS"]
    NPAIR = (NT + 1) // 2
    NCHUNK = ET // TA                           # 128-edge chunks per tile

    # ---------------- inputs ----------------
    afT = nc.dram_tensor("afT", [F0, AP_], F16, kind="ExternalInput")
    idxw = nc.dram_tensor("idxw", [128, NT, W16], I16, kind="ExternalInput")
    mparr = nc.dram_tensor("mparr", [128, NT, M], I8, kind="ExternalInput")
    nbrT = nc.dram_tensor("nbrT", [FB, EL], F8, kind="ExternalInput")
    oh_self = nc.dram_tensor("oh_self", [128, ET], F8, kind="ExternalInput")
    pone = nc.dram_tensor("pone", [NT, 128, 512], F16, kind="ExternalInput")
    scidx = nc.dram_tensor("scidx", [128, 32], I16, kind="ExternalInput")
    invcnt = nc.dram_tensor("invcnt", [NCB, 1], F32, kind="ExternalInput")
    w_emb = nc.dram_tensor("w_emb", [F0, F], F16, kind="ExternalInput")
    b_emb = nc.dram_tensor("b_emb", [F, 1], F32, kind="ExternalInput")
    w_self = nc.dram_tensor("w_self", [L, F, G], F16, kind="ExternalInput")
    w_nbr = nc.dram_tensor("w_nbr", [L, F, G], F16, kind="ExternalInput")
    w_b = nc.dram_tensor("w_b", [L, FB, G], F8, kind="ExternalInput")
    g1 = nc.dram_tensor("g1", [L, G, 1], F32, kind="ExternalInput")
    be1 = nc.dram_tensor("be1", [L, G, 1], F32, kind="ExternalInput")
    g2 = nc.dram_tensor("g2", [L, F, 1], F32, kind="ExternalInput")
    be2 = nc.dram_tensor("be2", [L, F, 1], F32, kind="ExternalInput")
    w_fc = nc.dram_tensor("w_fc", [F, H], F16, kind="ExternalInput")
    b_fc = nc.dram_tensor("b_fc", [H, 1], F32, kind="ExternalInput")
    w_out = nc.dram_tensor("w_out", [H, 1], F16, kind="ExternalInput")
    b_out = nc.dram_tensor("b_out", [1, 1], F32, kind="ExternalInput")
    ident = nc.dram_tensor("ident", [128, 128], F32, kind="ExternalInput")
    identh = nc.dram_tensor("identh", [128, 128], F16, kind="ExternalInput")

    out_t = nc.dram_tensor("out", [1, NCB], F32, kind="ExternalOutput")
    if dbg_dump:
        L_, F_, G_, AP2, EL_ = c["NCONV"], c["F"], c["G"], c["A_pad"], c["E_loc"]
        dbgx0 = nc.dram_tensor("dbgx0", [F_, AP2], F32, kind="ExternalOutput")
        dbgx = nc.dram_tensor("dbgx", [L_, F_, AP2], F32, kind="ExternalOutput")
        dbgsum = nc.dram_tensor("dbgsum", [L_, F_, AP2], F32, kind="ExternalOutput")
        dbgst1 = nc.dram_tensor("dbgst1", [L_, G_, 2], F32, kind="ExternalOutput")
        dbggat = nc.dram_tensor("dbggat", [L_, NT, 128, ET], F8,
                                kind="ExternalOutput")
        dbgpool = nc.dram_tensor("dbgpool", [NCB, F_], F32, kind="ExternalOutput")

    # ---------------- internal DRAM ----------------
    yb = nc.dram_tensor("yb", [AP_, G], F8)                        # AG input bounce
    tbl = nc.dram_tensor("tbl", [R2, 2 * G], F8, addr_space="Shared")
    scr = nc.dram_tensor("scr", [NT, 128, ET], F8)
    ar1_in = nc.dram_tensor("ar1_in", [G, 2], F32)
    ar1_out = nc.dram_tensor("ar1_out", [G, 2], F32, addr_space="Shared")
    ar2_in = nc.dram_tensor("ar2_in", [F, 2], F32)
    ar2_out = nc.dram_tensor("ar2_out", [F, 2], F32, addr_space="Shared")
    pool_in = nc.dram_tensor("pool_in", [NCB, F], F32)
    pool_out = nc.dram_tensor("pool_out", [NCB, F], F32, addr_space="Shared")

    rg = [list(range(c["NCORES"]))]
    AF = mybir.ActivationFunctionType
    OP = mybir.AluOpType
    if stop is None:
        stop = (L, 9)

    # Tile cycles DMASW lane sems per Pool DMA in GLOBAL program order (%8);
    # each lane sem is queue-locked, so the SWDGE queue must follow the same
    # global counter (%4), not the per-layer tile index.
    gq = [0]

    def gather_q():
        q = gq[0] % 4
        gq[0] += 1
        return q

    def live(key):
        return key <= stop

    with tile.TileContext(nc) as tc, ExitStack() as top:
        # Tile assigns each Pool-engine DMA inst a DMASW lane (round-robin in
        # program order) and points consumer waits at that lane's semaphore.
        # A prepare_only gather bakes its completion sem into the descriptors
        # (pass 2 attaches no then_inc), so sem= MUST be the very lane sem the
        # tick pass will pick, tracked here with a mirror counter.
        swsems = tc.sems.swdge_block()
        pool_dma_lane = [0]

        def prep_slot():
            """(lane sem, queue) for the next prepared Pool DMA.  queue =
            lane % nqueues keeps every lane sem on one fixed queue (SWDGE
            sems are queue-locked)."""
            j = pool_dma_lane[0]
            pool_dma_lane[0] += 1
            return swsems[j % len(swsems)], (j % len(swsems)) % 4

        # persistent SBUF state
        x_cm = nc.alloc_sbuf_tensor("x_cm", [F, AP_], F16)
        summed = nc.alloc_sbuf_tensor("summed", [F, AP_], F16)
        ysr = nc.alloc_sbuf_tensor("ysr", [128, NT, G], F8)      # Y_self row-major
        idx_all = nc.alloc_sbuf_tensor("idx_all", [128, NT, W16], I16)
        mpar = nc.alloc_sbuf_tensor("mpar", [128, NT, M, 1], I8)

        const = top.enter_context(tc.tile_pool(name="const", bufs=1))
        stats = top.enter_context(tc.tile_pool(name="stats", bufs=1))

        # constants resident all kernel
        ohs_t = const.tile([128, ET], F8)
        nc.sync.dma_start(out=ohs_t[:], in_=oh_self[:, :])
        wemb_t = const.tile([F0, F], F16)
        nc.sync.dma_start(out=wemb_t[:], in_=w_emb[:, :])
        bemb_t = const.tile([F, 1], F32)
        nc.sync.dma_start(out=bemb_t[:], in_=b_emb[:, :])
        id_t = const.tile([128, 128], F32)
        nc.sync.dma_start(out=id_t[:], in_=ident[:, :])
        idh_t = const.tile([128, 128], F16)
        nc.sync.dma_start(out=idh_t[:], in_=identh[:, :])
        idh8 = const.tile([128, 128], F8)
        nc.vector.tensor_copy(out=idh8[:], in_=idh_t[:])
        eps_t = const.tile([128, 1], F32)
        nc.vector.memset(eps_t[:], EPS)
        zrow = const.tile([1, 2 * G], F8)
        nc.vector.memset(zrow[:], 0.0)
        zt128 = const.tile([128, F], F32)
        nc.vector.memset(zt128[:], 0.0)

        # layer-invariant gather indices and parity masks
        nc.sync.dma_start(out=idx_all[:, :, :], in_=idxw[:, :, :])
        nc.sync.dma_start(out=mpar[:, :, :, 0], in_=mparr[:, :, :])

        wS = []
        wN = []
        wB = []
        for l in range(L):
            t1 = const.tile([F, G], F16, tag=f"wS{l}")
            nc.sync.dma_start(out=t1[:], in_=w_self[l, :, :])
            t2 = const.tile([F, G], F16, tag=f"wN{l}")
            nc.sync.dma_start(out=t2[:], in_=w_nbr[l, :, :])
            t3 = const.tile([FB, G], F8, tag=f"wB{l}")
            nc.sync.dma_start(out=t3[:], in_=w_b[l, :, :])
            wS.append(t1)
            wN.append(t2)
            wB.append(t3)

        # stats buffers
        st1_s = stats.tile([G, NT], F32, tag="st1s")
        st1_q = stats.tile([G, NT], F32, tag="st1q")
        st2_s = stats.tile([F, 2 * NPAIR], F32, tag="st2s")
        st2_q = stats.tile([F, 2 * NPAIR], F32, tag="st2q")

        # zero table guard row + summed pads
        nc.sync.dma_start(out=tbl[0:1, :], in_=zrow[:])
        nc.vector.memset(summed[:, :], 0.0)

        # ---------------- embedding: x = atom_fea @ W_emb + b_emb ----------
        with tc.tile_pool(name="emb", bufs=3) as embp, \
             tc.tile_pool(name="embps", bufs=2, space="PSUM") as embps:
            CH = 512
            for j in range(0, AP_, CH):
                w = min(CH, AP_ - j)
                rhs = embp.tile([F0, CH], F16, tag="embr")
                nc.sync.dma_start(out=rhs[:, :w], in_=afT[:, j:j + w])
                ps = embps.tile([F, CH], F32, tag="embp")
                nc.tensor.matmul(ps[:, :w], lhsT=wemb_t[:], rhs=rhs[:, :w],
                                 start=True, stop=True)
                nc.scalar.activation(out=x_cm[:, j:j + w], in_=ps[:, :w],
                                     func=AF.Identity, bias=bemb_t[:], scale=1.0)
        if dbg_dump:
            nc.sync.dma_start(out=dbgx0[:, :], in_=x_cm[:, :])

        # ---------------- conv layers ----------------
        for l in range(L):
            if not live((l, 0)):
                break
            # ---- phase Y: Y_self (SBUF) / Y_nbr (-> bounce -> AllGather) ----
            with tc.tile_pool(name="yph", bufs=3) as yp, \
                 tc.tile_pool(name="yps", bufs=2, space="PSUM") as yps:
                lastreal = c["A_shard"] - (NT - 1) * TA
                for t in range(NT):
                    xa = x_cm[:, t * TA:(t + 1) * TA]
                    psS = yps.tile([TA, G], F32, tag="psS")
                    nc.tensor.matmul(psS[:], lhsT=xa, rhs=wS[l][:],
                                     start=True, stop=True)
                    # pad atoms of the last tile must contribute exactly zero
                    # through the self one-hot matmul
                    nreal = TA if t < NT - 1 else lastreal
                    if nreal < TA:
                        nc.vector.memset(ysr[:, t, :], 0.0)
                    nc.scalar.activation(out=ysr[0:nreal, t, :],
                                         in_=psS[0:nreal, :], func=AF.Copy)
                    psN = yps.tile([TA, G], F32, tag="psN")
                    nc.tensor.matmul(psN[:], lhsT=xa, rhs=wN[l][:],
                                     start=True, stop=True)
                    yn = yp.tile([TA, G], F8, tag="yn")
                    nc.scalar.activation(out=yn[:], in_=psN[:], func=AF.Copy)
                    nc.sync.dma_start(out=yb[t * TA:(t + 1) * TA, :], in_=yn[:])

            if not live((l, 1)):
                break
            tc.strict_bb_all_engine_barrier()
            nc.gpsimd.collective_compute(
                "AllGather", OP.bypass, replica_groups=rg,
                ins=[yb[:, :]], outs=[tbl[1:R2, :]])
            tc.strict_bb_all_engine_barrier()

            if not live((l, 2)):
                break
            # ---- pass A: edges -> gated scratch + stats1 ----
            with tc.tile_pool(name="pa", bufs=8) as pa, \
                 tc.tile_pool(name="paps", bufs=2, space="PSUM") as paps:
                for t in range(NT):
                    nbt = pa.tile([FB, ET], F8, tag="nbt")
                    nc.sync.dma_start(out=nbt[:], in_=nbrT[:, t * ET:(t + 1) * ET])

                    ge = pa.tile([128, NCHUNK, 2 * G], F8, tag="ge")
                    # prepare_only decouples Q7 desc-gen from the transfer:
                    # the gather DMA fires at trigger time, so transfers on
                    # the 4 SWDGE queues overlap each other and compute
                    # instead of serializing on the Pool engine.
                    nc.gpsimd.dma_gather(ge[:], tbl[:, :], idx_all[:, t, :],
                                         ET, ET, 2 * G, single_packet=False,
                                         queue_num=gather_q())
                    # parity select: overwrite even-atom slab with odd-atom
                    # slab wherever the edge's target index is odd
                    nc.vector.copy_predicated(
                        out=ge[:, :, 0:G],
                        mask=mpar[:, t, :, :].broadcast_to([128, NCHUNK, G]),
                        data=ge[:, :, G:2 * G])

                    # every 128-col region: chunk transpose (start) -> wB ->
                    # one-hot self term (stop)
                    ps = paps.tile([G, ET], F32, tag="aps")
                    for i in range(NCHUNK):
                        cs = slice(i * 128, (i + 1) * 128)
                        nc.tensor.matmul(ps[:, cs], lhsT=ge[:, i, 0:G],
                                         rhs=idh8[:], start=(i % 4 == 0),
                                         stop=False)
                    for s in range(ET // 512):
                        sl = slice(s * 512, (s + 1) * 512)
                        nc.tensor.matmul(ps[:, sl], lhsT=wB[l][:], rhs=nbt[:, sl],
                                         start=False, stop=False)
                        nc.tensor.matmul(ps[:, sl], lhsT=ysr[:, t, :],
                                         rhs=ohs_t[:, sl], start=False, stop=True)

                    gat = pa.tile([128, ET], F8, tag="gat")
                    nc.scalar.activation(out=gat[:], in_=ps[:], func=AF.Copy,
                                         accum_out=st1_s[:, t:t + 1])
                    sqd = pa.tile([128, ET], F16, tag="sqd")
                    nc.scalar.activation(out=sqd[:], in_=ps[:], func=AF.Square,
                                         accum_out=st1_q[:, t:t + 1])
                    nc.sync.dma_start(out=scr[t, :, :], in_=gat[:])

            if not live((l, 3)):
                break
            # ---- stats1 reduce + AllReduce + affine ----
            with tc.tile_pool(name="s1", bufs=1) as s1p:
                pack1 = s1p.tile([G, 2], F32, tag="pack1")
                nc.vector.tensor_reduce(out=pack1[:, 0:1], in_=st1_s[:],
                                        axis=mybir.AxisListType.X, op=OP.add)
                nc.vector.tensor_reduce(out=pack1[:, 1:2], in_=st1_q[:],
                                        axis=mybir.AxisListType.X, op=OP.add)
                nc.sync.dma_start(out=ar1_in[:, :], in_=pack1[:])
                tc.strict_bb_all_engine_barrier()
                nc.gpsimd.collective_compute(
                    "AllReduce", OP.add, replica_groups=rg,
                    ins=[ar1_in[:, :]], outs=[ar1_out[:, :]])
                tc.strict_bb_all_engine_barrier()

                red1 = s1p.tile([G, 2], F32, tag="red1")
                nc.sync.dma_start(out=red1[:], in_=ar1_out[:, :])
                if dbg_dump:
                    nc.sync.dma_start(out=dbgst1[l, :, :], in_=red1[:])
                    for t in range(NT):
                        nc.gpsimd.dma_start(out=dbggat[l, t, :, :],
                                            in_=scr[t, :, :])
                g1_t = s1p.tile([G, 1], F32, tag="g1t")
                nc.sync.dma_start(out=g1_t[:], in_=g1[l, :, :])
                be1_t = s1p.tile([G, 1], F32, tag="be1t")
                nc.sync.dma_start(out=be1_t[:], in_=be1[l, :, :])

                # rsqrt(var+eps) = exp(-0.5*ln(var+eps)) + one Newton step
                # (no sqrt table needed; Ln/Exp share one ACT table set)
                invE = 1.0 / float(N * M)
                mmt = s1p.tile([G, 8], F32, tag="mmt")
                mcol = mmt[:, 0:1]
                nc.vector.tensor_scalar(out=mcol, in0=red1[:, 0:1], scalar1=invE,
                                        scalar2=None, op0=OP.mult)
                ex2 = mmt[:, 1:2]
                nc.vector.tensor_scalar(out=ex2, in0=red1[:, 1:2], scalar1=invE,
                                        scalar2=None, op0=OP.mult)
                msq = mmt[:, 2:3]
                nc.vector.tensor_tensor(out=msq, in0=mcol, in1=mcol, op=OP.mult)
                var = mmt[:, 3:4]
                nc.vector.tensor_tensor(out=var, in0=ex2, in1=msq, op=OP.subtract)
                lv = mmt[:, 4:5]
                nc.scalar.activation(out=lv, in_=var, func=AF.Ln,
                                     bias=eps_t[0:G, :], scale=1.0)
                r0 = mmt[:, 5:6]
                nc.scalar.activation(out=r0, in_=lv, func=AF.Exp, scale=-0.5)
                # one Newton step: r1 = r0*(1.5 - 0.5*(var+eps)*r0^2)
                vpe = mmt[:, 6:7]
                nc.vector.tensor_scalar(out=vpe, in0=var, scalar1=eps_t[0:G, :],
                                        scalar2=0.5, op0=OP.add, op1=OP.mult)
                r0q = mmt[:, 7:8]
                nc.vector.tensor_tensor(out=r0q, in0=r0, in1=r0, op=OP.mult)
                nc.vector.tensor_tensor(out=r0q, in0=r0q, in1=vpe, op=OP.mult)
                nc.vector.tensor_scalar(out=r0q, in0=r0q, scalar1=-1.0,
                                        scalar2=1.5, op0=OP.mult, op1=OP.add)
                r1 = mmt[:, 6:7]
                nc.vector.tensor_tensor(out=r1, in0=r0, in1=r0q, op=OP.mult)

                s1c_ = s1p.tile([G, 1], F32, tag="s1c")
                nc.vector.tensor_tensor(out=s1c_[:], in0=g1_t[:], in1=r1,
                                        op=OP.mult)
                t1c_ = s1p.tile([G, 1], F32, tag="t1c")
                nc.vector.tensor_tensor(out=t1c_[:], in0=mcol, in1=s1c_[:],
                                        op=OP.mult)
                nc.vector.scalar_tensor_tensor(out=t1c_[:], in0=t1c_[:],
                                               scalar=-1.0, in1=be1_t[:],
                                               op0=OP.mult, op1=OP.add)
                # replicated (packed-pair) scale/bias
                sF = s1p.tile([128, 1], F32, tag="sF")
                tF = s1p.tile([128, 1], F32, tag="tF")
                sC = s1p.tile([128, 1], F32, tag="sC")
                tC = s1p.tile([128, 1], F32, tag="tC")
                for half in range(2):
                    hp = slice(half * F, half * F + F)
                    nc.sync.dma_start(out=sF[hp, :], in_=s1c_[0:F, :])
                    nc.sync.dma_start(out=tF[hp, :], in_=t1c_[0:F, :])
                    nc.sync.dma_start(out=sC[hp, :], in_=s1c_[F:G, :])
                    nc.sync.dma_start(out=tC[hp, :], in_=t1c_[F:G, :])

                if not live((l, 4)):
                    break
                # ---- pass B: sigmoid*softplus, neighbor-sum, stats2 ----
                # K pairs per chunk share each ACT table load: all K sigmoids
                # run under sigmoid_and_others, then all K softplus under
                # softplus_and_others (2 reloads per chunk instead of 2/pair)
                tc.strict_bb_all_engine_barrier()
                KP = 6
                with tc.tile_pool(name="pb", bufs=KP + 2) as bp:
                    for pc in range(0, NPAIR, KP):
                        chunk = range(pc, min(pc + KP, NPAIR))
                        zfs, zcs, sgs, sps = {}, {}, {}, {}
                        for p in chunk:
                            t0, t1_ = 2 * p, min(2 * p + 1, NT - 1)
                            single = (2 * p + 1 > NT - 1)
                            zf = bp.tile([128, ET], F8, tag="zf")
                            zc = bp.tile([128, ET], F8, tag="zc")
                            nc.sync.dma_start(out=zf[0:F, :], in_=scr[t0, 0:F, :])
                            nc.sync.dma_start(out=zc[0:F, :], in_=scr[t0, F:G, :])
                            if not single:
                                nc.sync.dma_start(out=zf[F:G, :],
                                                  in_=scr[t1_, 0:F, :])
                                nc.sync.dma_start(out=zc[F:G, :],
                                                  in_=scr[t1_, F:G, :])
                            zfs[p], zcs[p] = zf, zc
                        for p in chunk:
                            single = (2 * p + 1 > NT - 1)
                            pp = 128 if not single else F
                            sg = bp.tile([128, ET], F16, tag="sg")
                            nc.scalar.activation(out=sg[0:pp, :],
                                                 in_=zfs[p][0:pp, :],
                                                 func=AF.Sigmoid,
                                                 bias=tF[0:pp, :],
                                                 scale=sF[0:pp, :])
                            sgs[p] = sg
                        for p in chunk:
                            single = (2 * p + 1 > NT - 1)
                            pp = 128 if not single else F
                            ec = bp.tile([128, ET], F32, tag="ec")
                            nc.scalar.activation(out=ec[0:pp, :],
                                                 in_=zcs[p][0:pp, :],
                                                 func=AF.Exp,
                                                 bias=tC[0:pp, :],
                                                 scale=sC[0:pp, :])
                            sp = bp.tile([128, ET], F16, tag="sp")
                            nc.scalar.activation(out=sp[0:pp, :],
                                                 in_=ec[0:pp, :],
                                                 func=AF.Ln, bias=1.0,
                                                 scale=1.0)
                            sps[p] = sp
                        for p in chunk:
                            t0, t1_ = 2 * p, min(2 * p + 1, NT - 1)
                            single = (2 * p + 1 > NT - 1)
                            pp = 128 if not single else F
                            sp = sps[p]
                            nc.vector.tensor_tensor(out=sp[0:pp, :],
                                                    in0=sp[0:pp, :],
                                                    in1=sgs[p][0:pp, :],
                                                    op=OP.mult)
                            # one full-width M-reduce for both pair halves
                            red2 = bp.tile([128, TA], F32, tag="red2")
                            nc.vector.tensor_reduce(
                                out=red2[0:pp, :],
                                in_=sp[0:pp, :].rearrange("p (a m) -> p a m",
                                                          m=M),
                                axis=mybir.AxisListType.X, op=OP.add)
                            halves = 1 if single else 2
                            for hh in range(halves):
                                tt = t0 if hh == 0 else t1_
                                hs = slice(hh * F, hh * F + F)
                                # aligned copy rides the DVE; the shifted one
                                # hits a DVE slow path, keep on scalar
                                if hh == 0:
                                    nc.vector.tensor_copy(
                                        out=summed[:, tt * TA:(tt + 1) * TA],
                                        in_=red2[hs, :])
                                else:
                                    nc.scalar.activation(
                                        out=summed[:, tt * TA:(tt + 1) * TA],
                                        in_=red2[hs, :], func=AF.Copy)
                                # stats over real atoms only
                                nreal = min(c["A_shard"] - tt * TA, TA)
                                col = 2 * p + hh
                                nc.vector.tensor_reduce(
                                    out=st2_s[:, col:col + 1],
                                    in_=red2[hs, 0:nreal],
                                    axis=mybir.AxisListType.X, op=OP.add)
                                sqt = bp.tile([F, TA], F32, tag="sqt")
                                nc.vector.tensor_tensor(
                                    out=sqt[:, 0:nreal], in0=red2[hs, 0:nreal],
                                    in1=red2[hs, 0:nreal], op=OP.mult)
                                nc.vector.tensor_reduce(
                                    out=st2_q[:, col:col + 1],
                                    in_=sqt[:, 0:nreal],
                                    axis=mybir.AxisListType.X, op=OP.add)

                if not live((l, 5)):
                    break
                # ---- stats2 AllReduce + affine2 + pass C ----
                ncols = NT
                pack2 = s1p.tile([F, 2], F32, tag="pack2")
                nc.vector.tensor_reduce(out=pack2[:, 0:1],
                                        in_=st2_s[:, 0:ncols],
                                        axis=mybir.AxisListType.X, op=OP.add)
                nc.vector.tensor_reduce(out=pack2[:, 1:2],
                                        in_=st2_q[:, 0:ncols],
                                        axis=mybir.AxisListType.X, op=OP.add)
                nc.sync.dma_start(out=ar2_in[:, :], in_=pack2[:])
                tc.strict_bb_all_engine_barrier()
                nc.gpsimd.collective_compute(
                    "AllReduce", OP.add, replica_groups=rg,
                    ins=[ar2_in[:, :]], outs=[ar2_out[:, :]])
                tc.strict_bb_all_engine_barrier()

                red2 = s1p.tile([F, 2], F32, tag="red2")
                nc.sync.dma_start(out=red2[:], in_=ar2_out[:, :])
                g2_t = s1p.tile([F, 1], F32, tag="g2t")
                nc.sync.dma_start(out=g2_t[:], in_=g2[l, :, :])
                be2_t = s1p.tile([F, 1], F32, tag="be2t")
                nc.sync.dma_start(out=be2_t[:], in_=be2[l, :, :])

                invN = 1.0 / float(N)
                mt2 = s1p.tile([F, 8], F32, tag="mt2")
                m2c = mt2[:, 0:1]
                nc.vector.tensor_scalar(out=m2c, in0=red2[:, 0:1], scalar1=invN,
                                        scalar2=None, op0=OP.mult)
                e2c = mt2[:, 1:2]
                nc.vector.tensor_scalar(out=e2c, in0=red2[:, 1:2], scalar1=invN,
                                        scalar2=None, op0=OP.mult)
                ms2 = mt2[:, 2:3]
                nc.vector.tensor_tensor(out=ms2, in0=m2c, in1=m2c, op=OP.mult)
                v2 = mt2[:, 3:4]
                nc.vector.tensor_tensor(out=v2, in0=e2c, in1=ms2, op=OP.subtract)
                lv2 = mt2[:, 4:5]
                nc.scalar.activation(out=lv2, in_=v2, func=AF.Ln,
                                     bias=eps_t[0:F, :], scale=1.0)
                r02 = mt2[:, 5:6]
                nc.scalar.activation(out=r02, in_=lv2, func=AF.Exp, scale=-0.5)
                vpe2 = mt2[:, 6:7]
                nc.vector.tensor_scalar(out=vpe2, in0=v2, scalar1=eps_t[0:F, :],
                                        scalar2=0.5, op0=OP.add, op1=OP.mult)
                r0q2 = mt2[:, 7:8]
                nc.vector.tensor_tensor(out=r0q2, in0=r02, in1=r02, op=OP.mult)
                nc.vector.tensor_tensor(out=r0q2, in0=r0q2, in1=vpe2, op=OP.mult)
                nc.vector.tensor_scalar(out=r0q2, in0=r0q2, scalar1=-1.0,
                                        scalar2=1.5, op0=OP.mult, op1=OP.add)
                r12 = mt2[:, 6:7]
                nc.vector.tensor_tensor(out=r12, in0=r02, in1=r0q2, op=OP.mult)
                s2c = s1p.tile([F, 1], F32, tag="s2c")
                nc.vector.tensor_tensor(out=s2c[:], in0=g2_t[:], in1=r12,
                                        op=OP.mult)
                t2c = s1p.tile([F, 1], F32, tag="t2c")
                nc.vector.tensor_tensor(out=t2c[:], in0=m2c, in1=s2c[:],
                                        op=OP.mult)
                nc.vector.scalar_tensor_tensor(out=t2c[:], in0=t2c[:],
                                               scalar=-1.0, in1=be2_t[:],
                                               op0=OP.mult, op1=OP.add)

                if not live((l, 6)):
                    break
                # pass C: x = softplus(x + s2*summed + t2) via ln(1+exp)
                with tc.tile_pool(name="pc", bufs=2) as pcp:
                    CW = 2048
                    for j in range(0, AP_, CW):
                        w = min(CW, AP_ - j)
                        pre = pcp.tile([F, CW], F32, tag="pre")
                        nc.vector.scalar_tensor_tensor(
                            out=pre[:, :w], in0=summed[:, j:j + w],
                            scalar=s2c[:, 0:1], in1=x_cm[:, j:j + w],
                            op0=OP.mult, op1=OP.add)
                        pex = pcp.tile([F, CW], F32, tag="pex")
                        nc.scalar.activation(out=pex[:, :w], in_=pre[:, :w],
                                             func=AF.Exp, bias=t2c[:], scale=1.0)
                        nc.scalar.activation(out=x_cm[:, j:j + w], in_=pex[:, :w],
                                             func=AF.Ln, bias=1.0, scale=1.0)
                if dbg_dump:
                    nc.gpsimd.dma_start(out=dbgsum[l, :, :], in_=summed[:, :])
                    nc.sync.dma_start(out=dbgx[l, :, :], in_=x_cm[:, :])

        for _ in range(1 if live((L, 0)) else 0):
            # ---------------- crystal pooling ----------------
            with tc.tile_pool(name="pool", bufs=4) as pp_, \
                 tc.tile_pool(name="poolacc", bufs=1, space="PSUM") as pacc, \
                 tc.tile_pool(name="poolps", bufs=2, space="PSUM") as pps:
                acc = pacc.tile([F, 512], F32, tag="pacc")
                for t in range(NT):
                    xps = pps.tile([TA, F], F16, tag="xps")
                    nc.tensor.transpose(out=xps[:], in_=x_cm[:, t * TA:(t + 1) * TA],
                                        identity=idh_t[0:F, 0:F])
                    xrm = pp_.tile([TA, F], F16, tag="xrm")
                    nc.scalar.activation(out=xrm[:], in_=xps[:], func=AF.Copy)
                    oht = pp_.tile([128, 512], F16, tag="oht")
                    nc.sync.dma_start(out=oht[:], in_=pone[t, :, :])
                    nc.tensor.matmul(acc[:], lhsT=xrm[:], rhs=oht[:],
                                     start=(t == 0), stop=(t == NT - 1))
                # zero pool_in, then scatter our window rows
                for r_ in range(NCB // 128):
                    nc.sync.dma_start(out=pool_in[128 * r_:128 * (r_ + 1), :],
                                      in_=zt128[:, 0:F])
                sci = pp_.tile([128, 32], I16, tag="sci")
                nc.sync.dma_start(out=sci[:], in_=scidx[:, :])
                tc.strict_bb_all_engine_barrier()
                accS = pp_.tile([F, 512], F32, tag="accS")
                nc.vector.tensor_copy(out=accS[:], in_=acc[:])
                asb4 = pp_.tile([128, 4, F], F32, tag="asb4")
                for ci in range(4):
                    # [F, 128] crystal block -> row-major [128, F]
                    tps = pps.tile([128, F], F32, tag="tps")
                    nc.tensor.transpose(out=tps[:],
                                        in_=accS[:, ci * 128:(ci + 1) * 128],
                                        identity=id_t[0:F, 0:F])
                    nc.vector.tensor_copy(out=asb4[:, ci, :], in_=tps[:])
                # one SWDGE scatter-add of the 512-crystal window (row i of
                # the window lives at asb4[i%128, i//128, :]); queue follows
                # the global Pool-DMA lane counter like the gathers
                nc.gpsimd.dma_scatter_add(
                    pool_in[:, :], asb4[:], sci[:, :], 512, 512, F,
                    single_packet=False, queue_num=gather_q())
                tc.strict_bb_all_engine_barrier()
                nc.gpsimd.collective_compute(
                    "AllReduce", OP.add, replica_groups=rg,
                    ins=[pool_in[:, :]], outs=[pool_out[:, :]])
                tc.strict_bb_all_engine_barrier()

            if dbg_dump:
                nc.sync.dma_start(out=dbgpool[:, :], in_=pool_out[:, :])


        for _ in range(1 if live((L, 1)) else 0):
            # ---------------- head ----------------
            with tc.tile_pool(name="head", bufs=3) as hp, \
                 tc.tile_pool(name="headps", bufs=2, space="PSUM") as hps:
                crys = nc.alloc_sbuf_tensor("crys", [F, NCB], F16)
                wfc_t = hp.tile([F, H], F16, tag="wfc")
                nc.sync.dma_start(out=wfc_t[:], in_=w_fc[:, :])
                bfc_t = hp.tile([H, 1], F32, tag="bfc")
                nc.sync.dma_start(out=bfc_t[:], in_=b_fc[:, :])
                wout_t = hp.tile([H, 1], F16, tag="wout")
                nc.sync.dma_start(out=wout_t[:], in_=w_out[:, :])
                bout_t = hp.tile([1, 1], F32, tag="bout")
                nc.sync.dma_start(out=bout_t[:], in_=b_out[:, :])

                for r_ in range(NCB // 128):
                    pt = hp.tile([128, F], F32, tag="pt")
                    nc.sync.dma_start(out=pt[:], in_=pool_out[128 * r_:128 * (r_ + 1), :])
                    ic = hp.tile([128, 1], F32, tag="ic")
                    nc.sync.dma_start(out=ic[:], in_=invcnt[128 * r_:128 * (r_ + 1), :])
                    nc.vector.tensor_scalar(out=pt[:], in0=pt[:], scalar1=ic[:, 0:1],
                                            scalar2=None, op0=OP.mult)
                    pex2 = hp.tile([128, F], F32, tag="pex2")
                    nc.scalar.activation(out=pex2[:], in_=pt[:], func=AF.Exp)
                    spt = hp.tile([128, F], F32, tag="spt")
                    nc.scalar.activation(out=spt[:], in_=pex2[:], func=AF.Ln,
                                         bias=1.0, scale=1.0)
                    tps = hps.tile([F, 128], F32, tag="tps")
                    nc.tensor.transpose(out=tps[:], in_=spt[:], identity=id_t[:, :])
                    nc.scalar.activation(out=crys[:, 128 * r_:128 * (r_ + 1)],
                                         in_=tps[:], func=AF.Copy)

                hc = nc.alloc_sbuf_tensor("hc", [H, NCB], F16)
                for j in range(0, NCB, 512):
                    psh = hps.tile([H, 512], F32, tag="psh")
                    nc.tensor.matmul(psh[:], lhsT=wfc_t[:], rhs=crys[:, j:j + 512],
                                     start=True, stop=True)
                    hex_ = hp.tile([H, 512], F32, tag="hex")
                    nc.scalar.activation(out=hex_[:], in_=psh[:],
                                         func=AF.Exp, bias=bfc_t[:], scale=1.0)
                    nc.scalar.activation(out=hc[:, j:j + 512], in_=hex_[:],
                                         func=AF.Ln, bias=1.0, scale=1.0)
                ofin = hp.tile([1, NCB], F32, tag="ofin")
                for j in range(0, NCB, 512):
                    pso = hps.tile([1, 512], F32, tag="pso")
                    nc.tensor.matmul(pso[:], lhsT=wout_t[:], rhs=hc[:, j:j + 512],
                                     start=True, stop=True)
                    nc.scalar.activation(out=ofin[:, j:j + 512], in_=pso[:],
                                         func=AF.Identity, bias=bout_t[:], scale=1.0)
                nc.sync.dma_start(out=out_t[:, :], in_=ofin[:])

    nc.compile()
    return nc


# --------------------------------------------------------------------------
# host-side input preparation
# --------------------------------------------------------------------------

def prepare_inputs(c, atom_fea, nbr_fea, nbr_fea_idx, crystal_atom_idx,
                   W_emb, b_emb, W_full, b_full, g1, be1, g2, be2,
                   W_fc, b_fc, W_out, b_out):
    N, M, F0, FB, F, H, NC, L = (c["N"], c["M"], c["F0"], c["FB"], c["F"],
                                 c["H"], c["NC"], c["NCONV"])
    G, TA, NT, AP_, ET, EL = (c["G"], c["TA"], c["ntile"], c["A_pad"],
                              c["E_tile"], c["E_loc"])
    W16, NCB, K = c["W16"], c["NCB"], c["NCORES"]
    AS = c["A_shard"]

    atom_fea = np.asarray(atom_fea, np.float32)
    nbr_fea = np.asarray(nbr_fea, np.float32)
    nbr_fea_idx = np.asarray(nbr_fea_idx, np.int64)
    crystal_atom_idx = np.asarray(crystal_atom_idx, np.int64)

    # shared (replicated) tensors
    oh = np.zeros((128, ET), ml_dtypes.float8_e4m3)
    for j in range(ET):
        oh[j // M, j] = 1.0
    shared = {
        "oh_self": oh,
        "w_emb": np.asarray(W_emb, np.float16),
        "b_emb": np.asarray(b_emb, np.float32).reshape(F, 1),
        "w_self": np.asarray(W_full[:, :F, :], np.float16),
        "w_nbr": np.asarray(W_full[:, F:2 * F, :], np.float16),
        "w_b": np.asarray(W_full[:, 2 * F:, :]).astype(ml_dtypes.float8_e4m3),
        "g1": np.asarray(g1, np.float32).reshape(L, G, 1),
        "be1": np.asarray(be1, np.float32).reshape(L, G, 1),
        "g2": np.asarray(g2, np.float32).reshape(L, F, 1),
        "be2": np.asarray(be2, np.float32).reshape(L, F, 1),
        "w_fc": np.asarray(W_fc, np.float16),
        "b_fc": np.asarray(b_fc, np.float32).reshape(H, 1),
        "w_out": np.asarray(W_out, np.float16),
        "b_out": np.asarray(b_out, np.float32).reshape(1, 1),
        "ident": np.eye(128, dtype=np.float32),
        "identh": np.eye(128, dtype=np.float16),
    }
    # crystal counts (global, from index data only)
    cnt = np.bincount(crystal_atom_idx, minlength=NC).astype(np.float32)
    icnt = np.zeros((NCB, 1), np.float32)
    icnt[:NC, 0] = 1.0 / np.maximum(cnt, 1.0)
    shared["invcnt"] = icnt

    # b_full is mathematically irrelevant (cancelled by training-mode BN)

    in_maps = []
    for k in range(K):
        a0 = k * AS
        af = np.zeros((F0, AP_), np.float16)
        af[:, :AS] = atom_fea[a0:a0 + AS].T
        # edge ordering: e = a*M + m within each tile of TA atoms
        gi = np.zeros((NT * TA, M), np.int64)
        gi_raw = nbr_fea_idx[a0:a0 + AS]
        gi[:AS] = gi_raw
        valid = np.zeros((NT * TA, M), bool)
        valid[:AS] = True
        jj = (gi // AS) * AP_ + (gi % AS)          # padded-global atom index
        rows = np.where(valid, 1 + jj // 2, 0).astype(np.int64)
        par = np.where(valid, jj & 1, 0).astype(np.int64)
        idxw = np.zeros((128, NT, W16), np.int16)
        mparr = np.zeros((128, NT, M), np.int8)
        j = np.arange(ET)
        for t in range(NT):
            fl = rows[t * TA:(t + 1) * TA].reshape(ET)
            wrap = np.zeros((16, W16), np.int16)
            wrap[j % 16, j // 16] = fl
            idxw[:, t, :] = np.tile(wrap, (8, 1))
            # ge[p, i, :] holds edge i*128+p -> mask[p, i]
            mparr[:, t, :] = par[t * TA:(t + 1) * TA].reshape(ET)[
                (np.arange(M)[None, :] * 128 + np.arange(128)[:, None])]
        nb = np.zeros((FB, EL), ml_dtypes.float8_e4m3)
        nb_l = nbr_fea[a0:a0 + AS].reshape(AS * M, FB)
        src = np.zeros((NT * TA * M, FB), np.float32)
        src[:AS * M] = nb_l
        # src is already in (a, m) order; tiles are contiguous runs of ET
        nb[:, :] = src.T.astype(ml_dtypes.float8_e4m3)

        cry = np.zeros(NT * TA, np.int64)
        cry[:AS] = crystal_atom_idx[a0:a0 + AS]
        cb = int(crystal_atom_idx[a0:a0 + AS].min())
        cmax = int(crystal_atom_idx[a0:a0 + AS].max())
        assert cmax - cb < 512, f"crystal window too wide: {cmax - cb}"
        # pool one-hot: [atom (partition), crystal-window col]
        pone = np.zeros((NT, TA, 512), np.float16)
        for t in range(NT):
            for a in range(TA):
                ga = t * TA + a
                if ga >= AS:
                    continue
                pone[t, a, int(cry[ga]) - cb] = 1.0
        # scatter-add indices: window row j -> crystal cb+j, wrapped in 16
        # partitions and replicated (SWDGE idx convention)
        sw = np.zeros((16, 32), np.int16)
        jj512 = np.arange(512)
        sw[jj512 % 16, jj512 // 16] = (cb + jj512).astype(np.int16)
        scidx = np.tile(sw, (8, 1))
        assert cb + 512 <= NCB

        in_maps.append(dict(shared,
                            afT=af, idxw=idxw, mparr=mparr, nbrT=nb, pone=pone,
                            scidx=scidx))
    return in_maps


# --------------------------------------------------------------------------
# public entry point
# --------------------------------------------------------------------------

_PROG_CACHE = {}


def _get_program(c):
    key = tuple(sorted((k, v) for k, v in c.items()))
    if key not in _PROG_CACHE:
        _PROG_CACHE[key] = build_program(c)
    return _PROG_CACHE[key]


def kernel(atom_fea, nbr_fea, nbr_fea_idx, crystal_atom_idx, W_emb, b_emb,
           W_full, b_full, g1, be1, g2, be2, W_fc, b_fc, W_out, b_out,
           _trace=False):
    from concourse import bass_utils
    c = CFG
    nc = _get_program(c)
    in_maps = prepare_inputs(c, atom_fea, nbr_fea, nbr_fea_idx,
                             crystal_atom_idx, W_emb, b_emb, W_full, b_full,
                             g1, be1, g2, be2, W_fc, b_fc, W_out, b_out)
    res = bass_utils.run_bass_kernel_spmd(
        nc, in_maps, core_ids=list(range(c["NCORES"])), trace=_trace)
    out = np.asarray(res.results[0]["out"], np.float32)
    ret = out[0, :c["NC"]].reshape(c["NC"], 1)
    if _trace:
        return ret, res
    return ret



# revision 33
# speedup vs baseline: 2.1834x; 1.0073x over previous
"""CrystalGraphConvNet forward pass as a distributed Bass/Tile kernel on 8 TRN2
NeuronCores.

Strategy (graph/data parallel, per sharding hint):
  - Atoms sharded contiguously across 8 cores (7500 each, padded to 7552).
  - Per conv layer, each core computes Y_self/Y_nbr for its shard; Y_nbr
    shards are AllGathered into a replicated fp8 table of PAIR rows (two
    atoms = 256 B per row + a zero guard row), so one int16-indexed
    dma_gather per tile fetches both parity candidates of every edge in a
    single 256 B packet.  Gathers rotate the 4 SWDGE queues with a GLOBAL
    counter (the Tile DMASW lane sems are queue-locked, so queue must track
    the lane cycle, not the per-layer tile index).  Pass A is paced by the
    DMA packet rate (~5 us per 1536-packet gather at 4 queues), so all other
    staged traffic is fp8: nbr features + W_b (double-fp8 matmul), the gated
    scratch, and the gather table itself.
  - A copy_predicated parity select + 12 accumulating fp8 transpose-matmuls
    fold the gathered rows into the channel-major PSUM accumulator, on top
    of the nbr_fea projection and the one-hot self term.
  - Training-mode batchnorm needs global stats: gated values stage to DRAM
    in fp8 while per-channel sum/sumsq accumulate (scalar copy/square with
    accum); a tiny AllReduce yields the affine.  Pass B batches K=4 tile
    pairs per ACT table load: all sigmoids run under sigmoid_and_others
    (with the BN affine folded into scale/bias), then softplus via exp+ln
    under the combined ln/exp table (this pwp build has no Softplus entry);
    DVE does the product, the M-reduce, and the summed-stats squares.
  - Crystal mean-pooling is one accumulating one-hot matmul per tile into a
    512-crystal window, written by a single SWDGE dma_scatter_add into a
    global crystal array and AllReduced; the MLP head runs on every core.
"""

import math
import os
import ml_dtypes
import numpy as np

import concourse.bass as bass
import concourse.bacc as bacc
import concourse.tile as tile
from concourse import mybir
from contextlib import ExitStack


def _single_act_table(orig):
    """Route exp/ln to the one ACT table set that contains BOTH (the default
    chooser splits them across exp_and_others/natural_log, thrashing table
    reloads on every exp<->ln transition in the batchnorm affines).  Copy/
    Square/Identity stay in EVERY set so pass A and the sigmoid/softplus
    batches never force a reload for them; Sigmoid and Softplus live in their
    own sets and are batch-switched in pass B."""
    AF = mybir.ActivationFunctionType
    both = {AF.Exp, AF.Ln}
    shared = {AF.Exp, AF.Ln}
    home = None
    for name, funcs in orig.items():
        if both <= funcs:
            home = name
            break
    if home is None:
        return orig
    out = {}
    for name, funcs in orig.items():
        out[name] = funcs if name == home else (funcs - shared)
    return out


_orig_get_activation_tables = bacc.get_activation_tables


def _patched_get_activation_tables(arch):
    return _single_act_table(_orig_get_activation_tables(arch))


bacc.get_activation_tables = _patched_get_activation_tables

F16 = mybir.dt.float16
F32 = mybir.dt.float32
F8 = mybir.dt.float8e4
I8 = mybir.dt.int8
I16 = mybir.dt.int16
I32 = mybir.dt.int32


def make_cfg(N=60000, M=12, F0=92, FB=41, F=64, H=128, NC=2000, NCONV=3,
             EPS=1e-5, NCORES=8, TA=128):
    c = dict(N=N, M=M, F0=F0, FB=FB, F=F, H=H, NC=NC, NCONV=NCONV, EPS=EPS,
             NCORES=NCORES, TA=TA)
    assert N % NCORES == 0
    c["A_shard"] = N // NCORES
    c["ntile"] = (c["A_shard"] + TA - 1) // TA
    c["A_pad"] = c["ntile"] * TA
    c["E_tile"] = TA * M
    c["E_loc"] = c["ntile"] * c["E_tile"]
    assert (NCORES * c["A_pad"]) % 2 == 0
    c["R2"] = 1 + NCORES * c["A_pad"] // 2      # pair-table rows (zero row at 0)
    assert c["R2"] <= 32768, "pair table must stay int16-addressable"
    c["W16"] = c["E_tile"] // 16
    c["NCB"] = 512 * ((NC + 511) // 512) + 512  # crystal bounce rows
    c["G"] = 2 * F                              # gated channels
    return c


CFG = make_cfg()


# --------------------------------------------------------------------------
# program builder
# --------------------------------------------------------------------------

def build_program(c, debug=False, dbg_dump=False, stop=None):
    # stop: optional (layer, stage) tuple for bisection; stages within a layer:
    # 0=Y, 1=AG, 2=A, 3=AR1, 4=B, 5=AR2, 6=C; (L,0)=pool, (L,1)=head
    nc = bacc.Bacc("TRN2", target_bir_lowering=False, debug=debug,
                   num_devices=c["NCORES"], num_swdge_queues=4)

    N, M, F0, FB, F, H, NC, L = (c["N"], c["M"], c["F0"], c["FB"], c["F"],
                                 c["H"], c["NC"], c["NCONV"])
    G, TA, NT, AP_, ET, EL = (c["G"], c["TA"], c["ntile"], c["A_pad"],
                              c["E_tile"], c["E_loc"])
    R2, W16, NCB, EPS = c["R2"], c["W16"], c["NCB"], c["EPS"]
    NPAIR = (NT + 1) // 2
    NCHUNK = ET // TA                           # 128-edge chunks per tile

    # ---------------- inputs ----------------
    afT = nc.dram_tensor("afT", [F0, AP_], F16, kind="ExternalInput")
    idxw = nc.dram_tensor("idxw", [128, NT, W16], I16, kind="ExternalInput")
    mparr = nc.dram_tensor("mparr", [128, NT, M], I8, kind="ExternalInput")
    nbrT = nc.dram_tensor("nbrT", [FB, EL], F8, kind="ExternalInput")
    oh_self = nc.dram_tensor("oh_self", [128, ET], F16, kind="ExternalInput")
    pone = nc.dram_tensor("pone", [NT, 128, 512], F16, kind="ExternalInput")
    scidx = nc.dram_tensor("scidx", [128, 32], I16, kind="ExternalInput")
    invcnt = nc.dram_tensor("invcnt", [NCB, 1], F32, kind="ExternalInput")
    w_emb = nc.dram_tensor("w_emb", [F0, F], F16, kind="ExternalInput")
    b_emb = nc.dram_tensor("b_emb", [F, 1], F32, kind="ExternalInput")
    w_self = nc.dram_tensor("w_self", [L, F, G], F16, kind="ExternalInput")
    w_nbr = nc.dram_tensor("w_nbr", [L, F, G], F16, kind="ExternalInput")
    w_b = nc.dram_tensor("w_b", [L, FB, G], F8, kind="ExternalInput")
    g1 = nc.dram_tensor("g1", [L, G, 1], F32, kind="ExternalInput")
    be1 = nc.dram_tensor("be1", [L, G, 1], F32, kind="ExternalInput")
    g2 = nc.dram_tensor("g2", [L, F, 1], F32, kind="ExternalInput")
    be2 = nc.dram_tensor("be2", [L, F, 1], F32, kind="ExternalInput")
    w_fc = nc.dram_tensor("w_fc", [F, H], F16, kind="ExternalInput")
    b_fc = nc.dram_tensor("b_fc", [H, 1], F32, kind="ExternalInput")
    w_out = nc.dram_tensor("w_out", [H, 1], F16, kind="ExternalInput")
    b_out = nc.dram_tensor("b_out", [1, 1], F32, kind="ExternalInput")
    ident = nc.dram_tensor("ident", [128, 128], F32, kind="ExternalInput")
    identh = nc.dram_tensor("identh", [128, 128], F16, kind="ExternalInput")

    out_t = nc.dram_tensor("out", [1, NCB], F32, kind="ExternalOutput")
    if dbg_dump:
        L_, F_, G_, AP2, EL_ = c["NCONV"], c["F"], c["G"], c["A_pad"], c["E_loc"]
        dbgx0 = nc.dram_tensor("dbgx0", [F_, AP2], F32, kind="ExternalOutput")
        dbgx = nc.dram_tensor("dbgx", [L_, F_, AP2], F32, kind="ExternalOutput")
        dbgsum = nc.dram_tensor("dbgsum", [L_, F_, AP2], F32, kind="ExternalOutput")
        dbgst1 = nc.dram_tensor("dbgst1", [L_, G_, 2], F32, kind="ExternalOutput")
        dbggat = nc.dram_tensor("dbggat", [L_, NT, 128, ET], F8,
                                kind="ExternalOutput")
        dbgpool = nc.dram_tensor("dbgpool", [NCB, F_], F32, kind="ExternalOutput")

    # ---------------- internal DRAM ----------------
    yb = nc.dram_tensor("yb", [AP_, G], F8)                        # AG input bounce
    tbl = nc.dram_tensor("tbl", [R2, 2 * G], F8, addr_space="Shared")
    scr = nc.dram_tensor("scr", [NT, 128, ET], F8)
    ar1_in = nc.dram_tensor("ar1_in", [G, 2], F32)
    ar1_out = nc.dram_tensor("ar1_out", [G, 2], F32, addr_space="Shared")
    ar2_in = nc.dram_tensor("ar2_in", [F, 2], F32)
    ar2_out = nc.dram_tensor("ar2_out", [F, 2], F32, addr_space="Shared")
    pool_in = nc.dram_tensor("pool_in", [NCB, F], F32)
    pool_out = nc.dram_tensor("pool_out", [NCB, F], F32, addr_space="Shared")

    rg = [list(range(c["NCORES"]))]
    AF = mybir.ActivationFunctionType
    OP = mybir.AluOpType
    if stop is None:
        stop = (L, 9)

    # Tile cycles DMASW lane sems per Pool DMA in GLOBAL program order (%8);
    # each lane sem is queue-locked, so the SWDGE queue must follow the same
    # global counter (%4), not the per-layer tile index.
    gq = [0]

    def gather_q():
        q = gq[0] % 4
        gq[0] += 1
        return q

    def live(key):
        return key <= stop

    with tile.TileContext(nc) as tc, ExitStack() as top:
        # Tile assigns each Pool-engine DMA inst a DMASW lane (round-robin in
        # program order) and points consumer waits at that lane's semaphore.
        # A prepare_only gather bakes its completion sem into the descriptors
        # (pass 2 attaches no then_inc), so sem= MUST be the very lane sem the
        # tick pass will pick, tracked here with a mirror counter.
        swsems = tc.sems.swdge_block()
        pool_dma_lane = [0]

        def prep_slot():
            """(lane sem, queue) for the next prepared Pool DMA.  queue =
            lane % nqueues keeps every lane sem on one fixed queue (SWDGE
            sems are queue-locked)."""
            j = pool_dma_lane[0]
            pool_dma_lane[0] += 1
            return swsems[j % len(swsems)], (j % len(swsems)) % 4

        # persistent SBUF state
        x_cm = nc.alloc_sbuf_tensor("x_cm", [F, AP_], F16)
        summed = nc.alloc_sbuf_tensor("summed", [F, AP_], F16)
        ysr = nc.alloc_sbuf_tensor("ysr", [128, NT, G], F16)      # Y_self row-major
        idx_all = nc.alloc_sbuf_tensor("idx_all", [128, NT, W16], I16)
        mpar = nc.alloc_sbuf_tensor("mpar", [128, NT, M, 1], I8)

        const = top.enter_context(tc.tile_pool(name="const", bufs=1))
        stats = top.enter_context(tc.tile_pool(name="stats", bufs=1))

        # constants resident all kernel
        ohs_t = const.tile([128, ET], F16)
        nc.sync.dma_start(out=ohs_t[:], in_=oh_self[:, :])
        wemb_t = const.tile([F0, F], F16)
        nc.sync.dma_start(out=wemb_t[:], in_=w_emb[:, :])
        bemb_t = const.tile([F, 1], F32)
        nc.sync.dma_start(out=bemb_t[:], in_=b_emb[:, :])
        id_t = const.tile([128, 128], F32)
        nc.sync.dma_start(out=id_t[:], in_=ident[:, :])
        idh_t = const.tile([128, 128], F16)
        nc.sync.dma_start(out=idh_t[:], in_=identh[:, :])
        idh8 = const.tile([128, 128], F8)
        nc.vector.tensor_copy(out=idh8[:], in_=idh_t[:])
        eps_t = const.tile([128, 1], F32)
        nc.vector.memset(eps_t[:], EPS)
        zrow = const.tile([1, 2 * G], F8)
        nc.vector.memset(zrow[:], 0.0)
        zt128 = const.tile([128, F], F32)
        nc.vector.memset(zt128[:], 0.0)

        # layer-invariant gather indices and parity masks
        nc.sync.dma_start(out=idx_all[:, :, :], in_=idxw[:, :, :])
        nc.sync.dma_start(out=mpar[:, :, :, 0], in_=mparr[:, :, :])

        wS = []
        wN = []
        wB = []
        for l in range(L):
            t1 = const.tile([F, G], F16, tag=f"wS{l}")
            nc.sync.dma_start(out=t1[:], in_=w_self[l, :, :])
            t2 = const.tile([F, G], F16, tag=f"wN{l}")
            nc.sync.dma_start(out=t2[:], in_=w_nbr[l, :, :])
            t3 = const.tile([FB, G], F8, tag=f"wB{l}")
            nc.sync.dma_start(out=t3[:], in_=w_b[l, :, :])
            wS.append(t1)
            wN.append(t2)
            wB.append(t3)

        # stats buffers
        st1_s = stats.tile([G, NT], F32, tag="st1s")
        st1_q = stats.tile([G, NT], F32, tag="st1q")
        st2_s = stats.tile([F, 2 * NPAIR], F32, tag="st2s")
        st2_q = stats.tile([F, 2 * NPAIR], F32, tag="st2q")

        # zero table guard row + summed pads
        nc.sync.dma_start(out=tbl[0:1, :], in_=zrow[:])
        nc.vector.memset(summed[:, :], 0.0)

        # ---------------- embedding: x = atom_fea @ W_emb + b_emb ----------
        with tc.tile_pool(name="emb", bufs=3) as embp, \
             tc.tile_pool(name="embps", bufs=2, space="PSUM") as embps:
            CH = 512
            for j in range(0, AP_, CH):
                w = min(CH, AP_ - j)
                rhs = embp.tile([F0, CH], F16, tag="embr")
                nc.sync.dma_start(out=rhs[:, :w], in_=afT[:, j:j + w])
                ps = embps.tile([F, CH], F32, tag="embp")
                nc.tensor.matmul(ps[:, :w], lhsT=wemb_t[:], rhs=rhs[:, :w],
                                 start=True, stop=True)
                nc.scalar.activation(out=x_cm[:, j:j + w], in_=ps[:, :w],
                                     func=AF.Identity, bias=bemb_t[:], scale=1.0)
        if dbg_dump:
            nc.sync.dma_start(out=dbgx0[:, :], in_=x_cm[:, :])

        # ---------------- conv layers ----------------
        for l in range(L):
            if not live((l, 0)):
                break
            # ---- phase Y: Y_self (SBUF) / Y_nbr (-> bounce -> AllGather) ----
            with tc.tile_pool(name="yph", bufs=3) as yp, \
                 tc.tile_pool(name="yps", bufs=2, space="PSUM") as yps:
                lastreal = c["A_shard"] - (NT - 1) * TA
                for t in range(NT):
                    xa = x_cm[:, t * TA:(t + 1) * TA]
                    psS = yps.tile([TA, G], F32, tag="psS")
                    nc.tensor.matmul(psS[:], lhsT=xa, rhs=wS[l][:],
                                     start=True, stop=True)
                    # pad atoms of the last tile must contribute exactly zero
                    # through the self one-hot matmul
                    nreal = TA if t < NT - 1 else lastreal
                    if nreal < TA:
                        nc.vector.memset(ysr[:, t, :], 0.0)
                    nc.scalar.activation(out=ysr[0:nreal, t, :],
                                         in_=psS[0:nreal, :], func=AF.Copy)
                    psN = yps.tile([TA, G], F32, tag="psN")
                    nc.tensor.matmul(psN[:], lhsT=xa, rhs=wN[l][:],
                                     start=True, stop=True)
                    yn = yp.tile([TA, G], F8, tag="yn")
                    nc.scalar.activation(out=yn[:], in_=psN[:], func=AF.Copy)
                    nc.sync.dma_start(out=yb[t * TA:(t + 1) * TA, :], in_=yn[:])

            if not live((l, 1)):
                break
            tc.strict_bb_all_engine_barrier()
            nc.gpsimd.collective_compute(
                "AllGather", OP.bypass, replica_groups=rg,
                ins=[yb[:, :]], outs=[tbl[1:R2, :]])
            tc.strict_bb_all_engine_barrier()

            if not live((l, 2)):
                break
            # ---- pass A: edges -> gated scratch + stats1 ----
            with tc.tile_pool(name="pa", bufs=8) as pa, \
                 tc.tile_pool(name="paps", bufs=2, space="PSUM") as paps:
                for t in range(NT):
                    nbt = pa.tile([FB, ET], F8, tag="nbt")
                    nc.sync.dma_start(out=nbt[:], in_=nbrT[:, t * ET:(t + 1) * ET])

                    ge = pa.tile([128, NCHUNK, 2 * G], F8, tag="ge")
                    # prepare_only decouples Q7 desc-gen from the transfer:
                    # the gather DMA fires at trigger time, so transfers on
                    # the 4 SWDGE queues overlap each other and compute
                    # instead of serializing on the Pool engine.
                    nc.gpsimd.dma_gather(ge[:], tbl[:, :], idx_all[:, t, :],
                                         ET, ET, 2 * G, single_packet=False,
                                         queue_num=gather_q())
                    # parity select: overwrite even-atom slab with odd-atom
                    # slab wherever the edge's target index is odd
                    nc.vector.copy_predicated(
                        out=ge[:, :, 0:G],
                        mask=mpar[:, t, :, :].broadcast_to([128, NCHUNK, G]),
                        data=ge[:, :, G:2 * G])

                    # every 128-col region: chunk transpose (start) -> wB ->
                    # one-hot self term (stop)
                    ps = paps.tile([G, ET], F32, tag="aps")
                    for i in range(NCHUNK):
                        cs = slice(i * 128, (i + 1) * 128)
                        nc.tensor.matmul(ps[:, cs], lhsT=ge[:, i, 0:G],
                                         rhs=idh8[:], start=(i % 4 == 0),
                                         stop=False)
                    for s in range(ET // 512):
                        sl = slice(s * 512, (s + 1) * 512)
                        nc.tensor.matmul(ps[:, sl], lhsT=wB[l][:], rhs=nbt[:, sl],
                                         start=False, stop=False)
                        nc.tensor.matmul(ps[:, sl], lhsT=ysr[:, t, :],
                                         rhs=ohs_t[:, sl], start=False, stop=True)

                    gat = pa.tile([128, ET], F8, tag="gat")
                    nc.scalar.activation(out=gat[:], in_=ps[:], func=AF.Copy,
                                         accum_out=st1_s[:, t:t + 1])
                    sqd = pa.tile([128, ET], F16, tag="sqd")
                    nc.scalar.activation(out=sqd[:], in_=ps[:], func=AF.Square,
                                         accum_out=st1_q[:, t:t + 1])
                    nc.sync.dma_start(out=scr[t, :, :], in_=gat[:])

            if not live((l, 3)):
                break
            # ---- stats1 reduce + AllReduce + affine ----
            with tc.tile_pool(name="s1", bufs=1) as s1p:
                pack1 = s1p.tile([G, 2], F32, tag="pack1")
                nc.vector.tensor_reduce(out=pack1[:, 0:1], in_=st1_s[:],
                                        axis=mybir.AxisListType.X, op=OP.add)
                nc.vector.tensor_reduce(out=pack1[:, 1:2], in_=st1_q[:],
                                        axis=mybir.AxisListType.X, op=OP.add)
                nc.sync.dma_start(out=ar1_in[:, :], in_=pack1[:])
                tc.strict_bb_all_engine_barrier()
                nc.gpsimd.collective_compute(
                    "AllReduce", OP.add, replica_groups=rg,
                    ins=[ar1_in[:, :]], outs=[ar1_out[:, :]])
                tc.strict_bb_all_engine_barrier()

                red1 = s1p.tile([G, 2], F32, tag="red1")
                nc.sync.dma_start(out=red1[:], in_=ar1_out[:, :])
                if dbg_dump:
                    nc.sync.dma_start(out=dbgst1[l, :, :], in_=red1[:])
                    for t in range(NT):
                        nc.gpsimd.dma_start(out=dbggat[l, t, :, :],
                                            in_=scr[t, :, :])
                g1_t = s1p.tile([G, 1], F32, tag="g1t")
                nc.sync.dma_start(out=g1_t[:], in_=g1[l, :, :])
                be1_t = s1p.tile([G, 1], F32, tag="be1t")
                nc.sync.dma_start(out=be1_t[:], in_=be1[l, :, :])

                # rsqrt(var+eps) = exp(-0.5*ln(var+eps)) + one Newton step
                # (no sqrt table needed; Ln/Exp share one ACT table set)
                invE = 1.0 / float(N * M)
                mmt = s1p.tile([G, 8], F32, tag="mmt")
                mcol = mmt[:, 0:1]
                nc.vector.tensor_scalar(out=mcol, in0=red1[:, 0:1], scalar1=invE,
                                        scalar2=None, op0=OP.mult)
                ex2 = mmt[:, 1:2]
                nc.vector.tensor_scalar(out=ex2, in0=red1[:, 1:2], scalar1=invE,
                                        scalar2=None, op0=OP.mult)
                msq = mmt[:, 2:3]
                nc.vector.tensor_tensor(out=msq, in0=mcol, in1=mcol, op=OP.mult)
                var = mmt[:, 3:4]
                nc.vector.tensor_tensor(out=var, in0=ex2, in1=msq, op=OP.subtract)
                lv = mmt[:, 4:5]
                nc.scalar.activation(out=lv, in_=var, func=AF.Ln,
                                     bias=eps_t[0:G, :], scale=1.0)
                r0 = mmt[:, 5:6]
                nc.scalar.activation(out=r0, in_=lv, func=AF.Exp, scale=-0.5)
                # one Newton step: r1 = r0*(1.5 - 0.5*(var+eps)*r0^2)
                vpe = mmt[:, 6:7]
                nc.vector.tensor_scalar(out=vpe, in0=var, scalar1=eps_t[0:G, :],
                                        scalar2=0.5, op0=OP.add, op1=OP.mult)
                r0q = mmt[:, 7:8]
                nc.vector.tensor_tensor(out=r0q, in0=r0, in1=r0, op=OP.mult)
                nc.vector.tensor_tensor(out=r0q, in0=r0q, in1=vpe, op=OP.mult)
                nc.vector.tensor_scalar(out=r0q, in0=r0q, scalar1=-1.0,
                                        scalar2=1.5, op0=OP.mult, op1=OP.add)
                r1 = mmt[:, 6:7]
                nc.vector.tensor_tensor(out=r1, in0=r0, in1=r0q, op=OP.mult)

                s1c_ = s1p.tile([G, 1], F32, tag="s1c")
                nc.vector.tensor_tensor(out=s1c_[:], in0=g1_t[:], in1=r1,
                                        op=OP.mult)
                t1c_ = s1p.tile([G, 1], F32, tag="t1c")
                nc.vector.tensor_tensor(out=t1c_[:], in0=mcol, in1=s1c_[:],
                                        op=OP.mult)
                nc.vector.scalar_tensor_tensor(out=t1c_[:], in0=t1c_[:],
                                               scalar=-1.0, in1=be1_t[:],
                                               op0=OP.mult, op1=OP.add)
                # replicated (packed-pair) scale/bias
                sF = s1p.tile([128, 1], F32, tag="sF")
                tF = s1p.tile([128, 1], F32, tag="tF")
                sC = s1p.tile([128, 1], F32, tag="sC")
                tC = s1p.tile([128, 1], F32, tag="tC")
                for half in range(2):
                    hp = slice(half * F, half * F + F)
                    nc.sync.dma_start(out=sF[hp, :], in_=s1c_[0:F, :])
                    nc.sync.dma_start(out=tF[hp, :], in_=t1c_[0:F, :])
                    nc.sync.dma_start(out=sC[hp, :], in_=s1c_[F:G, :])
                    nc.sync.dma_start(out=tC[hp, :], in_=t1c_[F:G, :])

                if not live((l, 4)):
                    break
                # ---- pass B: sigmoid*softplus, neighbor-sum, stats2 ----
                # K pairs per chunk share each ACT table load: all K sigmoids
                # run under sigmoid_and_others, then all K softplus under
                # softplus_and_others (2 reloads per chunk instead of 2/pair)
                tc.strict_bb_all_engine_barrier()
                KP = 6
                with tc.tile_pool(name="pb", bufs=KP + 2) as bp:
                    for pc in range(0, NPAIR, KP):
                        chunk = range(pc, min(pc + KP, NPAIR))
                        zfs, zcs, sgs, sps = {}, {}, {}, {}
                        for p in chunk:
                            t0, t1_ = 2 * p, min(2 * p + 1, NT - 1)
                            single = (2 * p + 1 > NT - 1)
                            zf = bp.tile([128, ET], F8, tag="zf")
                            zc = bp.tile([128, ET], F8, tag="zc")
                            nc.sync.dma_start(out=zf[0:F, :], in_=scr[t0, 0:F, :])
                            nc.sync.dma_start(out=zc[0:F, :], in_=scr[t0, F:G, :])
                            if not single:
                                nc.sync.dma_start(out=zf[F:G, :],
                                                  in_=scr[t1_, 0:F, :])
                                nc.sync.dma_start(out=zc[F:G, :],
                                                  in_=scr[t1_, F:G, :])
                            zfs[p], zcs[p] = zf, zc
                        for p in chunk:
                            single = (2 * p + 1 > NT - 1)
                            pp = 128 if not single else F
                            sg = bp.tile([128, ET], F16, tag="sg")
                            nc.scalar.activation(out=sg[0:pp, :],
                                                 in_=zfs[p][0:pp, :],
                                                 func=AF.Sigmoid,
                                                 bias=tF[0:pp, :],
                                                 scale=sF[0:pp, :])
                            sgs[p] = sg
                        for p in chunk:
                            single = (2 * p + 1 > NT - 1)
                            pp = 128 if not single else F
                            ec = bp.tile([128, ET], F32, tag="ec")
                            nc.scalar.activation(out=ec[0:pp, :],
                                                 in_=zcs[p][0:pp, :],
                                                 func=AF.Exp,
                                                 bias=tC[0:pp, :],
                                                 scale=sC[0:pp, :])
                            sp = bp.tile([128, ET], F16, tag="sp")
                            nc.scalar.activation(out=sp[0:pp, :],
                                                 in_=ec[0:pp, :],
                                                 func=AF.Ln, bias=1.0,
                                                 scale=1.0)
                            sps[p] = sp
                        for p in chunk:
                            t0, t1_ = 2 * p, min(2 * p + 1, NT - 1)
                            single = (2 * p + 1 > NT - 1)
                            pp = 128 if not single else F
                            sp = sps[p]
                            nc.vector.tensor_tensor(out=sp[0:pp, :],
                                                    in0=sp[0:pp, :],
                                                    in1=sgs[p][0:pp, :],
                                                    op=OP.mult)
                            # one full-width M-reduce for both pair halves
                            red2 = bp.tile([128, TA], F32, tag="red2")
                            nc.vector.tensor_reduce(
                                out=red2[0:pp, :],
                                in_=sp[0:pp, :].rearrange("p (a m) -> p a m",
                                                          m=M),
                                axis=mybir.AxisListType.X, op=OP.add)
                            halves = 1 if single else 2
                            for hh in range(halves):
                                tt = t0 if hh == 0 else t1_
                                hs = slice(hh * F, hh * F + F)
                                # aligned copy rides the DVE; the shifted one
                                # hits a DVE slow path, keep on scalar
                                if hh == 0:
                                    nc.vector.tensor_copy(
                                        out=summed[:, tt * TA:(tt + 1) * TA],
                                        in_=red2[hs, :])
                                else:
                                    nc.scalar.activation(
                                        out=summed[:, tt * TA:(tt + 1) * TA],
                                        in_=red2[hs, :], func=AF.Copy)
                                # stats over real atoms only
                                nreal = min(c["A_shard"] - tt * TA, TA)
                                col = 2 * p + hh
                                nc.vector.tensor_reduce(
                                    out=st2_s[:, col:col + 1],
                                    in_=red2[hs, 0:nreal],
                                    axis=mybir.AxisListType.X, op=OP.add)
                                sqt = bp.tile([F, TA], F32, tag="sqt")
                                nc.vector.tensor_tensor(
                                    out=sqt[:, 0:nreal], in0=red2[hs, 0:nreal],
                                    in1=red2[hs, 0:nreal], op=OP.mult)
                                nc.vector.tensor_reduce(
                                    out=st2_q[:, col:col + 1],
                                    in_=sqt[:, 0:nreal],
                                    axis=mybir.AxisListType.X, op=OP.add)

                if not live((l, 5)):
                    break
                # ---- stats2 AllReduce + affine2 + pass C ----
                ncols = NT
                pack2 = s1p.tile([F, 2], F32, tag="pack2")
                nc.vector.tensor_reduce(out=pack2[:, 0:1],
                                        in_=st2_s[:, 0:ncols],
                                        axis=mybir.AxisListType.X, op=OP.add)
                nc.vector.tensor_reduce(out=pack2[:, 1:2],
                                        in_=st2_q[:, 0:ncols],
                                        axis=mybir.AxisListType.X, op=OP.add)
                nc.sync.dma_start(out=ar2_in[:, :], in_=pack2[:])
                tc.strict_bb_all_engine_barrier()
                nc.gpsimd.collective_compute(
                    "AllReduce", OP.add, replica_groups=rg,
                    ins=[ar2_in[:, :]], outs=[ar2_out[:, :]])
                tc.strict_bb_all_engine_barrier()

                red2 = s1p.tile([F, 2], F32, tag="red2")
                nc.sync.dma_start(out=red2[:], in_=ar2_out[:, :])
                g2_t = s1p.tile([F, 1], F32, tag="g2t")
                nc.sync.dma_start(out=g2_t[:], in_=g2[l, :, :])
                be2_t = s1p.tile([F, 1], F32, tag="be2t")
                nc.sync.dma_start(out=be2_t[:], in_=be2[l, :, :])

                invN = 1.0 / float(N)
                mt2 = s1p.tile([F, 8], F32, tag="mt2")
                m2c = mt2[:, 0:1]
                nc.vector.tensor_scalar(out=m2c, in0=red2[:, 0:1], scalar1=invN,
                                        scalar2=None, op0=OP.mult)
                e2c = mt2[:, 1:2]
                nc.vector.tensor_scalar(out=e2c, in0=red2[:, 1:2], scalar1=invN,
                                        scalar2=None, op0=OP.mult)
                ms2 = mt2[:, 2:3]
                nc.vector.tensor_tensor(out=ms2, in0=m2c, in1=m2c, op=OP.mult)
                v2 = mt2[:, 3:4]
                nc.vector.tensor_tensor(out=v2, in0=e2c, in1=ms2, op=OP.subtract)
                lv2 = mt2[:, 4:5]
                nc.scalar.activation(out=lv2, in_=v2, func=AF.Ln,
                                     bias=eps_t[0:F, :], scale=1.0)
                r02 = mt2[:, 5:6]
                nc.scalar.activation(out=r02, in_=lv2, func=AF.Exp, scale=-0.5)
                vpe2 = mt2[:, 6:7]
                nc.vector.tensor_scalar(out=vpe2, in0=v2, scalar1=eps_t[0:F, :],
                                        scalar2=0.5, op0=OP.add, op1=OP.mult)
                r0q2 = mt2[:, 7:8]
                nc.vector.tensor_tensor(out=r0q2, in0=r02, in1=r02, op=OP.mult)
                nc.vector.tensor_tensor(out=r0q2, in0=r0q2, in1=vpe2, op=OP.mult)
                nc.vector.tensor_scalar(out=r0q2, in0=r0q2, scalar1=-1.0,
                                        scalar2=1.5, op0=OP.mult, op1=OP.add)
                r12 = mt2[:, 6:7]
                nc.vector.tensor_tensor(out=r12, in0=r02, in1=r0q2, op=OP.mult)
                s2c = s1p.tile([F, 1], F32, tag="s2c")
                nc.vector.tensor_tensor(out=s2c[:], in0=g2_t[:], in1=r12,
                                        op=OP.mult)
                t2c = s1p.tile([F, 1], F32, tag="t2c")
                nc.vector.tensor_tensor(out=t2c[:], in0=m2c, in1=s2c[:],
                                        op=OP.mult)
                nc.vector.scalar_tensor_tensor(out=t2c[:], in0=t2c[:],
                                               scalar=-1.0, in1=be2_t[:],
                                               op0=OP.mult, op1=OP.add)

                if not live((l, 6)):
                    break
                # pass C: x = softplus(x + s2*summed + t2) via ln(1+exp)
                with tc.tile_pool(name="pc", bufs=2) as pcp:
                    CW = 2048
                    for j in range(0, AP_, CW):
                        w = min(CW, AP_ - j)
                        pre = pcp.tile([F, CW], F32, tag="pre")
                        nc.vector.scalar_tensor_tensor(
                            out=pre[:, :w], in0=summed[:, j:j + w],
                            scalar=s2c[:, 0:1], in1=x_cm[:, j:j + w],
                            op0=OP.mult, op1=OP.add)
                        pex = pcp.tile([F, CW], F32, tag="pex")
                        nc.scalar.activation(out=pex[:, :w], in_=pre[:, :w],
                                             func=AF.Exp, bias=t2c[:], scale=1.0)
                        nc.scalar.activation(out=x_cm[:, j:j + w], in_=pex[:, :w],
                                             func=AF.Ln, bias=1.0, scale=1.0)
                if dbg_dump:
                    nc.gpsimd.dma_start(out=dbgsum[l, :, :], in_=summed[:, :])
                    nc.sync.dma_start(out=dbgx[l, :, :], in_=x_cm[:, :])

        for _ in range(1 if live((L, 0)) else 0):
            # ---------------- crystal pooling ----------------
            with tc.tile_pool(name="pool", bufs=4) as pp_, \
                 tc.tile_pool(name="poolacc", bufs=1, space="PSUM") as pacc, \
                 tc.tile_pool(name="poolps", bufs=2, space="PSUM") as pps:
                acc = pacc.tile([F, 512], F32, tag="pacc")
                for t in range(NT):
                    xps = pps.tile([TA, F], F16, tag="xps")
                    nc.tensor.transpose(out=xps[:], in_=x_cm[:, t * TA:(t + 1) * TA],
                                        identity=idh_t[0:F, 0:F])
                    xrm = pp_.tile([TA, F], F16, tag="xrm")
                    nc.scalar.activation(out=xrm[:], in_=xps[:], func=AF.Copy)
                    oht = pp_.tile([128, 512], F16, tag="oht")
                    nc.sync.dma_start(out=oht[:], in_=pone[t, :, :])
                    nc.tensor.matmul(acc[:], lhsT=xrm[:], rhs=oht[:],
                                     start=(t == 0), stop=(t == NT - 1))
                # zero pool_in, then scatter our window rows
                for r_ in range(NCB // 128):
                    nc.sync.dma_start(out=pool_in[128 * r_:128 * (r_ + 1), :],
                                      in_=zt128[:, 0:F])
                sci = pp_.tile([128, 32], I16, tag="sci")
                nc.sync.dma_start(out=sci[:], in_=scidx[:, :])
                tc.strict_bb_all_engine_barrier()
                accS = pp_.tile([F, 512], F32, tag="accS")
                nc.vector.tensor_copy(out=accS[:], in_=acc[:])
                asb4 = pp_.tile([128, 4, F], F32, tag="asb4")
                for ci in range(4):
                    # [F, 128] crystal block -> row-major [128, F]
                    tps = pps.tile([128, F], F32, tag="tps")
                    nc.tensor.transpose(out=tps[:],
                                        in_=accS[:, ci * 128:(ci + 1) * 128],
                                        identity=id_t[0:F, 0:F])
                    nc.vector.tensor_copy(out=asb4[:, ci, :], in_=tps[:])
                # one SWDGE scatter-add of the 512-crystal window (row i of
                # the window lives at asb4[i%128, i//128, :]); queue follows
                # the global Pool-DMA lane counter like the gathers
                nc.gpsimd.dma_scatter_add(
                    pool_in[:, :], asb4[:], sci[:, :], 512, 512, F,
                    single_packet=False, queue_num=gather_q())
                tc.strict_bb_all_engine_barrier()
                nc.gpsimd.collective_compute(
                    "AllReduce", OP.add, replica_groups=rg,
                    ins=[pool_in[:, :]], outs=[pool_out[:, :]])
                tc.strict_bb_all_engine_barrier()

            if dbg_dump:
                nc.sync.dma_start(out=dbgpool[:, :], in_=pool_out[:, :])


        for _ in range(1 if live((L, 1)) else 0):
            # ---------------- head ----------------
            with tc.tile_pool(name="head", bufs=3) as hp, \
                 tc.tile_pool(name="headps", bufs=2, space="PSUM") as hps:
                crys = nc.alloc_sbuf_tensor("crys", [F, NCB], F16)
                wfc_t = hp.tile([F, H], F16, tag="wfc")
                nc.sync.dma_start(out=wfc_t[:], in_=w_fc[:, :])
                bfc_t = hp.tile([H, 1], F32, tag="bfc")
                nc.sync.dma_start(out=bfc_t[:], in_=b_fc[:, :])
                wout_t = hp.tile([H, 1], F16, tag="wout")
                nc.sync.dma_start(out=wout_t[:], in_=w_out[:, :])
                bout_t = hp.tile([1, 1], F32, tag="bout")
                nc.sync.dma_start(out=bout_t[:], in_=b_out[:, :])

                for r_ in range(NCB // 128):
                    pt = hp.tile([128, F], F32, tag="pt")
                    nc.sync.dma_start(out=pt[:], in_=pool_out[128 * r_:128 * (r_ + 1), :])
                    ic = hp.tile([128, 1], F32, tag="ic")
                    nc.sync.dma_start(out=ic[:], in_=invcnt[128 * r_:128 * (r_ + 1), :])
                    nc.vector.tensor_scalar(out=pt[:], in0=pt[:], scalar1=ic[:, 0:1],
                                            scalar2=None, op0=OP.mult)
                    pex2 = hp.tile([128, F], F32, tag="pex2")
                    nc.scalar.activation(out=pex2[:], in_=pt[:], func=AF.Exp)
                    spt = hp.tile([128, F], F32, tag="spt")
                    nc.scalar.activation(out=spt[:], in_=pex2[:], func=AF.Ln,
                                         bias=1.0, scale=1.0)
                    tps = hps.tile([F, 128], F32, tag="tps")
                    nc.tensor.transpose(out=tps[:], in_=spt[:], identity=id_t[:, :])
                    nc.scalar.activation(out=crys[:, 128 * r_:128 * (r_ + 1)],
                                         in_=tps[:], func=AF.Copy)

                hc = nc.alloc_sbuf_tensor("hc", [H, NCB], F16)
                for j in range(0, NCB, 512):
                    psh = hps.tile([H, 512], F32, tag="psh")
                    nc.tensor.matmul(psh[:], lhsT=wfc_t[:], rhs=crys[:, j:j + 512],
                                     start=True, stop=True)
                    hex_ = hp.tile([H, 512], F32, tag="hex")
                    nc.scalar.activation(out=hex_[:], in_=psh[:],
                                         func=AF.Exp, bias=bfc_t[:], scale=1.0)
                    nc.scalar.activation(out=hc[:, j:j + 512], in_=hex_[:],
                                         func=AF.Ln, bias=1.0, scale=1.0)
                ofin = hp.tile([1, NCB], F32, tag="ofin")
                for j in range(0, NCB, 512):
                    pso = hps.tile([1, 512], F32, tag="pso")
                    nc.tensor.matmul(pso[:], lhsT=wout_t[:], rhs=hc[:, j:j + 512],
                                     start=True, stop=True)
                    nc.scalar.activation(out=ofin[:, j:j + 512], in_=pso[:],
                                         func=AF.Identity, bias=bout_t[:], scale=1.0)
                nc.sync.dma_start(out=out_t[:, :], in_=ofin[:])

    nc.compile()
    return nc


# --------------------------------------------------------------------------
# host-side input preparation
# --------------------------------------------------------------------------

def prepare_inputs(c, atom_fea, nbr_fea, nbr_fea_idx, crystal_atom_idx,
                   W_emb, b_emb, W_full, b_full, g1, be1, g2, be2,
                   W_fc, b_fc, W_out, b_out):
    N, M, F0, FB, F, H, NC, L = (c["N"], c["M"], c["F0"], c["FB"], c["F"],
                                 c["H"], c["NC"], c["NCONV"])
    G, TA, NT, AP_, ET, EL = (c["G"], c["TA"], c["ntile"], c["A_pad"],
                              c["E_tile"], c["E_loc"])
    W16, NCB, K = c["W16"], c["NCB"], c["NCORES"]
    AS = c["A_shard"]

    atom_fea = np.asarray(atom_fea, np.float32)
    nbr_fea = np.asarray(nbr_fea, np.float32)
    nbr_fea_idx = np.asarray(nbr_fea_idx, np.int64)
    crystal_atom_idx = np.asarray(crystal_atom_idx, np.int64)

    # shared (replicated) tensors
    oh = np.zeros((128, ET), np.float16)
    for j in range(ET):
        oh[j // M, j] = 1.0
    shared = {
        "oh_self": oh,
        "w_emb": np.asarray(W_emb, np.float16),
        "b_emb": np.asarray(b_emb, np.float32).reshape(F, 1),
        "w_self": np.asarray(W_full[:, :F, :], np.float16),
        "w_nbr": np.asarray(W_full[:, F:2 * F, :], np.float16),
        "w_b": np.asarray(W_full[:, 2 * F:, :]).astype(ml_dtypes.float8_e4m3),
        "g1": np.asarray(g1, np.float32).reshape(L, G, 1),
        "be1": np.asarray(be1, np.float32).reshape(L, G, 1),
        "g2": np.asarray(g2, np.float32).reshape(L, F, 1),
        "be2": np.asarray(be2, np.float32).reshape(L, F, 1),
        "w_fc": np.asarray(W_fc, np.float16),
        "b_fc": np.asarray(b_fc, np.float32).reshape(H, 1),
        "w_out": np.asarray(W_out, np.float16),
        "b_out": np.asarray(b_out, np.float32).reshape(1, 1),
        "ident": np.eye(128, dtype=np.float32),
        "identh": np.eye(128, dtype=np.float16),
    }
    # crystal counts (global, from index data only)
    cnt = np.bincount(crystal_atom_idx, minlength=NC).astype(np.float32)
    icnt = np.zeros((NCB, 1), np.float32)
    icnt[:NC, 0] = 1.0 / np.maximum(cnt, 1.0)
    shared["invcnt"] = icnt

    # b_full is mathematically irrelevant (cancelled by training-mode BN)

    in_maps = []
    for k in range(K):
        a0 = k * AS
        af = np.zeros((F0, AP_), np.float16)
        af[:, :AS] = atom_fea[a0:a0 + AS].T
        # edge ordering: e = a*M + m within each tile of TA atoms
        gi = np.zeros((NT * TA, M), np.int64)
        gi_raw = nbr_fea_idx[a0:a0 + AS]
        gi[:AS] = gi_raw
        valid = np.zeros((NT * TA, M), bool)
        valid[:AS] = True
        jj = (gi // AS) * AP_ + (gi % AS)          # padded-global atom index
        rows = np.where(valid, 1 + jj // 2, 0).astype(np.int64)
        par = np.where(valid, jj & 1, 0).astype(np.int64)
        idxw = np.zeros((128, NT, W16), np.int16)
        mparr = np.zeros((128, NT, M), np.int8)
        j = np.arange(ET)
        for t in range(NT):
            fl = rows[t * TA:(t + 1) * TA].reshape(ET)
            wrap = np.zeros((16, W16), np.int16)
            wrap[j % 16, j // 16] = fl
            idxw[:, t, :] = np.tile(wrap, (8, 1))
            # ge[p, i, :] holds edge i*128+p -> mask[p, i]
            mparr[:, t, :] = par[t * TA:(t + 1) * TA].reshape(ET)[
                (np.arange(M)[None, :] * 128 + np.arange(128)[:, None])]
        nb = np.zeros((FB, EL), ml_dtypes.float8_e4m3)
        nb_l = nbr_fea[a0:a0 + AS].reshape(AS * M, FB)
        src = np.zeros((NT * TA * M, FB), np.float32)
        src[:AS * M] = nb_l
        # src is already in (a, m) order; tiles are contiguous runs of ET
        nb[:, :] = src.T.astype(ml_dtypes.float8_e4m3)

        cry = np.zeros(NT * TA, np.int64)
        cry[:AS] = crystal_atom_idx[a0:a0 + AS]
        cb = int(crystal_atom_idx[a0:a0 + AS].min())
        cmax = int(crystal_atom_idx[a0:a0 + AS].max())
        assert cmax - cb < 512, f"crystal window too wide: {cmax - cb}"
        # pool one-hot: [atom (partition), crystal-window col]
        pone = np.zeros((NT, TA, 512), np.float16)
        for t in range(NT):
            for a in range(TA):
                ga = t * TA + a
                if ga >= AS:
                    continue
                pone[t, a, int(cry[ga]) - cb] = 1.0
        # scatter-add indices: window row j -> crystal cb+j, wrapped in 16
        # partitions and replicated (SWDGE idx convention)
        sw = np.zeros((16, 32), np.int16)
        jj512 = np.arange(512)
        sw[jj512 % 16, jj512 // 16] = (cb + jj512).astype(np.int16)
        scidx = np.tile(sw, (8, 1))
        assert cb + 512 <= NCB

        in_maps.append(dict(shared,
                            afT=af, idxw=idxw, mparr=mparr, nbrT=nb, pone=pone,
                            scidx=scidx))
    return in_maps


# --------------------------------------------------------------------------
# public entry point
# --------------------------------------------------------------------------

_PROG_CACHE = {}


def _get_program(c):
    key = tuple(sorted((k, v) for k, v in c.items()))
    if key not in _PROG_CACHE:
        _PROG_CACHE[key] = build_program(c)
    return _PROG_CACHE[key]


def kernel(atom_fea, nbr_fea, nbr_fea_idx, crystal_atom_idx, W_emb, b_emb,
           W_full, b_full, g1, be1, g2, be2, W_fc, b_fc, W_out, b_out,
           _trace=False):
    from concourse import bass_utils
    c = CFG
    nc = _get_program(c)
    in_maps = prepare_inputs(c, atom_fea, nbr_fea, nbr_fea_idx,
                             crystal_atom_idx, W_emb, b_emb, W_full, b_full,
                             g1, be1, g2, be2, W_fc, b_fc, W_out, b_out)
    res = bass_utils.run_bass_kernel_spmd(
        nc, in_maps, core_ids=list(range(c["NCORES"])), trace=_trace)
    out = np.asarray(res.results[0]["out"], np.float32)
    ret = out[0, :c["NC"]].reshape(c["NC"], 1)
    if _trace:
        return ret, res
    return ret

